# revision 1
# baseline (speedup 1.0000x reference)
"""DIEN (GRU + attention + AUGRU) Trainium2 kernel.

Data-parallel over 8 NeuronCores: each core handles a batch slice of 64.
All on-chip state is feature-major (feature dim on SBUF partitions, batch
on the free dim), so the two sequential recurrences need no per-step
transposes.

Structure:
  - X^T (D, T*64) in bf16, built via PE transposes of natural-layout DMA
    tiles; the same stationary tiles also accumulate the masked history
    sum (f32) via N=1 matmuls against 1/len-scaled mask columns.
  - GRU input projections land in PSUM chunks laid out (128, 4, [r|z|n]);
    per-step recurrent matmuls accumulate onto the same PSUM slice, so
    gi + gh needs no add, and r|z adjacency gives one merged sigmoid.
  - Attention runs per 8-step chunk interleaved with the GRU loop (keeps
    PE warm, hides the whole phase); layer-2 is per-step (40,64)^T @ w2
    matmuls landing scores batch-major for the softmax.
  - AUGRU consumes attention weights broadcast across partitions via
    ones(1,128)^T @ w_row matmuls (bf16). Masked steps have weight 0
    (u'=0, h unchanged), so the final AUGRU state IS the gathered
    aug_out[b, len-1] — no gather needed.
  - Matmuls are bf16 (f32 PSUM accumulation); the masked history mean
    and softmax stay f32.
"""

import sys

if "/opt/trn_rl_repo" not in sys.path:
    sys.path.insert(0, "/opt/trn_rl_repo")

from contextlib import ExitStack

import ml_dtypes
import numpy as np

import concourse.bacc as bacc
import concourse.bass as bass
import concourse.mybir as mybir
import concourse.tile as tile
from concourse.bass_utils import run_bass_kernel_spmd
from concourse.masks import make_identity

f32 = mybir.dt.float32
bf16 = mybir.dt.bfloat16
i32 = mybir.dt.int32
AF = mybir.ActivationFunctionType
ALU = mybir.AluOpType
BF = ml_dtypes.bfloat16

NCORES = 8
B, T, D = 512, 200, 128
BL = B // NCORES          # 64 batch rows per core
CG = 4                    # GRU/AUGRU psum chunk: steps per chunk
NCG = T // CG             # 50
WG = CG * BL              # 256
CH = 8                    # attention chunk: steps per chunk
NCHUNK = T // CH          # 25
W = CH * BL               # 512
G3 = 3 * D
HB = BL // 2              # recurrence batch half-group (chain interleaving)


def _mm(nc, out, lhsT, rhs, start, stop, skip=False):
    nc.tensor.matmul(out, lhsT, rhs, start=start, stop=stop,
                     skip_group_check=skip)


def build_nc():
    nc = bacc.Bacc("TRN2", target_bir_lowering=False)

    hist = nc.declare_dram_parameter("hist", [BL, T, D], bf16, isOutput=False)
    item = nc.declare_dram_parameter("item", [BL, D], f32, isOutput=False)
    user = nc.declare_dram_parameter("user", [BL, D], f32, isOutput=False)
    maskd = nc.declare_dram_parameter("maskd", [BL, T], f32, isOutput=False)
    seqd = nc.declare_dram_parameter("seqd", [BL, 1], i32, isOutput=False)
    WihTd = nc.declare_dram_parameter("WihT", [D, G3], bf16, isOutput=False)
    WhhTd = nc.declare_dram_parameter("WhhT", [D, G3], bf16, isOutput=False)
    bihTd = nc.declare_dram_parameter("bihT", [D, 3], f32, isOutput=False)
    bhhTd = nc.declare_dram_parameter("bhhT", [D, 3], f32, isOutput=False)
    W0Td = nc.declare_dram_parameter("W0T", [D, 320], bf16, isOutput=False)
    b0d = nc.declare_dram_parameter("b0", [80, 1], f32, isOutput=False)
    W1Td = nc.declare_dram_parameter("W1T", [80, 40], bf16, isOutput=False)
    b1d = nc.declare_dram_parameter("b1", [40, 1], f32, isOutput=False)
    W2Td = nc.declare_dram_parameter("W2T", [40, 1], bf16, isOutput=False)
    b2d = nc.declare_dram_parameter("b2", [1, 1], f32, isOutput=False)
    augWd = nc.declare_dram_parameter("augW", [D, 6 * D], bf16, isOutput=False)
    augbd = nc.declare_dram_parameter("augb", [D, 3], f32, isOutput=False)
    outWTd = nc.declare_dram_parameter("outWT", [D, 5], bf16, isOutput=False)
    outbd = nc.declare_dram_parameter("outb", [1, 1], f32, isOutput=False)
    outd = nc.declare_dram_parameter("out", [1, BL], f32, isOutput=True)

    with tile.TileContext(nc) as tc, ExitStack() as ctx:
        big = ctx.enter_context(tc.tile_pool(name="big", bufs=1))
        wp = ctx.enter_context(tc.tile_pool(name="wp", bufs=1))
        pp = ctx.enter_context(tc.tile_pool(name="pp", bufs=1))
        hp = ctx.enter_context(tc.tile_pool(name="hp", bufs=4))

        XT = big.tile([D, T * BL], bf16)
        gruT = big.tile([D, T * BL], bf16)

        ident = pp.tile([128, 128], f32)
        make_identity(nc, ident)
        identB = pp.tile([BL, BL], bf16)
        make_identity(nc, identB)
        identB128 = pp.tile([128, 128], bf16)
        make_identity(nc, identB128)

        # ------------- weights (bf16 arrive pre-converted via DMA) -------
        WihT = wp.tile([D, G3], bf16)
        nc.sync.dma_start(WihT[:, :], WihTd[:, :])
        WhhT = wp.tile([D, G3], bf16)
        nc.sync.dma_start(WhhT[:, :], WhhTd[:, :])
        W0T = wp.tile([D, 320], bf16)
        nc.sync.dma_start(W0T[:, :], W0Td[:, :])
        W1T = wp.tile([80, 40], bf16)
        nc.sync.dma_start(W1T[:, :], W1Td[:, :])
        W2T = wp.tile([40, 1], bf16)
        nc.sync.dma_start(W2T[:, :], W2Td[:, :])
        augW = wp.tile([D, 6 * D], bf16)
        nc.sync.dma_start(augW[:, :], augWd[:, :])
        outWT = wp.tile([D, 5], bf16)
        nc.sync.dma_start(outWT[:, :], outWTd[:, :])
        WrhT, WrxT = augW[:, 0:D], augW[:, D:2 * D]
        WuhT, WuxT = augW[:, 2 * D:3 * D], augW[:, 3 * D:4 * D]
        WahT, WaxT = augW[:, 4 * D:5 * D], augW[:, 5 * D:6 * D]

        bihT = wp.tile([D, 3], f32)
        nc.sync.dma_start(bihT[:, :], bihTd[:, :])
        bhhT = wp.tile([D, 3], f32)
        nc.sync.dma_start(bhhT[:, :], bhhTd[:, :])
        brz = wp.tile([D, 2], f32)
        nc.vector.tensor_add(brz[:, :], bihT[:, 0:2], bhhT[:, 0:2])
        b_hn, b_in = bhhT[:, 2:3], bihT[:, 2:3]

        b0v = wp.tile([80, 1], f32)
        nc.sync.dma_start(b0v[:, :], b0d[:, :])
        b1v = wp.tile([40, 1], f32)
        nc.sync.dma_start(b1v[:, :], b1d[:, :])
        b2v = wp.tile([1, 1], f32)
        nc.sync.dma_start(b2v[:, :], b2d[:, :])
        augb = wp.tile([D, 3], f32)
        nc.sync.dma_start(augb[:, :], augbd[:, :])
        outb = wp.tile([1, 1], f32)
        nc.sync.dma_start(outb[:, :], outbd[:, :])

        ones_f = wp.tile([1, 128], f32)
        nc.vector.memset(ones_f[:, :], 1.0)
        ones1 = wp.tile([1, 128], bf16)
        nc.vector.memset(ones1[:, :], 1.0)

        # per-gate bias blocks broadcast to 64 columns each: folded into the
        # PSUM gate chunks so the merged r|z sigmoid needs no ACT bias.
        def bias_block(name, cols):
            blk = wp.tile([D, 128], f32, name=name)
            for i, col in enumerate(cols):
                nc.vector.tensor_scalar(blk[:, i * BL:(i + 1) * BL],
                                        ident[:, 0:BL], 0.0, col,
                                        ALU.mult, ALU.add)
            return blk

        brz2 = bias_block("brz2", [brz[:, 0:1], brz[:, 1:2]])
        bru2 = bias_block("bru2", [augb[:, 0:1], augb[:, 1:2]])
        ab_h = augb[:, 2:3]

        # ---------------- preprocessing --------------------------------
        histT = pp.tile([D, BL], bf16)
        qT = pp.tile([D, BL], bf16)
        userT = pp.tile([D, BL], bf16)
        qRep = pp.tile([D, W], bf16)
        maskS = pp.tile([BL, T], f32)
        b2col = pp.tile([BL, 1], f32)
        histF = pp.tile([D, BL], f32)

        with tc.tile_pool(name="pre", bufs=4) as pre, \
             tc.tile_pool(name="pps", bufs=2, space="PSUM") as pps, \
             tc.tile_pool(name="hps", bufs=1, space="PSUM") as hps:
            nc.sync.dma_start(maskS[:, :], maskd[:, :])
            seqi = pre.tile([BL, 1], i32)
            nc.sync.dma_start(seqi[:, :], seqd[:, :])
            seqf = pre.tile([BL, 1], f32)
            nc.vector.tensor_copy(seqf[:, :], seqi[:, :])
            rsec = pre.tile([BL, 1], f32)
            nc.vector.reciprocal(rsec[:, :], seqf[:, :])
            maskSc = pre.tile([BL, T], f32)
            nc.vector.tensor_scalar_mul(maskSc[:, :], maskS[:, :], rsec[:, 0:1])

            mstp0 = pps.tile([128, BL], f32, tag="mstp")
            nc.tensor.transpose(mstp0[:, :], maskSc[:, 0:128], ident[0:BL, 0:BL])
            mst0 = pre.tile([128, BL], bf16, tag="mst0", bufs=1)
            nc.scalar.copy(mst0[:, :], mstp0[:, :])
            mstp1 = pps.tile([128, BL], f32, tag="mstp")
            nc.tensor.transpose(mstp1[0:72, :], maskSc[:, 128:200], ident[0:BL, 0:BL])
            mst1 = pre.tile([72, BL], bf16, tag="mst1", bufs=1)
            nc.scalar.copy(mst1[:, :], mstp1[0:72, :])

            itn = pre.tile([BL, D], f32, tag="itn")
            nc.sync.dma_start(itn[:, :], item[:, :])
            itp = pps.tile([D, BL], f32, tag="small_t")
            nc.tensor.transpose(itp[:, :], itn[:, :], ident[0:BL, 0:BL])
            nc.scalar.copy(qT[:, :], itp[:, :])
            usn = pre.tile([BL, D], f32, tag="itn")
            nc.sync.dma_start(usn[:, :], user[:, :])
            usp = pps.tile([D, BL], f32, tag="small_t")
            nc.tensor.transpose(usp[:, :], usn[:, :], ident[0:BL, 0:BL])
            nc.scalar.copy(userT[:, :], usp[:, :])
            b2p = pps.tile([BL, 1], f32, tag="small_t")
            _mm(nc, b2p[:, :], ones_f[0:1, 0:BL], b2v, start=True, stop=True)
            nc.scalar.copy(b2col[:, :], b2p[:, :])
            for s in range(CH):
                nc.scalar.copy(qRep[:, s * BL:(s + 1) * BL], qT[:, :])

            # X^T build + masked history sum. First-half (t<128) tiles for
            # ALL b come first: GRU chunk 0 only depends on those, so the
            # second half and the history matmuls overlap the running GRU.
            histp0 = hps.tile([D, BL], f32, tag="histp0")
            histp1 = hps.tile([D, BL], f32, tag="histp1")
            XT3 = XT.rearrange("p (t b) -> p t b", b=BL)
            for h, (t0, tn) in enumerate(((0, 128), (128, 72))):
                histp = histp0 if h == 0 else histp1
                for b in range(BL):
                    xn = pre.tile([128, D], bf16, tag="xn")
                    nc.sync.dma_start(xn[0:tn, :], hist[b, t0:t0 + tn, :])
                    xtp = pps.tile([D, 128], bf16, tag="xtp")
                    nc.tensor.transpose(xtp[:, 0:tn], xn[0:tn, :],
                                        identB128[0:tn, 0:tn])
                    dst = XT3[:, t0:t0 + tn, b]
                    if b % 2 == 0:
                        nc.scalar.copy(dst, xtp[:, 0:tn])
                    else:
                        nc.vector.tensor_copy(dst, xtp[:, 0:tn])
                    mcol = (mst0 if h == 0 else mst1)[0:tn, b:b + 1]
                    _mm(nc, histp[:, b:b + 1], xn[0:tn, :], mcol,
                        start=True, stop=True)
            nc.scalar.copy(histF[:, :], histp0[:, :])
            nc.vector.tensor_add(histF[:, :], histF[:, :], histp1[:, :])
            nc.scalar.copy(histT[:, :], histF[:, :])

        # ---------------- GRU + interleaved attention -------------------
        gruT3 = gruT.rearrange("p (t b) -> p t b", b=BL)
        SIG, TANH = AF.Sigmoid, AF.Tanh
        wgt = pp.tile([BL, T], f32)

        with tc.tile_pool(name="gip", bufs=2, space="PSUM") as gip, \
             tc.tile_pool(name="hnp", bufs=1, space="PSUM") as hnp, \
             tc.tile_pool(name="gt", bufs=6) as gt, \
             tc.tile_pool(name="aps", bufs=1, space="PSUM") as aps, \
             tc.tile_pool(name="sps", bufs=1, space="PSUM") as sps, \
             tc.tile_pool(name="at", bufs=3) as at:
            scp = sps.tile([BL, T], f32)

            def attention_chunk(c):
                gc = gruT[:, c * W:(c + 1) * W]
                r2 = at.tile([D, W], bf16, tag="r2")
                nc.vector.tensor_mul(r2[:, :], gc, qRep[:, :])
                r3 = at.tile([D, W], bf16, tag="r3")
                nc.vector.tensor_sub(r3[:, :], qRep[:, :], gc)
                y0p = aps.tile([80, W], f32, tag="y0")
                _mm(nc, y0p[:, :], W0T[:, 0:80], gc, start=True, stop=False)
                _mm(nc, y0p[:, :], W0T[:, 80:160], qRep, start=False, stop=False)
                _mm(nc, y0p[:, :], W0T[:, 160:240], r2, start=False, stop=False)
                _mm(nc, y0p[:, :], W0T[:, 240:320], r3, start=False, stop=True)
                y0 = at.tile([80, W], bf16, tag="y0s")
                nc.vector.tensor_scalar(y0[:, :], y0p[:, :], b0v[:, 0:1], 0.0,
                                        ALU.add, ALU.max)
                y1p = aps.tile([40, W], f32, tag="y1")
                _mm(nc, y1p[:, :], W1T, y0, start=True, stop=True)
                y1 = at.tile([40, W], bf16, tag="y1s")
                nc.vector.tensor_scalar(y1[:, :], y1p[:, :], b1v[:, 0:1], 0.0,
                                        ALU.add, ALU.max)
                for s in range(CH):
                    t = c * CH + s
                    _mm(nc, scp[:, t:t + 1], y1[:, s * BL:(s + 1) * BL],
                        W2T, start=True, stop=True)

            for c in range(NCG):
                XTc = XT[:, c * WG:(c + 1) * WG]
                # gate-major psum layout: [r: CG*64 | z: CG*64] in one bank
                prz = gip.tile([D, CG * 128], f32, tag="girz")
                prz4 = prz.rearrange("p (g s c) -> p g s c", g=2, c=BL)
                pn = gip.tile([D, CG * BL], f32, tag="gin")
                pn3 = pn.rearrange("p (s g) -> p s g", g=BL)
                _mm(nc, prz[:, 0:WG], WihT[:, 0:D], XTc,
                    start=True, stop=True)
                _mm(nc, prz[:, WG:2 * WG], WihT[:, D:2 * D], XTc,
                    start=True, stop=True)
                _mm(nc, pn[:, :], WihT[:, 2 * D:G3], XTc,
                    start=True, stop=True)
                nc.vector.tensor_add(
                    prz4[:, :, :, :], prz4[:, :, :, :],
                    brz2[:, :].rearrange("p (g c) -> p g c", g=2)
                    .unsqueeze(2).broadcast_to([D, 2, CG, BL]))
                for s in range(CG):
                    t = c * CG + s
                    if t > 0:
                        hprev = gruT3[:, t - 1, :]
                        _mm(nc, prz[:, s * BL:(s + 1) * BL], WhhT[:, 0:D],
                            hprev, start=False, stop=True, skip=True)
                        _mm(nc, prz[:, WG + s * BL:WG + (s + 1) * BL],
                            WhhT[:, D:2 * D], hprev,
                            start=False, stop=True, skip=True)
                        hn = hnp.tile([D, BL], f32, tag="hn")
                        _mm(nc, hn[:, :], WhhT[:, 2 * D:G3], hprev,
                            start=True, stop=True)
                    rz = gt.tile([D, 2 * BL], f32, tag="rz")
                    nc.scalar.activation(
                        rz[:, :].rearrange("p (g c) -> p g c", g=2),
                        prz4[:, :, s, :], SIG)
                    r, z = rz[:, 0:BL], rz[:, BL:2 * BL]
                    tmp2 = gt.tile([D, BL], f32, tag="tmp2")
                    if t > 0:
                        tmp = gt.tile([D, BL], f32, tag="tmp")
                        nc.vector.scalar_tensor_tensor(
                            tmp[:, :], hn[:, :], b_hn, r, ALU.add, ALU.mult)
                        nc.vector.tensor_add(tmp2[:, :], tmp[:, :],
                                             pn3[:, s, :])
                    else:
                        tmp = gt.tile([D, BL], f32, tag="tmp")
                        nc.vector.tensor_scalar_mul(tmp[:, :], r, b_hn)
                        nc.vector.tensor_add(tmp2[:, :], tmp[:, :],
                                             pn3[:, s, :])
                    n = gt.tile([D, BL], f32, tag="n")
                    nc.scalar.activation(n[:, :], tmp2[:, :], TANH, bias=b_in)
                    hdst = gruT3[:, t, :]
                    if t > 0:
                        d = gt.tile([D, BL], f32, tag="d")
                        nc.vector.tensor_sub(d[:, :], hprev, n[:, :])
                        zd = gt.tile([D, BL], f32, tag="zd")
                        nc.vector.tensor_mul(zd[:, :], z, d[:, :])
                        nc.vector.tensor_add(hdst, n[:, :], zd[:, :])
                    else:
                        zn = gt.tile([D, BL], f32, tag="zd")
                        nc.vector.tensor_mul(zn[:, :], z, n[:, :])
                        nc.vector.tensor_sub(hdst, n[:, :], zn[:, :])
                if c % 2 == 1:
                    attention_chunk(c // 2)

            # masked softmax over t (b-major); last MLP layer ReLU'd w/ b2
            rawr = at.tile([BL, T], f32, tag="rawr")
            nc.scalar.activation(rawr[:, :], scp[:, :], AF.Relu,
                                 bias=b2col[:, 0:1])
            rawm = at.tile([BL, T], f32, tag="rawm")
            nc.vector.tensor_mul(rawm[:, :], rawr[:, :], maskS[:, :])
            mxn = at.tile([BL, 1], f32, tag="mxn")
            nc.vector.tensor_reduce(mxn[:, :], rawm[:, :],
                                    axis=mybir.AxisListType.X,
                                    op=ALU.max, negate=True)
            ex = at.tile([BL, T], f32, tag="ex")
            nc.scalar.activation(ex[:, :], rawr[:, :], AF.Exp, bias=mxn[:, 0:1])
            em = at.tile([BL, T], f32, tag="em")
            nc.vector.tensor_mul(em[:, :], ex[:, :], maskS[:, :])
            sm = at.tile([BL, 1], f32, tag="sm")
            nc.vector.tensor_reduce(sm[:, :], em[:, :],
                                    axis=mybir.AxisListType.X, op=ALU.add)
            rs = at.tile([BL, 1], f32, tag="rs")
            nc.vector.reciprocal(rs[:, :], sm[:, :])
            nc.vector.tensor_scalar_mul(wgt[:, :], em[:, :], rs[:, 0:1])

        # attention weights go to partition-0 bf16 rows per AUGRU chunk
        # (the row transposes + broadcasts overlap the AUGRU itself)
        wgt_bf = pp.tile([BL, T], bf16)
        nc.scalar.copy(wgt_bf[:, :], wgt[:, :])

        # ---------------- AUGRU ----------------------------------------
        hA = [None]
        with tc.tile_pool(name="axp", bufs=2, space="PSUM") as axp, \
             tc.tile_pool(name="abp", bufs=2, space="PSUM") as abp, \
             tc.tile_pool(name="rtp", bufs=2, space="PSUM") as rtp, \
             tc.tile_pool(name="ut", bufs=6) as ut:
            for c in range(NCG):
                gc = gruT[:, c * WG:(c + 1) * WG]
                rowp = rtp.tile([1, WG], f32, tag="rowp")
                for s in range(CG):
                    t = c * CG + s
                    _mm(nc, rowp[0:1, s * BL:(s + 1) * BL], wgt_bf[:, t:t + 1],
                        identB, start=True, stop=True)
                rows = ut.tile([1, WG], bf16, tag="rows")
                nc.vector.tensor_copy(rows[0:1, :], rowp[0:1, :])
                pru = axp.tile([D, CG * 128], f32, tag="pxru")
                pru4 = pru.rearrange("p (g s c) -> p g s c", g=2, c=BL)
                pxh = axp.tile([D, CG * BL], f32, tag="pxh")
                pxh3 = pxh.rearrange("p (s g) -> p s g", g=BL)
                _mm(nc, pru[:, 0:WG], WrxT, gc, start=True, stop=True)
                _mm(nc, pru[:, WG:2 * WG], WuxT, gc, start=True, stop=True)
                _mm(nc, pxh[:, :], WaxT, gc, start=True, stop=True)
                nc.vector.tensor_add(
                    pru4[:, :, :, :], pru4[:, :, :, :],
                    bru2[:, :].rearrange("p (g c) -> p g c", g=2)
                    .unsqueeze(2).broadcast_to([D, 2, CG, BL]))
                pab = abp.tile([D, WG], f32, tag="pab")
                for s in range(CG):
                    _mm(nc, pab[:, s * BL:(s + 1) * BL], ones1,
                        rows[0:1, s * BL:(s + 1) * BL], start=True, stop=True)
                for s in range(CG):
                    t = c * CG + s
                    hAg = hA[0]
                    if t > 0:
                        _mm(nc, pru[:, s * BL:(s + 1) * BL], WrhT, hAg,
                            start=False, stop=True, skip=True)
                        _mm(nc, pru[:, WG + s * BL:WG + (s + 1) * BL],
                            WuhT, hAg, start=False, stop=True, skip=True)
                    ru = ut.tile([D, 2 * BL], f32, tag="ru")
                    nc.scalar.activation(
                        ru[:, :].rearrange("p (g c) -> p g c", g=2),
                        pru4[:, :, s, :], SIG)
                    r, u = ru[:, 0:BL], ru[:, BL:2 * BL]
                    if t > 0:
                        hr = ut.tile([D, BL], bf16, tag="ahr")
                        nc.vector.tensor_mul(hr[:, :], hAg, r)
                        _mm(nc, pxh3[:, s, :], WahT, hr,
                            start=False, stop=True, skip=True)
                    hh = ut.tile([D, BL], f32, tag="ahh")
                    nc.scalar.activation(hh[:, :], pxh3[:, s, :], TANH,
                                         bias=ab_h)
                    up = ut.tile([D, BL], f32, tag="aup")
                    nc.vector.tensor_mul(up[:, :], pab[:, s * BL:(s + 1) * BL],
                                         u)
                    hnew = hp.tile([D, BL], bf16, tag="hA")
                    if t > 0:
                        dd = ut.tile([D, BL], f32, tag="add")
                        nc.vector.tensor_sub(dd[:, :], hh[:, :], hAg)
                        ud = ut.tile([D, BL], f32, tag="aud")
                        nc.vector.tensor_mul(ud[:, :], up[:, :], dd[:, :])
                        nc.vector.tensor_add(hnew[:, :], hAg, ud[:, :])
                    else:
                        nc.vector.tensor_mul(hnew[:, :], up[:, :], hh[:, :])
                    hA[0] = hnew

        # ---------------- output layer ---------------------------------
        with tc.tile_pool(name="ops", bufs=1, space="PSUM") as ops, \
             tc.tile_pool(name="ot", bufs=1) as ot:
            ih = ot.tile([D, BL], bf16)
            nc.vector.tensor_mul(ih[:, :], qT[:, :], histF[:, :])
            po = ops.tile([1, BL], f32)
            pieces = [userT, qT, histT, ih, hA[0]]
            for g, piece in enumerate(pieces):
                _mm(nc, po[:, :], outWT[:, g:g + 1], piece,
                    start=(g == 0), stop=(g == 4))
            outs = ot.tile([1, BL], f32)
            nc.scalar.activation(outs[:, :], po[:, :], AF.Identity,
                                 bias=outb[:, 0:1])
            nc.sync.dma_start(outd[:, :], outs[:, :])

    nc.finalize()
    return nc


_NC = None


def _get_nc():
    global _NC
    if _NC is None:
        _NC = build_nc()
    return _NC


def make_in_maps(inputs):
    """Slice full inputs into per-core input maps (host-side layout only)."""
    f = {k: np.asarray(v) for k, v in inputs.items()}
    WihT = np.ascontiguousarray(f["gru_Wih"].T)          # (128, 384)
    WhhT = np.ascontiguousarray(f["gru_Whh"].T)
    bihT = np.ascontiguousarray(f["gru_bih"].reshape(3, D).T)  # (128, 3)
    bhhT = np.ascontiguousarray(f["gru_bhh"].reshape(3, D).T)
    W0T = np.ascontiguousarray(
        f["attn_W0"].T.reshape(4, D, 80).transpose(1, 0, 2).reshape(D, 320))
    b0 = np.ascontiguousarray(f["attn_b0"].reshape(80, 1))
    W1T = np.ascontiguousarray(f["attn_W1"].T)           # (80, 40)
    b1 = np.ascontiguousarray(f["attn_b1"].reshape(40, 1))
    W2T = np.ascontiguousarray(f["attn_W2"].T)           # (40, 1)
    b2 = f["attn_b2"].reshape(1, 1)
    augW = np.concatenate(
        [np.ascontiguousarray(f[k][:, p * D:(p + 1) * D].T)
         for k in ("aug_Wr", "aug_Wu", "aug_Wh") for p in (0, 1)],
        axis=1)                                          # (128, 768)
    augb = np.stack([f["aug_br"], f["aug_bu"], f["aug_bh"]], axis=1)  # (128,3)
    outWT = np.ascontiguousarray(f["out_W"].reshape(5, D).T)          # (128,5)
    outb = f["out_b"].reshape(1, 1)

    shared_bf = dict(WihT=WihT, WhhT=WhhT, W0T=W0T, W1T=W1T, W2T=W2T,
                     augW=augW, outWT=outWT)
    shared = dict(bihT=bihT, bhhT=bhhT, b0=b0, b1=b1, b2=b2, augb=augb,
                  outb=outb)
    shared = {k: np.ascontiguousarray(v.astype(np.float32)) for k, v in
              shared.items()}
    shared.update({k: np.ascontiguousarray(v.astype(BF)) for k, v in
                   shared_bf.items()})

    in_maps = []
    for c in range(NCORES):
        s = slice(c * BL, (c + 1) * BL)
        m = dict(shared)
        m["hist"] = np.ascontiguousarray(
            f["item_historical_embedding"][s].astype(BF))
        m["item"] = np.ascontiguousarray(f["item_embedding"][s].astype(np.float32))
        m["user"] = np.ascontiguousarray(f["user_embedding"][s].astype(np.float32))
        m["maskd"] = np.ascontiguousarray(f["mask"][s].astype(np.float32))
        m["seqd"] = np.ascontiguousarray(
            f["sequential_length"][s].reshape(BL, 1).astype(np.int32))
        in_maps.append(m)
    return in_maps


def kernel(**inputs) -> np.ndarray:
    nc = _get_nc()
    in_maps = make_in_maps(inputs)
    res = run_bass_kernel_spmd(nc, in_maps, list(range(NCORES)))
    return np.concatenate(
        [np.asarray(res.results[c]["out"]).reshape(BL) for c in range(NCORES)])



# revision 5
# speedup vs baseline: 1.1105x; 1.1105x over previous
"""DIEN (GRU + attention + AUGRU) Trainium2 kernel.

Data-parallel over 8 NeuronCores: each core handles a batch slice of 64.
All on-chip state is feature-major (feature dim on SBUF partitions, batch
on the free dim), so the two sequential recurrences need no per-step
transposes.

Structure:
  - X^T (D, T*64) in bf16, built via PE transposes of natural-layout DMA
    tiles; the same stationary tiles also accumulate the masked history
    sum (f32) via N=1 matmuls against 1/len-scaled mask columns.
  - GRU input projections land in PSUM chunks laid out (128, 4, [r|z|n]).
    Gate sigmoids are split per-gate so the gate bias rides the ACT
    instruction (no DVE bias adds), and (1-z) comes from sigmoid(-x).
    The combine uses h' = (1-z)*n + z*h with z*h computed off the
    critical path.
  - Attention runs per 8-step chunk interleaved with the GRU loop; its
    elementwise prep (fact*q, q-fact) runs on the idle GPSIMD engine so
    the DVE stays free for the recurrence chain.
  - AUGRU consumes attention weights via rows pre-transposed once after
    the softmax; per chunk a single ones-weight load broadcasts 4 rows
    into PSUM. Masked steps have weight 0 (u'=0, h unchanged), so the
    final AUGRU state IS the gathered aug_out[b, len-1] - no gather.
  - Matmuls are bf16 (f32 PSUM accumulation); elementwise recurrence ops
    run bf16 where both operands sit in SBUF (2x DVE mode); the masked
    history mean and softmax stay f32.
"""

import sys

if "/opt/trn_rl_repo" not in sys.path:
    sys.path.insert(0, "/opt/trn_rl_repo")

from contextlib import ExitStack

import ml_dtypes
import numpy as np

import concourse.bacc as bacc
import concourse.bass as bass
import concourse.mybir as mybir
import concourse.tile as tile
from concourse.bass_utils import run_bass_kernel_spmd
from concourse.masks import make_identity

f32 = mybir.dt.float32
bf16 = mybir.dt.bfloat16
i32 = mybir.dt.int32
AF = mybir.ActivationFunctionType
ALU = mybir.AluOpType
BF = ml_dtypes.bfloat16

NCORES = 8
B, T, D = 512, 200, 128
BL = B // NCORES          # 64 batch rows per core
CG = 4                    # GRU/AUGRU psum chunk: steps per chunk
NCG = T // CG             # 50
WG = CG * BL              # 256
CH = 8                    # attention chunk: steps per chunk
NCHUNK = T // CH          # 25
W = CH * BL               # 512
G3 = 3 * D


def _mm(nc, out, lhsT, rhs, start, stop, skip=False):
    nc.tensor.matmul(out, lhsT, rhs, start=start, stop=stop,
                     skip_group_check=skip)


def build_nc():
    nc = bacc.Bacc("TRN2", target_bir_lowering=False)

    hist = nc.declare_dram_parameter("hist", [BL, T, D], bf16, isOutput=False)
    item = nc.declare_dram_parameter("item", [BL, D], f32, isOutput=False)
    user = nc.declare_dram_parameter("user", [BL, D], f32, isOutput=False)
    maskd = nc.declare_dram_parameter("maskd", [BL, T], f32, isOutput=False)
    seqd = nc.declare_dram_parameter("seqd", [BL, 1], i32, isOutput=False)
    WihTd = nc.declare_dram_parameter("WihT", [D, G3], bf16, isOutput=False)
    WhhTd = nc.declare_dram_parameter("WhhT", [D, G3], bf16, isOutput=False)
    bihTd = nc.declare_dram_parameter("bihT", [D, 3], f32, isOutput=False)
    bhhTd = nc.declare_dram_parameter("bhhT", [D, 3], f32, isOutput=False)
    W0Td = nc.declare_dram_parameter("W0T", [D, 320], bf16, isOutput=False)
    b0d = nc.declare_dram_parameter("b0", [80, 1], f32, isOutput=False)
    W1Td = nc.declare_dram_parameter("W1T", [80, 40], bf16, isOutput=False)
    b1d = nc.declare_dram_parameter("b1", [40, 1], f32, isOutput=False)
    W2Td = nc.declare_dram_parameter("W2T", [40, 1], bf16, isOutput=False)
    b2d = nc.declare_dram_parameter("b2", [1, 1], f32, isOutput=False)
    augWd = nc.declare_dram_parameter("augW", [D, 6 * D], bf16, isOutput=False)
    augbd = nc.declare_dram_parameter("augb", [D, 3], f32, isOutput=False)
    outWTd = nc.declare_dram_parameter("outWT", [D, 5], bf16, isOutput=False)
    outbd = nc.declare_dram_parameter("outb", [1, 1], f32, isOutput=False)
    outd = nc.declare_dram_parameter("out", [1, BL], f32, isOutput=True)

    with tile.TileContext(nc) as tc, ExitStack() as ctx:
        big = ctx.enter_context(tc.tile_pool(name="big", bufs=1))
        wp = ctx.enter_context(tc.tile_pool(name="wp", bufs=1))
        pp = ctx.enter_context(tc.tile_pool(name="pp", bufs=1))
        hp = ctx.enter_context(tc.tile_pool(name="hp", bufs=4))

        XT = big.tile([D, T * BL], bf16)
        gruT = big.tile([D, T * BL], bf16)

        ident = pp.tile([128, 128], f32)
        make_identity(nc, ident)
        identB = pp.tile([BL, BL], bf16)
        make_identity(nc, identB)
        identB128 = pp.tile([128, 128], bf16)
        make_identity(nc, identB128)

        # ------------- weights (bf16 arrive pre-converted via DMA) -------
        WihT = wp.tile([D, G3], bf16)
        nc.sync.dma_start(WihT[:, :], WihTd[:, :])
        WhhT = wp.tile([D, G3], bf16)
        nc.sync.dma_start(WhhT[:, :], WhhTd[:, :])
        W0T = wp.tile([D, 320], bf16)
        nc.sync.dma_start(W0T[:, :], W0Td[:, :])
        W1T = wp.tile([80, 40], bf16)
        nc.sync.dma_start(W1T[:, :], W1Td[:, :])
        W2T = wp.tile([40, 1], bf16)
        nc.sync.dma_start(W2T[:, :], W2Td[:, :])
        augW = wp.tile([D, 6 * D], bf16)
        nc.sync.dma_start(augW[:, :], augWd[:, :])
        outWT = wp.tile([D, 5], bf16)
        nc.sync.dma_start(outWT[:, :], outWTd[:, :])
        WrhT, WrxT = augW[:, 0:D], augW[:, D:2 * D]
        WuhT, WuxT = augW[:, 2 * D:3 * D], augW[:, 3 * D:4 * D]
        WahT, WaxT = augW[:, 4 * D:5 * D], augW[:, 5 * D:6 * D]

        bihT = wp.tile([D, 3], f32)
        nc.sync.dma_start(bihT[:, :], bihTd[:, :])
        bhhT = wp.tile([D, 3], f32)
        nc.sync.dma_start(bhhT[:, :], bhhTd[:, :])
        brz = wp.tile([D, 2], f32)
        nc.vector.tensor_add(brz[:, :], bihT[:, 0:2], bhhT[:, 0:2])
        nbz = wp.tile([D, 1], f32)
        nc.vector.tensor_scalar(nbz[:, :], brz[:, 1:2], -1.0, 0.0,
                                ALU.mult, ALU.add)
        b_hn, b_in = bhhT[:, 2:3], bihT[:, 2:3]

        b0v = wp.tile([80, 1], f32)
        nc.sync.dma_start(b0v[:, :], b0d[:, :])
        b1v = wp.tile([40, 1], f32)
        nc.sync.dma_start(b1v[:, :], b1d[:, :])
        b2v = wp.tile([1, 1], f32)
        nc.sync.dma_start(b2v[:, :], b2d[:, :])
        augb = wp.tile([D, 3], f32)
        nc.sync.dma_start(augb[:, :], augbd[:, :])
        outb = wp.tile([1, 1], f32)
        nc.sync.dma_start(outb[:, :], outbd[:, :])
        ab_r, ab_u, ab_h = augb[:, 0:1], augb[:, 1:2], augb[:, 2:3]

        ones_f = wp.tile([1, 128], f32)
        nc.vector.memset(ones_f[:, :], 1.0)
        ones1 = wp.tile([1, 128], bf16)
        nc.vector.memset(ones1[:, :], 1.0)

        # ---------------- preprocessing --------------------------------
        histT = pp.tile([D, BL], bf16)
        qT = pp.tile([D, BL], bf16)
        userT = pp.tile([D, BL], bf16)
        qRep = pp.tile([D, W], bf16)
        maskS = pp.tile([BL, T], f32)
        b2col = pp.tile([BL, 1], f32)
        histF = pp.tile([D, BL], f32)

        with tc.tile_pool(name="pre", bufs=4) as pre, \
             tc.tile_pool(name="pps", bufs=2, space="PSUM") as pps, \
             tc.tile_pool(name="hps", bufs=1, space="PSUM") as hps:
            nc.sync.dma_start(maskS[:, :], maskd[:, :])
            seqi = pre.tile([BL, 1], i32)
            nc.sync.dma_start(seqi[:, :], seqd[:, :])
            seqf = pre.tile([BL, 1], f32)
            nc.vector.tensor_copy(seqf[:, :], seqi[:, :])
            rsec = pre.tile([BL, 1], f32)
            nc.vector.reciprocal(rsec[:, :], seqf[:, :])
            maskSc = pre.tile([BL, T], f32)
            nc.vector.tensor_scalar_mul(maskSc[:, :], maskS[:, :], rsec[:, 0:1])

            mstp0 = pps.tile([128, BL], f32, tag="mstp")
            nc.tensor.transpose(mstp0[:, :], maskSc[:, 0:128], ident[0:BL, 0:BL])
            mst0 = pre.tile([128, BL], bf16, tag="mst0", bufs=1)
            nc.scalar.copy(mst0[:, :], mstp0[:, :])
            mstp1 = pps.tile([128, BL], f32, tag="mstp")
            nc.tensor.transpose(mstp1[0:72, :], maskSc[:, 128:200], ident[0:BL, 0:BL])
            mst1 = pre.tile([72, BL], bf16, tag="mst1", bufs=1)
            nc.scalar.copy(mst1[:, :], mstp1[0:72, :])

            itn = pre.tile([BL, D], f32, tag="itn")
            nc.sync.dma_start(itn[:, :], item[:, :])
            itp = pps.tile([D, BL], f32, tag="small_t")
            nc.tensor.transpose(itp[:, :], itn[:, :], ident[0:BL, 0:BL])
            nc.scalar.copy(qT[:, :], itp[:, :])
            usn = pre.tile([BL, D], f32, tag="itn")
            nc.sync.dma_start(usn[:, :], user[:, :])
            usp = pps.tile([D, BL], f32, tag="small_t")
            nc.tensor.transpose(usp[:, :], usn[:, :], ident[0:BL, 0:BL])
            nc.scalar.copy(userT[:, :], usp[:, :])
            b2p = pps.tile([BL, 1], f32, tag="small_t")
            _mm(nc, b2p[:, :], ones_f[0:1, 0:BL], b2v, start=True, stop=True)
            nc.scalar.copy(b2col[:, :], b2p[:, :])
            for s in range(CH):
                nc.scalar.copy(qRep[:, s * BL:(s + 1) * BL], qT[:, :])

            # X^T build + masked history sum. First-half (t<128) tiles for
            # ALL b come first: GRU chunk 0 only depends on those, so the
            # second half and the history matmuls overlap the running GRU.
            histp0 = hps.tile([D, BL], f32, tag="histp0")
            histp1 = hps.tile([D, BL], f32, tag="histp1")
            XT3 = XT.rearrange("p (t b) -> p t b", b=BL)
            for h, (t0, tn) in enumerate(((0, 128), (128, 72))):
                histp = histp0 if h == 0 else histp1
                for b in range(BL):
                    xn = pre.tile([128, D], bf16, tag="xn")
                    nc.sync.dma_start(xn[0:tn, :], hist[b, t0:t0 + tn, :])
                    xtp = pps.tile([D, 128], bf16, tag="xtp")
                    nc.tensor.transpose(xtp[:, 0:tn], xn[0:tn, :],
                                        identB128[0:tn, 0:tn])
                    dst = XT3[:, t0:t0 + tn, b]
                    if b % 2 == 0:
                        nc.scalar.copy(dst, xtp[:, 0:tn])
                    else:
                        nc.vector.tensor_copy(dst, xtp[:, 0:tn])
                    mcol = (mst0 if h == 0 else mst1)[0:tn, b:b + 1]
                    _mm(nc, histp[:, b:b + 1], xn[0:tn, :], mcol,
                        start=True, stop=True)
            nc.scalar.copy(histF[:, :], histp0[:, :])
            nc.vector.tensor_add(histF[:, :], histF[:, :], histp1[:, :])
            nc.scalar.copy(histT[:, :], histF[:, :])

        # ---------------- GRU + interleaved attention -------------------
        gruT3 = gruT.rearrange("p (t b) -> p t b", b=BL)
        SIG, TANH = AF.Sigmoid, AF.Tanh
        wgt = pp.tile([BL, T], f32)

        with tc.tile_pool(name="gip", bufs=2, space="PSUM") as gip, \
             tc.tile_pool(name="hnp", bufs=1, space="PSUM") as hnp, \
             tc.tile_pool(name="gt", bufs=6) as gt, \
             tc.tile_pool(name="aps", bufs=1, space="PSUM") as aps, \
             tc.tile_pool(name="sps", bufs=1, space="PSUM") as sps, \
             tc.tile_pool(name="at", bufs=3) as at:
            scp = sps.tile([BL, T], f32)

            def attention_chunk(c):
                gc = gruT[:, c * W:(c + 1) * W]
                r2 = at.tile([D, W], bf16, tag="r2")
                nc.gpsimd.tensor_mul(r2[:, :], gc, qRep[:, :])
                r3 = at.tile([D, W], bf16, tag="r3")
                nc.gpsimd.tensor_sub(r3[:, :], qRep[:, :], gc)
                y0p = aps.tile([80, W], f32, tag="y0")
                _mm(nc, y0p[:, :], W0T[:, 0:80], gc, start=True, stop=False)
                _mm(nc, y0p[:, :], W0T[:, 80:160], qRep, start=False, stop=False)
                _mm(nc, y0p[:, :], W0T[:, 160:240], r2, start=False, stop=False)
                _mm(nc, y0p[:, :], W0T[:, 240:320], r3, start=False, stop=True)
                y0 = at.tile([80, W], bf16, tag="y0s")
                nc.vector.tensor_scalar(y0[:, 0:256], y0p[:, 0:256],
                                        b0v[:, 0:1], 0.0, ALU.add, ALU.max)
                nc.vector.tensor_scalar(y0[:, 256:512], y0p[:, 256:512],
                                        b0v[:, 0:1], 0.0, ALU.add, ALU.max)
                y1p = aps.tile([40, W], f32, tag="y1")
                _mm(nc, y1p[:, :], W1T, y0, start=True, stop=True)
                y1 = at.tile([40, W], bf16, tag="y1s")
                nc.vector.tensor_scalar(y1[:, 0:256], y1p[:, 0:256],
                                        b1v[:, 0:1], 0.0, ALU.add, ALU.max)
                nc.vector.tensor_scalar(y1[:, 256:512], y1p[:, 256:512],
                                        b1v[:, 0:1], 0.0, ALU.add, ALU.max)
                for s in range(CH):
                    t = c * CH + s
                    _mm(nc, scp[:, t:t + 1], y1[:, s * BL:(s + 1) * BL],
                        W2T, start=True, stop=True)

            for c in range(NCG):
                XTc = XT[:, c * WG:(c + 1) * WG]
                # gate-major psum layout: [r: CG*64 | z: CG*64] in one bank
                prz = gip.tile([D, CG * 128], f32, tag="girz")
                prz4 = prz.rearrange("p (g s c) -> p g s c", g=2, c=BL)
                pn = gip.tile([D, CG * BL], f32, tag="gin")
                pn3 = pn.rearrange("p (s g) -> p s g", g=BL)
                _mm(nc, prz[:, 0:WG], WihT[:, 0:D], XTc,
                    start=True, stop=True)
                _mm(nc, prz[:, WG:2 * WG], WihT[:, D:2 * D], XTc,
                    start=True, stop=True)
                _mm(nc, pn[:, :], WihT[:, 2 * D:G3], XTc,
                    start=True, stop=True)
                for s in range(CG):
                    t = c * CG + s
                    if t > 0:
                        hprev = gruT3[:, t - 1, :]
                        _mm(nc, prz[:, s * BL:(s + 1) * BL], WhhT[:, 0:D],
                            hprev, start=False, stop=True, skip=True)
                        _mm(nc, prz[:, WG + s * BL:WG + (s + 1) * BL],
                            WhhT[:, D:2 * D], hprev,
                            start=False, stop=True, skip=True)
                        hn = hnp.tile([D, BL], f32, tag="hn")
                        _mm(nc, hn[:, :], WhhT[:, 2 * D:G3], hprev,
                            start=True, stop=True)
                    # gate activations; biases ride the ACT instruction
                    r = gt.tile([D, BL], bf16, tag="r")
                    nc.scalar.activation(r[:, :], prz4[:, 0, s, :], SIG,
                                         bias=brz[:, 0:1])
                    z = gt.tile([D, BL], bf16, tag="z")
                    nc.scalar.activation(z[:, :], prz4[:, 1, s, :], SIG,
                                         bias=brz[:, 1:2])
                    omz = gt.tile([D, BL], bf16, tag="omz")
                    nc.scalar.activation(omz[:, :], prz4[:, 1, s, :], SIG,
                                         bias=nbz[:, 0:1], scale=-1.0)
                    tmp = gt.tile([D, BL], bf16, tag="tmp")
                    if t > 0:
                        nc.vector.scalar_tensor_tensor(
                            tmp[:, :], hn[:, :], b_hn, r[:, :],
                            ALU.add, ALU.mult)
                    else:
                        nc.vector.tensor_scalar_mul(tmp[:, :], r[:, :], b_hn)
                    tmp2 = gt.tile([D, BL], bf16, tag="tmp2")
                    nc.vector.tensor_add(tmp2[:, :], tmp[:, :], pn3[:, s, :])
                    if t > 0:
                        zh = gt.tile([D, BL], bf16, tag="zh")
                        nc.vector.tensor_mul(zh[:, :], z[:, :], hprev)
                    n = gt.tile([D, BL], bf16, tag="n")
                    nc.scalar.activation(n[:, :], tmp2[:, :], TANH, bias=b_in)
                    hdst = gruT3[:, t, :]
                    if t > 0:
                        omzn = gt.tile([D, BL], bf16, tag="omzn")
                        nc.vector.tensor_mul(omzn[:, :], omz[:, :], n[:, :])
                        nc.vector.tensor_add(hdst, omzn[:, :], zh[:, :])
                    else:
                        nc.vector.tensor_mul(hdst, omz[:, :], n[:, :])
                if c % 2 == 1:
                    attention_chunk(c // 2)

            # masked softmax over t (b-major); last MLP layer ReLU'd w/ b2
            rawr = at.tile([BL, T], f32, tag="rawr")
            nc.scalar.activation(rawr[:, :], scp[:, :], AF.Relu,
                                 bias=b2col[:, 0:1])
            rawm = at.tile([BL, T], f32, tag="rawm")
            nc.vector.tensor_mul(rawm[:, :], rawr[:, :], maskS[:, :])
            mxn = at.tile([BL, 1], f32, tag="mxn")
            nc.vector.tensor_reduce(mxn[:, :], rawm[:, :],
                                    axis=mybir.AxisListType.X,
                                    op=ALU.max, negate=True)
            ex = at.tile([BL, T], f32, tag="ex")
            nc.scalar.activation(ex[:, :], rawr[:, :], AF.Exp, bias=mxn[:, 0:1])
            em = at.tile([BL, T], f32, tag="em")
            nc.vector.tensor_mul(em[:, :], ex[:, :], maskS[:, :])
            sm = at.tile([BL, 1], f32, tag="sm")
            nc.vector.tensor_reduce(sm[:, :], em[:, :],
                                    axis=mybir.AxisListType.X, op=ALU.add)
            rs = at.tile([BL, 1], f32, tag="rs")
            nc.vector.reciprocal(rs[:, :], sm[:, :])
            nc.vector.tensor_scalar_mul(wgt[:, :], em[:, :], rs[:, 0:1])

        # attention weights go to partition-0 bf16 rows per AUGRU chunk
        # (the row transposes + broadcasts overlap the AUGRU itself)
        wgt_bf = pp.tile([BL, T], bf16)
        nc.scalar.copy(wgt_bf[:, :], wgt[:, :])

        # ---------------- AUGRU ----------------------------------------
        hA = [None]
        with tc.tile_pool(name="axp", bufs=2, space="PSUM") as axp, \
             tc.tile_pool(name="abp", bufs=2, space="PSUM") as abp, \
             tc.tile_pool(name="rtp", bufs=2, space="PSUM") as rtp, \
             tc.tile_pool(name="ut", bufs=6) as ut:
            for c in range(NCG):
                gc = gruT[:, c * WG:(c + 1) * WG]
                rowp = rtp.tile([1, WG], f32, tag="rowp")
                for s in range(CG):
                    t = c * CG + s
                    _mm(nc, rowp[0:1, s * BL:(s + 1) * BL], wgt_bf[:, t:t + 1],
                        identB, start=True, stop=True)
                rows = ut.tile([1, WG], bf16, tag="rows")
                nc.vector.tensor_copy(rows[0:1, :], rowp[0:1, :])
                pru = axp.tile([D, CG * 128], f32, tag="pxru")
                pru4 = pru.rearrange("p (g s c) -> p g s c", g=2, c=BL)
                pxh = axp.tile([D, CG * BL], f32, tag="pxh")
                pxh3 = pxh.rearrange("p (s g) -> p s g", g=BL)
                _mm(nc, pru[:, 0:WG], WrxT, gc, start=True, stop=True)
                _mm(nc, pru[:, WG:2 * WG], WuxT, gc, start=True, stop=True)
                _mm(nc, pxh[:, :], WaxT, gc, start=True, stop=True)
                pab = abp.tile([D, WG], f32, tag="pab")
                _mm(nc, pab[:, :], ones1, rows[0:1, :], start=True, stop=True)
                for s in range(CG):
                    t = c * CG + s
                    hAg = hA[0]
                    if t > 0:
                        _mm(nc, pru[:, s * BL:(s + 1) * BL], WrhT, hAg,
                            start=False, stop=True, skip=True)
                        _mm(nc, pru[:, WG + s * BL:WG + (s + 1) * BL],
                            WuhT, hAg, start=False, stop=True, skip=True)
                    r = ut.tile([D, BL], bf16, tag="ar")
                    nc.scalar.activation(r[:, :], pru4[:, 0, s, :], SIG,
                                         bias=ab_r)
                    u = ut.tile([D, BL], bf16, tag="au")
                    nc.scalar.activation(u[:, :], pru4[:, 1, s, :], SIG,
                                         bias=ab_u)
                    if t > 0:
                        hr = ut.tile([D, BL], bf16, tag="ahr")
                        nc.vector.tensor_mul(hr[:, :], hAg, r[:, :])
                        _mm(nc, pxh3[:, s, :], WahT, hr,
                            start=False, stop=True, skip=True)
                    up = ut.tile([D, BL], f32, tag="aup")
                    nc.vector.tensor_mul(up[:, :], pab[:, s * BL:(s + 1) * BL],
                                         u[:, :])
                    hh = ut.tile([D, BL], bf16, tag="ahh")
                    nc.scalar.activation(hh[:, :], pxh3[:, s, :], TANH,
                                         bias=ab_h)
                    hnew = hp.tile([D, BL], bf16, tag="hA")
                    if t > 0:
                        dd = ut.tile([D, BL], bf16, tag="add")
                        nc.vector.tensor_sub(dd[:, :], hh[:, :], hAg)
                        ud = ut.tile([D, BL], bf16, tag="aud")
                        nc.vector.tensor_mul(ud[:, :], up[:, :], dd[:, :])
                        nc.vector.tensor_add(hnew[:, :], hAg, ud[:, :])
                    else:
                        nc.vector.tensor_mul(hnew[:, :], up[:, :], hh[:, :])
                    hA[0] = hnew

        # ---------------- output layer ---------------------------------
        with tc.tile_pool(name="ops", bufs=1, space="PSUM") as ops, \
             tc.tile_pool(name="ot", bufs=1) as ot:
            ih = ot.tile([D, BL], bf16)
            nc.vector.tensor_mul(ih[:, :], qT[:, :], histF[:, :])
            po = ops.tile([1, BL], f32)
            pieces = [userT, qT, histT, ih, hA[0]]
            for g, piece in enumerate(pieces):
                _mm(nc, po[:, :], outWT[:, g:g + 1], piece,
                    start=(g == 0), stop=(g == 4))
            outs = ot.tile([1, BL], f32)
            nc.scalar.activation(outs[:, :], po[:, :], AF.Identity,
                                 bias=outb[:, 0:1])
            nc.sync.dma_start(outd[:, :], outs[:, :])

    nc.finalize()
    return nc


_NC = None


def _get_nc():
    global _NC
    if _NC is None:
        _NC = build_nc()
    return _NC


def make_in_maps(inputs):
    """Slice full inputs into per-core input maps (host-side layout only)."""
    f = {k: np.asarray(v) for k, v in inputs.items()}
    WihT = np.ascontiguousarray(f["gru_Wih"].T)          # (128, 384)
    WhhT = np.ascontiguousarray(f["gru_Whh"].T)
    bihT = np.ascontiguousarray(f["gru_bih"].reshape(3, D).T)  # (128, 3)
    bhhT = np.ascontiguousarray(f["gru_bhh"].reshape(3, D).T)
    W0T = np.ascontiguousarray(
        f["attn_W0"].T.reshape(4, D, 80).transpose(1, 0, 2).reshape(D, 320))
    b0 = np.ascontiguousarray(f["attn_b0"].reshape(80, 1))
    W1T = np.ascontiguousarray(f["attn_W1"].T)           # (80, 40)
    b1 = np.ascontiguousarray(f["attn_b1"].reshape(40, 1))
    W2T = np.ascontiguousarray(f["attn_W2"].T)           # (40, 1)
    b2 = f["attn_b2"].reshape(1, 1)
    augW = np.concatenate(
        [np.ascontiguousarray(f[k][:, p * D:(p + 1) * D].T)
         for k in ("aug_Wr", "aug_Wu", "aug_Wh") for p in (0, 1)],
        axis=1)                                          # (128, 768)
    augb = np.stack([f["aug_br"], f["aug_bu"], f["aug_bh"]], axis=1)  # (128,3)
    outWT = np.ascontiguousarray(f["out_W"].reshape(5, D).T)          # (128,5)
    outb = f["out_b"].reshape(1, 1)

    shared_bf = dict(WihT=WihT, WhhT=WhhT, W0T=W0T, W1T=W1T, W2T=W2T,
                     augW=augW, outWT=outWT)
    shared = dict(bihT=bihT, bhhT=bhhT, b0=b0, b1=b1, b2=b2, augb=augb,
                  outb=outb)
    shared = {k: np.ascontiguousarray(v.astype(np.float32)) for k, v in
              shared.items()}
    shared.update({k: np.ascontiguousarray(v.astype(BF)) for k, v in
                   shared_bf.items()})

    in_maps = []
    for c in range(NCORES):
        s = slice(c * BL, (c + 1) * BL)
        m = dict(shared)
        m["hist"] = np.ascontiguousarray(
            f["item_historical_embedding"][s].astype(BF))
        m["item"] = np.ascontiguousarray(f["item_embedding"][s].astype(np.float32))
        m["user"] = np.ascontiguousarray(f["user_embedding"][s].astype(np.float32))
        m["maskd"] = np.ascontiguousarray(f["mask"][s].astype(np.float32))
        m["seqd"] = np.ascontiguousarray(
            f["sequential_length"][s].reshape(BL, 1).astype(np.int32))
        in_maps.append(m)
    return in_maps


def kernel(**inputs) -> np.ndarray:
    nc = _get_nc()
    in_maps = make_in_maps(inputs)
    res = run_bass_kernel_spmd(nc, in_maps, list(range(NCORES)))
    return np.concatenate(
        [np.asarray(res.results[c]["out"]).reshape(BL) for c in range(NCORES)])


# revision 12
# speedup vs baseline: 1.1453x; 1.0314x over previous
"""DIEN (GRU + attention + AUGRU) Trainium2 kernel.

Data-parallel over 8 NeuronCores: each core handles a batch slice of 64.
All on-chip state is feature-major (feature dim on SBUF partitions, batch
on the free dim), so the two sequential recurrences need no per-step
transposes.

Structure:
  - X^T (D, T*64) in bf16, built via PE transposes of natural-layout DMA
    tiles; the same stationary tiles also accumulate the masked history
    sum (f32) via N=1 matmuls against 1/len-scaled mask columns.
  - GRU input projections land in PSUM chunks laid out (128, 4, [r|z|n]).
    Gate sigmoids are split per-gate so the gate bias rides the ACT
    instruction (no DVE bias adds), and (1-z) comes from sigmoid(-x).
    The combine uses h' = (1-z)*n + z*h with z*h computed off the
    critical path.
  - Attention runs per 8-step chunk interleaved with the GRU loop; its
    elementwise prep (fact*q, q-fact) runs on the idle GPSIMD engine so
    the DVE stays free for the recurrence chain.
  - AUGRU consumes attention weights via rows pre-transposed once after
    the softmax; per chunk a single ones-weight load broadcasts 4 rows
    into PSUM. Masked steps have weight 0 (u'=0, h unchanged), so the
    final AUGRU state IS the gathered aug_out[b, len-1] - no gather.
  - Matmuls are bf16 (f32 PSUM accumulation); elementwise recurrence ops
    run bf16 where both operands sit in SBUF (2x DVE mode); the masked
    history mean and softmax stay f32.
"""

import sys

if "/opt/trn_rl_repo" not in sys.path:
    sys.path.insert(0, "/opt/trn_rl_repo")

from contextlib import ExitStack

import ml_dtypes
import numpy as np

import concourse.bacc as bacc
import concourse.bass as bass
import concourse.mybir as mybir
import concourse.tile as tile
from concourse.bass_utils import run_bass_kernel_spmd
from concourse.masks import make_identity

f32 = mybir.dt.float32
bf16 = mybir.dt.bfloat16
i32 = mybir.dt.int32
AF = mybir.ActivationFunctionType
ALU = mybir.AluOpType
BF = ml_dtypes.bfloat16

NCORES = 8
B, T, D = 512, 200, 128
BL = B // NCORES          # 64 batch rows per core
CG = 4                    # GRU/AUGRU psum chunk: steps per chunk
NCG = T // CG             # 50
WG = CG * BL              # 256
CH = 8                    # attention chunk: steps per chunk
NCHUNK = T // CH          # 25
W = CH * BL               # 512
G3 = 3 * D


def _mm(nc, out, lhsT, rhs, start, stop, skip=False):
    nc.tensor.matmul(out, lhsT, rhs, start=start, stop=stop,
                     skip_group_check=skip)


def build_nc():
    nc = bacc.Bacc("TRN2", target_bir_lowering=False)

    hist = nc.declare_dram_parameter("hist", [BL, T, D], bf16, isOutput=False)
    item = nc.declare_dram_parameter("item", [BL, D], f32, isOutput=False)
    user = nc.declare_dram_parameter("user", [BL, D], f32, isOutput=False)
    maskd = nc.declare_dram_parameter("maskd", [BL, T], f32, isOutput=False)
    seqd = nc.declare_dram_parameter("seqd", [BL, 1], i32, isOutput=False)
    WihTd = nc.declare_dram_parameter("WihT", [D, G3], bf16, isOutput=False)
    WhhTd = nc.declare_dram_parameter("WhhT", [D, G3], bf16, isOutput=False)
    bihTd = nc.declare_dram_parameter("bihT", [D, 3], f32, isOutput=False)
    bhhTd = nc.declare_dram_parameter("bhhT", [D, 3], f32, isOutput=False)
    W0Td = nc.declare_dram_parameter("W0T", [D, 240], bf16, isOutput=False)
    b0d = nc.declare_dram_parameter("b0", [80, 1], f32, isOutput=False)
    W1Td = nc.declare_dram_parameter("W1T", [80, 40], bf16, isOutput=False)
    b1d = nc.declare_dram_parameter("b1", [40, 1], f32, isOutput=False)
    W2Td = nc.declare_dram_parameter("W2T", [40, 1], bf16, isOutput=False)
    b2d = nc.declare_dram_parameter("b2", [1, 1], f32, isOutput=False)
    augWd = nc.declare_dram_parameter("augW", [D, 6 * D], bf16, isOutput=False)
    augbd = nc.declare_dram_parameter("augb", [D, 3], f32, isOutput=False)
    outWTd = nc.declare_dram_parameter("outWT", [D, 5], bf16, isOutput=False)
    outbd = nc.declare_dram_parameter("outb", [1, 1], f32, isOutput=False)
    outd = nc.declare_dram_parameter("out", [1, BL], f32, isOutput=True)

    with tile.TileContext(nc) as tc, ExitStack() as ctx:
        big = ctx.enter_context(tc.tile_pool(name="big", bufs=1))
        wp = ctx.enter_context(tc.tile_pool(name="wp", bufs=1))
        pp = ctx.enter_context(tc.tile_pool(name="pp", bufs=1))
        hp = ctx.enter_context(tc.tile_pool(name="hp", bufs=4))

        XT = big.tile([D, T * BL], bf16)
        gruT = big.tile([D, T * BL], bf16)

        ident = pp.tile([128, 128], f32)
        make_identity(nc, ident)
        identB = pp.tile([BL, BL], bf16)
        make_identity(nc, identB)
        identB128 = pp.tile([128, 128], bf16)
        make_identity(nc, identB128)

        # ------------- weights (bf16 arrive pre-converted via DMA) -------
        WihT = wp.tile([D, G3], bf16)
        nc.sync.dma_start(WihT[:, :], WihTd[:, :])
        WhhT = wp.tile([D, G3], bf16)
        nc.sync.dma_start(WhhT[:, :], WhhTd[:, :])
        W0T = wp.tile([D, 240], bf16)
        nc.sync.dma_start(W0T[:, :], W0Td[:, :])
        W1T = wp.tile([80, 40], bf16)
        nc.sync.dma_start(W1T[:, :], W1Td[:, :])
        W2T = wp.tile([40, 1], bf16)
        nc.sync.dma_start(W2T[:, :], W2Td[:, :])
        augW = wp.tile([D, 6 * D], bf16)
        nc.sync.dma_start(augW[:, :], augWd[:, :])
        outWT = wp.tile([D, 5], bf16)
        nc.sync.dma_start(outWT[:, :], outWTd[:, :])
        WrhT, WrxT = augW[:, 0:D], augW[:, D:2 * D]
        WuhT, WuxT = augW[:, 2 * D:3 * D], augW[:, 3 * D:4 * D]
        WahT, WaxT = augW[:, 4 * D:5 * D], augW[:, 5 * D:6 * D]

        bihT = wp.tile([D, 3], f32)
        nc.sync.dma_start(bihT[:, :], bihTd[:, :])
        bhhT = wp.tile([D, 3], f32)
        nc.sync.dma_start(bhhT[:, :], bhhTd[:, :])
        brz = wp.tile([D, 2], f32)
        nc.vector.tensor_add(brz[:, :], bihT[:, 0:2], bhhT[:, 0:2])
        nbz = wp.tile([D, 1], f32)
        nc.vector.tensor_scalar(nbz[:, :], brz[:, 1:2], -1.0, 0.0,
                                ALU.mult, ALU.add)
        b_hn, b_in = bhhT[:, 2:3], bihT[:, 2:3]

        b0v = wp.tile([80, 1], f32)
        nc.sync.dma_start(b0v[:, :], b0d[:, :])
        b1v = wp.tile([40, 1], f32)
        nc.sync.dma_start(b1v[:, :], b1d[:, :])
        b2v = wp.tile([1, 1], f32)
        nc.sync.dma_start(b2v[:, :], b2d[:, :])
        augb = wp.tile([D, 3], f32)
        nc.sync.dma_start(augb[:, :], augbd[:, :])
        outb = wp.tile([1, 1], f32)
        nc.sync.dma_start(outb[:, :], outbd[:, :])
        ab_r, ab_u, ab_h = augb[:, 0:1], augb[:, 1:2], augb[:, 2:3]

        ones_f = wp.tile([1, 128], f32)
        nc.vector.memset(ones_f[:, :], 1.0)
        ones1 = wp.tile([1, 128], bf16)
        nc.vector.memset(ones1[:, :], 1.0)

        # ---------------- preprocessing --------------------------------
        histT = pp.tile([D, BL], bf16)
        qT = pp.tile([D, BL], bf16)
        userT = pp.tile([D, BL], bf16)
        qRep = pp.tile([D, W], bf16)
        maskS = pp.tile([BL, T], f32)
        b2col = pp.tile([BL, 1], f32)
        histF = pp.tile([D, BL], f32)

        with tc.tile_pool(name="pre", bufs=4) as pre, \
             tc.tile_pool(name="pps", bufs=2, space="PSUM") as pps, \
             tc.tile_pool(name="hps", bufs=1, space="PSUM") as hps:
            nc.sync.dma_start(maskS[:, :], maskd[:, :])
            seqi = pre.tile([BL, 1], i32)
            nc.sync.dma_start(seqi[:, :], seqd[:, :])
            seqf = pre.tile([BL, 1], f32)
            nc.vector.tensor_copy(seqf[:, :], seqi[:, :])
            rsec = pre.tile([BL, 1], f32)
            nc.vector.reciprocal(rsec[:, :], seqf[:, :])
            maskSc = pre.tile([BL, T], f32)
            nc.vector.tensor_scalar_mul(maskSc[:, :], maskS[:, :], rsec[:, 0:1])

            mstp0 = pps.tile([128, BL], f32, tag="mstp")
            nc.tensor.transpose(mstp0[:, :], maskSc[:, 0:128], ident[0:BL, 0:BL])
            mst0 = pre.tile([128, BL], bf16, tag="mst0", bufs=1)
            nc.scalar.copy(mst0[:, :], mstp0[:, :])
            mstp1 = pps.tile([128, BL], f32, tag="mstp")
            nc.tensor.transpose(mstp1[0:72, :], maskSc[:, 128:200], ident[0:BL, 0:BL])
            mst1 = pre.tile([72, BL], bf16, tag="mst1", bufs=1)
            nc.scalar.copy(mst1[:, :], mstp1[0:72, :])

            itn = pre.tile([BL, D], f32, tag="itn")
            nc.sync.dma_start(itn[:, :], item[:, :])
            itp = pps.tile([D, BL], f32, tag="small_t")
            nc.tensor.transpose(itp[:, :], itn[:, :], ident[0:BL, 0:BL])
            nc.scalar.copy(qT[:, :], itp[:, :])
            usn = pre.tile([BL, D], f32, tag="itn")
            nc.sync.dma_start(usn[:, :], user[:, :])
            usp = pps.tile([D, BL], f32, tag="small_t")
            nc.tensor.transpose(usp[:, :], usn[:, :], ident[0:BL, 0:BL])
            nc.scalar.copy(userT[:, :], usp[:, :])
            b2p = pps.tile([BL, 1], f32, tag="small_t")
            _mm(nc, b2p[:, :], ones_f[0:1, 0:BL], b2v, start=True, stop=True)
            nc.scalar.copy(b2col[:, :], b2p[:, :])
            for s in range(CH):
                nc.scalar.copy(qRep[:, s * BL:(s + 1) * BL], qT[:, :])

            # X^T build + masked history sum. First-half (t<128) tiles for
            # ALL b come first: GRU chunk 0 only depends on those, so the
            # second half and the history matmuls overlap the running GRU.
            histp0 = hps.tile([D, BL], f32, tag="histp0")
            histp1 = hps.tile([D, BL], f32, tag="histp1")
            XT3 = XT.rearrange("p (t b) -> p t b", b=BL)
            for h, (t0, tn) in enumerate(((0, 128), (128, 72))):
                histp = histp0 if h == 0 else histp1
                for b in range(BL):
                    xn = pre.tile([128, D], bf16, tag="xn")
                    nc.sync.dma_start(xn[0:tn, :], hist[b, t0:t0 + tn, :])
                    xtp = pps.tile([D, 128], bf16, tag="xtp")
                    nc.tensor.transpose(xtp[:, 0:tn], xn[0:tn, :],
                                        identB128[0:tn, 0:tn])
                    dst = XT3[:, t0:t0 + tn, b]
                    if b % 2 == 0:
                        nc.scalar.copy(dst, xtp[:, 0:tn])
                    else:
                        nc.vector.tensor_copy(dst, xtp[:, 0:tn])
                    mcol = (mst0 if h == 0 else mst1)[0:tn, b:b + 1]
                    _mm(nc, histp[:, b:b + 1], xn[0:tn, :], mcol,
                        start=True, stop=True)
            nc.scalar.copy(histF[:, :], histp0[:, :])
            nc.vector.tensor_add(histF[:, :], histF[:, :], histp1[:, :])
            nc.scalar.copy(histT[:, :], histF[:, :])

        # ---------------- GRU + interleaved attention -------------------
        gruT3 = gruT.rearrange("p (t b) -> p t b", b=BL)
        SIG, TANH = AF.Sigmoid, AF.Tanh
        wgt = pp.tile([BL, T], f32)

        with tc.tile_pool(name="gip", bufs=2, space="PSUM") as gip, \
             tc.tile_pool(name="hnp", bufs=1, space="PSUM") as hnp, \
             tc.tile_pool(name="gt", bufs=6) as gt, \
             tc.tile_pool(name="aps", bufs=1, space="PSUM") as aps, \
             tc.tile_pool(name="sps", bufs=1, space="PSUM") as sps, \
             tc.tile_pool(name="at", bufs=3) as at:
            scp = sps.tile([BL, T], f32)

            def attention_pieces(a):
                """Attention for gru chunks [2a, 2a+1] as 8 emission pieces;
                each piece is PE/DVE/Pool work that fills one GRU step's
                engine-idle window without blocking the recurrence FIFO."""
                gc = gruT[:, a * W:(a + 1) * W]
                r2 = at.tile([D, W], bf16, tag="r2")
                y0p = aps.tile([80, W], f32, tag="y0")
                y0 = at.tile([80, W], bf16, tag="y0s")
                y1p = aps.tile([40, W], f32, tag="y1")
                y1 = at.tile([40, W], bf16, tag="y1s")

                def p0():
                    nc.gpsimd.tensor_mul(r2[:, :], gc, qRep[:, :])
                def p1():
                    _mm(nc, y0p[:, :], W0T[:, 0:80], gc,
                        start=True, stop=False)
                def p2():
                    _mm(nc, y0p[:, :], W0T[:, 80:160], qRep,
                        start=False, stop=False)
                def p3():
                    _mm(nc, y0p[:, :], W0T[:, 160:240], r2,
                        start=False, stop=True)
                def p4():
                    nc.vector.tensor_scalar(y0[:, 0:256], y0p[:, 0:256],
                                            b0v[:, 0:1], 0.0, ALU.add, ALU.max)
                    nc.vector.tensor_scalar(y0[:, 256:512], y0p[:, 256:512],
                                            b0v[:, 0:1], 0.0, ALU.add, ALU.max)
                def p5():
                    _mm(nc, y1p[:, :], W1T, y0, start=True, stop=True)
                    nc.vector.tensor_scalar(y1[:, 0:256], y1p[:, 0:256],
                                            b1v[:, 0:1], 0.0, ALU.add, ALU.max)
                def p6():
                    nc.vector.tensor_scalar(y1[:, 256:512], y1p[:, 256:512],
                                            b1v[:, 0:1], 0.0, ALU.add, ALU.max)
                    for s in range(4):
                        t = a * CH + s
                        _mm(nc, scp[:, t:t + 1], y1[:, s * BL:(s + 1) * BL],
                            W2T, start=True, stop=True)
                def p7():
                    for s in range(4, CH):
                        t = a * CH + s
                        _mm(nc, scp[:, t:t + 1], y1[:, s * BL:(s + 1) * BL],
                            W2T, start=True, stop=True)
                return [p0, p1, p2, p3, p4, p5, p6, p7]

            pending = []          # attention pieces awaiting emission
            WhT_r, WhT_z, WhT_n = (WhhT[:, 0:D], WhhT[:, D:2 * D],
                                   WhhT[:, 2 * D:G3])
            gslot = [None, None]  # (prz, prz4, pn3) per psum buf

            def gru_proj(c):
                XTc = XT[:, c * WG:(c + 1) * WG]
                prz = gip.tile([D, CG * 128], f32, tag="girz")
                pn = gip.tile([D, CG * BL], f32, tag="gin")
                _mm(nc, prz[:, 0:WG], WihT[:, 0:D], XTc,
                    start=True, stop=True)
                _mm(nc, prz[:, WG:2 * WG], WihT[:, D:2 * D], XTc,
                    start=True, stop=True)
                _mm(nc, pn[:, :], WihT[:, 2 * D:G3], XTc,
                    start=True, stop=True)
                gslot[c % 2] = (
                    prz,
                    prz.rearrange("p (g s c) -> p g s c", g=2, c=BL),
                    pn.rearrange("p (s g) -> p s g", g=BL),
                )

            gru_proj(0)
            zh_prev = [None]      # z_{t-1} * h_{t-2}
            omzn_prev = [None]    # (1-z_{t-1}) * n_{t-1}
            for c in range(NCG):
                prz, prz4, pn3 = gslot[c % 2]
                for s in range(CG):
                    t = c * CG + s
                    # --- PE block A: recurrent matmuls feeding step t ----
                    hx = hnp.tile([D, 2 * BL], f32, tag="hn2")
                    hn, tmp2 = hx[:, 0:BL], hx[:, BL:2 * BL]
                    if t == 1:
                        h0 = gruT3[:, 0, :]
                        _mm(nc, prz[:, s * BL:(s + 1) * BL], WhT_r, h0,
                            start=False, stop=True, skip=True)
                        _mm(nc, prz[:, WG + s * BL:WG + (s + 1) * BL],
                            WhT_z, h0, start=False, stop=True, skip=True)
                        _mm(nc, hn[:, :], WhT_n, h0, start=True, stop=True)
                    elif t > 1:
                        rsl = prz[:, s * BL:(s + 1) * BL]
                        _mm(nc, rsl, WhT_r, zh_prev[0],
                            start=False, stop=False, skip=True)
                        _mm(nc, rsl, WhT_r, omzn_prev[0],
                            start=False, stop=True, skip=True)
                        hprev = gruT3[:, t - 1, :]
                        _mm(nc, prz[:, WG + s * BL:WG + (s + 1) * BL],
                            WhT_z, hprev, start=False, stop=True, skip=True)
                        _mm(nc, hn[:, :], WhT_n, hprev, start=True, stop=True)
                    # --- gate activations (biases ride ACT) --------------
                    r = gt.tile([D, BL], bf16, tag="r")
                    nc.scalar.activation(r[:, :], prz4[:, 0, s, :], SIG,
                                         bias=brz[:, 0:1])
                    z = gt.tile([D, BL], bf16, tag="z")
                    nc.scalar.activation(z[:, :], prz4[:, 1, s, :], SIG,
                                         bias=brz[:, 1:2])
                    omz = gt.tile([D, BL], bf16, tag="omz")
                    nc.gpsimd.tensor_scalar(omz[:, :], z[:, :], -1.0, 1.0,
                                            ALU.mult, ALU.add)
                    tmp = gt.tile([D, BL], bf16, tag="tmp")
                    if t > 0:
                        nc.vector.scalar_tensor_tensor(
                            tmp[:, :], hn[:, :], b_hn, r[:, :],
                            ALU.add, ALU.mult)
                    else:
                        nc.vector.tensor_scalar_mul(tmp[:, :], r[:, :], b_hn)
                    nc.vector.tensor_add(tmp2[:, :], tmp[:, :], pn3[:, s, :])
                    if t > 0:
                        zh = gt.tile([D, BL], bf16, tag="zh")
                        nc.vector.tensor_mul(zh[:, :], z[:, :],
                                             gruT3[:, t - 1, :])
                    n = gt.tile([D, BL], bf16, tag="n")
                    nc.scalar.activation(n[:, :], tmp2[:, :], TANH, bias=b_in)
                    hdst = gruT3[:, t, :]
                    if t > 0:
                        omzn = gt.tile([D, BL], bf16, tag="omzn")
                        nc.vector.tensor_mul(omzn[:, :], omz[:, :], n[:, :])
                        nc.vector.tensor_add(hdst, omzn[:, :], zh[:, :])
                        zh_prev[0], omzn_prev[0] = zh, omzn
                    else:
                        nc.vector.tensor_mul(hdst, omz[:, :], n[:, :])
                    # --- PE block B: hoisted chunk/attention work --------
                    if s == 1 and c + 1 < NCG:
                        gru_proj(c + 1)
                    if pending:
                        pending.pop(0)()
                if c % 2 == 1:
                    pending = attention_pieces(c // 2)
            for p in pending:
                p()

            # masked softmax over t (b-major); last MLP layer ReLU'd w/ b2
            rawr = at.tile([BL, T], f32, tag="rawr")
            nc.scalar.activation(rawr[:, :], scp[:, :], AF.Relu,
                                 bias=b2col[:, 0:1])
            rawm = at.tile([BL, T], f32, tag="rawm")
            nc.vector.tensor_mul(rawm[:, :], rawr[:, :], maskS[:, :])
            mxn = at.tile([BL, 1], f32, tag="mxn")
            nc.vector.tensor_reduce(mxn[:, :], rawm[:, :],
                                    axis=mybir.AxisListType.X,
                                    op=ALU.max, negate=True)
            ex = at.tile([BL, T], f32, tag="ex")
            nc.scalar.activation(ex[:, :], rawr[:, :], AF.Exp, bias=mxn[:, 0:1])
            em = at.tile([BL, T], f32, tag="em")
            nc.vector.tensor_mul(em[:, :], ex[:, :], maskS[:, :])
            sm = at.tile([BL, 1], f32, tag="sm")
            nc.vector.tensor_reduce(sm[:, :], em[:, :],
                                    axis=mybir.AxisListType.X, op=ALU.add)
            rs = at.tile([BL, 1], f32, tag="rs")
            nc.vector.reciprocal(rs[:, :], sm[:, :])
            nc.vector.tensor_scalar_mul(wgt[:, :], em[:, :], rs[:, 0:1])

        # attention weights go to partition-0 bf16 rows per AUGRU chunk
        # (the row transposes + broadcasts overlap the AUGRU itself)
        wgt_bf = pp.tile([BL, T], bf16)
        nc.scalar.copy(wgt_bf[:, :], wgt[:, :])

        # ---------------- AUGRU ----------------------------------------
        hA = [None]
        with tc.tile_pool(name="axp", bufs=2, space="PSUM") as axp, \
             tc.tile_pool(name="abp", bufs=2, space="PSUM") as abp, \
             tc.tile_pool(name="rtp", bufs=2, space="PSUM") as rtp, \
             tc.tile_pool(name="ut", bufs=6) as ut:
            aslot = [None, None]  # (pru4, pxh3, pab) per psum buf

            def aug_proj(c):
                gc = gruT[:, c * WG:(c + 1) * WG]
                rowp = rtp.tile([1, WG], f32, tag="rowp")
                for s in range(CG):
                    t = c * CG + s
                    _mm(nc, rowp[0:1, s * BL:(s + 1) * BL], wgt_bf[:, t:t + 1],
                        identB, start=True, stop=True)
                rows = ut.tile([1, WG], bf16, tag="rows")
                nc.vector.tensor_copy(rows[0:1, :], rowp[0:1, :])
                pru = axp.tile([D, CG * 128], f32, tag="pxru")
                pxh = axp.tile([D, CG * BL], f32, tag="pxh")
                _mm(nc, pru[:, 0:WG], WrxT, gc, start=True, stop=True)
                _mm(nc, pru[:, WG:2 * WG], WuxT, gc, start=True, stop=True)
                _mm(nc, pxh[:, :], WaxT, gc, start=True, stop=True)
                pab = abp.tile([D, WG], f32, tag="pab")
                _mm(nc, pab[:, :], ones1, rows[0:1, :], start=True, stop=True)
                aslot[c % 2] = (
                    pru,
                    pru.rearrange("p (g s c) -> p g s c", g=2, c=BL),
                    pxh.rearrange("p (s g) -> p s g", g=BL),
                    pab,
                )

            aug_proj(0)
            hA_pp = [None]        # hA_{t-2}
            ud_prev = [None]      # ud_{t-1}
            for c in range(NCG):
                pru, pru4, pxh3, pab = aslot[c % 2]
                for s in range(CG):
                    t = c * CG + s
                    hAg = hA[0]
                    # --- PE block A: recurrent matmuls feeding step t ----
                    rsl = pru[:, s * BL:(s + 1) * BL]
                    usl = pru[:, WG + s * BL:WG + (s + 1) * BL]
                    if t == 1:
                        _mm(nc, rsl, WrhT, hAg, start=False, stop=True,
                            skip=True)
                        _mm(nc, usl, WuhT, hAg, start=False, stop=True,
                            skip=True)
                    elif t > 1:
                        _mm(nc, rsl, WrhT, hA_pp[0],
                            start=False, stop=False, skip=True)
                        _mm(nc, rsl, WrhT, ud_prev[0],
                            start=False, stop=True, skip=True)
                        _mm(nc, usl, WuhT, hA_pp[0],
                            start=False, stop=False, skip=True)
                        _mm(nc, usl, WuhT, ud_prev[0],
                            start=False, stop=True, skip=True)
                    r = ut.tile([D, BL], bf16, tag="ar")
                    nc.scalar.activation(r[:, :], pru4[:, 0, s, :], SIG,
                                         bias=ab_r)
                    u = ut.tile([D, BL], bf16, tag="au")
                    nc.scalar.activation(u[:, :], pru4[:, 1, s, :], SIG,
                                         bias=ab_u)
                    if t > 0:
                        hr = ut.tile([D, BL], bf16, tag="ahr")
                        nc.vector.tensor_mul(hr[:, :], hAg, r[:, :])
                        _mm(nc, pxh3[:, s, :], WahT, hr,
                            start=False, stop=True, skip=True)
                    up = ut.tile([D, BL], f32, tag="aup")
                    nc.vector.tensor_mul(up[:, :], pab[:, s * BL:(s + 1) * BL],
                                         u[:, :])
                    hh = ut.tile([D, BL], bf16, tag="ahh")
                    nc.scalar.activation(hh[:, :], pxh3[:, s, :], TANH,
                                         bias=ab_h)
                    hnew = hp.tile([D, BL], bf16, tag="hA")
                    if t > 0:
                        dd = ut.tile([D, BL], bf16, tag="add")
                        nc.vector.tensor_sub(dd[:, :], hh[:, :], hAg)
                        ud = ut.tile([D, BL], bf16, tag="aud")
                        nc.vector.tensor_mul(ud[:, :], up[:, :], dd[:, :])
                        nc.gpsimd.tensor_add(hnew[:, :], hAg, ud[:, :])
                        hA_pp[0], ud_prev[0] = hAg, ud
                    else:
                        nc.vector.tensor_mul(hnew[:, :], up[:, :], hh[:, :])
                    hA[0] = hnew
                    # --- PE block B: hoisted next-chunk work -------------
                    if s == 1 and c + 1 < NCG:
                        aug_proj(c + 1)

        # ---------------- output layer ---------------------------------
        with tc.tile_pool(name="ops", bufs=1, space="PSUM") as ops, \
             tc.tile_pool(name="ot", bufs=1) as ot:
            ih = ot.tile([D, BL], bf16)
            nc.vector.tensor_mul(ih[:, :], qT[:, :], histF[:, :])
            po = ops.tile([1, BL], f32)
            pieces = [userT, qT, histT, ih, hA[0]]
            for g, piece in enumerate(pieces):
                _mm(nc, po[:, :], outWT[:, g:g + 1], piece,
                    start=(g == 0), stop=(g == 4))
            outs = ot.tile([1, BL], f32)
            nc.scalar.activation(outs[:, :], po[:, :], AF.Identity,
                                 bias=outb[:, 0:1])
            nc.sync.dma_start(outd[:, :], outs[:, :])

    nc.finalize()
    return nc


_NC = None


def _get_nc():
    global _NC
    if _NC is None:
        _NC = build_nc()
    return _NC


def make_in_maps(inputs):
    """Slice full inputs into per-core input maps (host-side layout only)."""
    f = {k: np.asarray(v) for k, v in inputs.items()}
    WihT = np.ascontiguousarray(f["gru_Wih"].T)          # (128, 384)
    WhhT = np.ascontiguousarray(f["gru_Whh"].T)
    bihT = np.ascontiguousarray(f["gru_bih"].reshape(3, D).T)  # (128, 3)
    bhhT = np.ascontiguousarray(f["gru_bhh"].reshape(3, D).T)
    # fold the (q - fact) block into the fact/q blocks:
    # W0_f@f + W0_q@q + W0_fq@(f*q) + W0_qf@(q-f)
    #   = (W0_f - W0_qf)@f + (W0_q + W0_qf)@q + W0_fq@(f*q)
    W0 = f["attn_W0"]                                    # (80, 4D)
    W0f = W0[:, 0:D] - W0[:, 3 * D:4 * D]
    W0q = W0[:, D:2 * D] + W0[:, 3 * D:4 * D]
    W0fq = W0[:, 2 * D:3 * D]
    W0T = np.ascontiguousarray(
        np.concatenate([W0f.T, W0q.T, W0fq.T], axis=1))  # (D, 240)
    b0 = np.ascontiguousarray(f["attn_b0"].reshape(80, 1))
    W1T = np.ascontiguousarray(f["attn_W1"].T)           # (80, 40)
    b1 = np.ascontiguousarray(f["attn_b1"].reshape(40, 1))
    W2T = np.ascontiguousarray(f["attn_W2"].T)           # (40, 1)
    b2 = f["attn_b2"].reshape(1, 1)
    augW = np.concatenate(
        [np.ascontiguousarray(f[k][:, p * D:(p + 1) * D].T)
         for k in ("aug_Wr", "aug_Wu", "aug_Wh") for p in (0, 1)],
        axis=1)                                          # (128, 768)
    augb = np.stack([f["aug_br"], f["aug_bu"], f["aug_bh"]], axis=1)  # (128,3)
    outWT = np.ascontiguousarray(f["out_W"].reshape(5, D).T)          # (128,5)
    outb = f["out_b"].reshape(1, 1)

    shared_bf = dict(WihT=WihT, WhhT=WhhT, W0T=W0T, W1T=W1T, W2T=W2T,
                     augW=augW, outWT=outWT)
    shared = dict(bihT=bihT, bhhT=bhhT, b0=b0, b1=b1, b2=b2, augb=augb,
                  outb=outb)
    shared = {k: np.ascontiguousarray(v.astype(np.float32)) for k, v in
              shared.items()}
    shared.update({k: np.ascontiguousarray(v.astype(BF)) for k, v in
                   shared_bf.items()})

    in_maps = []
    for c in range(NCORES):
        s = slice(c * BL, (c + 1) * BL)
        m = dict(shared)
        m["hist"] = np.ascontiguousarray(
            f["item_historical_embedding"][s].astype(BF))
        m["item"] = np.ascontiguousarray(f["item_embedding"][s].astype(np.float32))
        m["user"] = np.ascontiguousarray(f["user_embedding"][s].astype(np.float32))
        m["maskd"] = np.ascontiguousarray(f["mask"][s].astype(np.float32))
        m["seqd"] = np.ascontiguousarray(
            f["sequential_length"][s].reshape(BL, 1).astype(np.int32))
        in_maps.append(m)
    return in_maps


def kernel(**inputs) -> np.ndarray:
    nc = _get_nc()
    in_maps = make_in_maps(inputs)
    res = run_bass_kernel_spmd(nc, in_maps, list(range(NCORES)))
    return np.concatenate(
        [np.asarray(res.results[c]["out"]).reshape(BL) for c in range(NCORES)])


# revision 19
# speedup vs baseline: 1.1603x; 1.0131x over previous
"""DIEN (GRU + attention + AUGRU) Trainium2 kernel.

Data-parallel over 8 NeuronCores: each core handles a batch slice of 64.
All on-chip state is feature-major (feature dim on SBUF partitions, batch
on the free dim), so the two sequential recurrences need no per-step
transposes.

Structure:
  - X^T (D, T*64) in bf16, built via PE transposes of natural-layout DMA
    tiles; the same stationary tiles also accumulate the masked history
    sum (f32) via N=1 matmuls against 1/len-scaled mask columns.
  - GRU input projections land in PSUM chunks laid out (128, 4, [r|z|n]).
    Gate sigmoids are split per-gate so the gate bias rides the ACT
    instruction (no DVE bias adds), and (1-z) comes from sigmoid(-x).
    The combine uses h' = (1-z)*n + z*h with z*h computed off the
    critical path.
  - Attention runs per 8-step chunk interleaved with the GRU loop; its
    elementwise prep (fact*q, q-fact) runs on the idle GPSIMD engine so
    the DVE stays free for the recurrence chain.
  - AUGRU consumes attention weights via rows pre-transposed once after
    the softmax; per chunk a single ones-weight load broadcasts 4 rows
    into PSUM. Masked steps have weight 0 (u'=0, h unchanged), so the
    final AUGRU state IS the gathered aug_out[b, len-1] - no gather.
  - Matmuls are bf16 (f32 PSUM accumulation); elementwise recurrence ops
    run bf16 where both operands sit in SBUF (2x DVE mode); the masked
    history mean and softmax stay f32.
"""

import sys

if "/opt/trn_rl_repo" not in sys.path:
    sys.path.insert(0, "/opt/trn_rl_repo")

from contextlib import ExitStack

import ml_dtypes
import numpy as np

import concourse.bacc as bacc
import concourse.bass as bass
import concourse.mybir as mybir
import concourse.tile as tile
from concourse.bass_utils import run_bass_kernel_spmd
from concourse.masks import make_identity

f32 = mybir.dt.float32
bf16 = mybir.dt.bfloat16
i32 = mybir.dt.int32
AF = mybir.ActivationFunctionType
ALU = mybir.AluOpType
BF = ml_dtypes.bfloat16

NCORES = 8
B, T, D = 512, 200, 128
BL = B // NCORES          # 64 batch rows per core
CG = 4                    # GRU/AUGRU psum chunk: steps per chunk
NCG = T // CG             # 50
WG = CG * BL              # 256
CH = 8                    # attention chunk: steps per chunk
NCHUNK = T // CH          # 25
W = CH * BL               # 512
G3 = 3 * D


def _mm(nc, out, lhsT, rhs, start, stop, skip=False):
    nc.tensor.matmul(out, lhsT, rhs, start=start, stop=stop,
                     skip_group_check=skip)


def build_nc():
    nc = bacc.Bacc("TRN2", target_bir_lowering=False)

    hist = nc.declare_dram_parameter("hist", [BL, T, D], bf16, isOutput=False)
    item = nc.declare_dram_parameter("item", [BL, D], f32, isOutput=False)
    user = nc.declare_dram_parameter("user", [BL, D], f32, isOutput=False)
    maskd = nc.declare_dram_parameter("maskd", [BL, T], f32, isOutput=False)
    seqd = nc.declare_dram_parameter("seqd", [BL, 1], i32, isOutput=False)
    WihTd = nc.declare_dram_parameter("WihT", [D, G3], bf16, isOutput=False)
    WhhTd = nc.declare_dram_parameter("WhhT", [D, G3], bf16, isOutput=False)
    bihTd = nc.declare_dram_parameter("bihT", [D, 3], f32, isOutput=False)
    bhhTd = nc.declare_dram_parameter("bhhT", [D, 3], f32, isOutput=False)
    W0Td = nc.declare_dram_parameter("W0T", [D, 240], bf16, isOutput=False)
    b0d = nc.declare_dram_parameter("b0", [80, 1], f32, isOutput=False)
    W1Td = nc.declare_dram_parameter("W1T", [80, 40], bf16, isOutput=False)
    b1d = nc.declare_dram_parameter("b1", [40, 1], f32, isOutput=False)
    W2Td = nc.declare_dram_parameter("W2T", [40, 1], bf16, isOutput=False)
    b2d = nc.declare_dram_parameter("b2", [1, 1], f32, isOutput=False)
    augWd = nc.declare_dram_parameter("augW", [D, 6 * D], bf16, isOutput=False)
    augbd = nc.declare_dram_parameter("augb", [D, 3], f32, isOutput=False)
    outWTd = nc.declare_dram_parameter("outWT", [D, 5], bf16, isOutput=False)
    outbd = nc.declare_dram_parameter("outb", [1, 1], f32, isOutput=False)
    outd = nc.declare_dram_parameter("out", [1, BL], f32, isOutput=True)

    with tile.TileContext(nc) as tc, ExitStack() as ctx:
        big = ctx.enter_context(tc.tile_pool(name="big", bufs=1))
        wp = ctx.enter_context(tc.tile_pool(name="wp", bufs=1))
        pp = ctx.enter_context(tc.tile_pool(name="pp", bufs=1))
        hp = ctx.enter_context(tc.tile_pool(name="hp", bufs=4))

        XT = big.tile([D, T * BL], bf16)
        gruT = big.tile([D, T * BL], bf16)

        ident = pp.tile([128, 128], f32)
        make_identity(nc, ident)
        identB = pp.tile([BL, BL], bf16)
        make_identity(nc, identB)
        identB128 = pp.tile([128, 128], bf16)
        make_identity(nc, identB128)

        # ------------- weights (bf16 arrive pre-converted via DMA) -------
        WihT = wp.tile([D, G3], bf16)
        nc.sync.dma_start(WihT[:, :], WihTd[:, :])
        WhhT = wp.tile([D, G3], bf16)
        nc.sync.dma_start(WhhT[:, :], WhhTd[:, :])
        W0T = wp.tile([D, 240], bf16)
        nc.sync.dma_start(W0T[:, :], W0Td[:, :])
        W1T = wp.tile([80, 40], bf16)
        nc.sync.dma_start(W1T[:, :], W1Td[:, :])
        W2T = wp.tile([40, 1], bf16)
        nc.sync.dma_start(W2T[:, :], W2Td[:, :])
        augW = wp.tile([D, 6 * D], bf16)
        nc.sync.dma_start(augW[:, :], augWd[:, :])
        outWT = wp.tile([D, 5], bf16)
        nc.sync.dma_start(outWT[:, :], outWTd[:, :])
        WrhT, WrxT = augW[:, 0:D], augW[:, D:2 * D]
        WuhT, WuxT = augW[:, 2 * D:3 * D], augW[:, 3 * D:4 * D]
        WahT, WaxT = augW[:, 4 * D:5 * D], augW[:, 5 * D:6 * D]

        bihT = wp.tile([D, 3], f32)
        nc.sync.dma_start(bihT[:, :], bihTd[:, :])
        bhhT = wp.tile([D, 3], f32)
        nc.sync.dma_start(bhhT[:, :], bhhTd[:, :])
        brz = wp.tile([D, 2], f32)
        nc.vector.tensor_add(brz[:, :], bihT[:, 0:2], bhhT[:, 0:2])
        nbz = wp.tile([D, 1], f32)
        nc.vector.tensor_scalar(nbz[:, :], brz[:, 1:2], -1.0, 0.0,
                                ALU.mult, ALU.add)
        b_hn, b_in = bhhT[:, 2:3], bihT[:, 2:3]

        b0v = wp.tile([80, 1], f32)
        nc.sync.dma_start(b0v[:, :], b0d[:, :])
        b1v = wp.tile([40, 1], f32)
        nc.sync.dma_start(b1v[:, :], b1d[:, :])
        b2v = wp.tile([1, 1], f32)
        nc.sync.dma_start(b2v[:, :], b2d[:, :])
        augb = wp.tile([D, 3], f32)
        nc.sync.dma_start(augb[:, :], augbd[:, :])
        outb = wp.tile([1, 1], f32)
        nc.sync.dma_start(outb[:, :], outbd[:, :])
        ab_r, ab_u, ab_h = augb[:, 0:1], augb[:, 1:2], augb[:, 2:3]

        ones_f = wp.tile([1, 128], f32)
        nc.vector.memset(ones_f[:, :], 1.0)
        ones1 = wp.tile([1, 128], bf16)
        nc.vector.memset(ones1[:, :], 1.0)

        # ---------------- preprocessing --------------------------------
        histT = pp.tile([D, BL], bf16)
        qT = pp.tile([D, BL], bf16)
        userT = pp.tile([D, BL], bf16)
        qRep = pp.tile([D, W], bf16)
        maskS = pp.tile([BL, T], f32)
        b2col = pp.tile([BL, 1], f32)
        histF = pp.tile([D, BL], f32)

        with tc.tile_pool(name="pre", bufs=4) as pre, \
             tc.tile_pool(name="pps", bufs=2, space="PSUM") as pps, \
             tc.tile_pool(name="hps", bufs=1, space="PSUM") as hps:
            nc.sync.dma_start(maskS[:, :], maskd[:, :])
            seqi = pre.tile([BL, 1], i32)
            nc.sync.dma_start(seqi[:, :], seqd[:, :])
            seqf = pre.tile([BL, 1], f32)
            nc.vector.tensor_copy(seqf[:, :], seqi[:, :])
            rsec = pre.tile([BL, 1], f32)
            nc.vector.reciprocal(rsec[:, :], seqf[:, :])
            maskSc = pre.tile([BL, T], f32)
            nc.vector.tensor_scalar_mul(maskSc[:, :], maskS[:, :], rsec[:, 0:1])

            mstp0 = pps.tile([128, BL], f32, tag="mstp")
            nc.tensor.transpose(mstp0[:, :], maskSc[:, 0:128], ident[0:BL, 0:BL])
            mst0 = pre.tile([128, BL], bf16, tag="mst0", bufs=1)
            nc.scalar.copy(mst0[:, :], mstp0[:, :])
            mstp1 = pps.tile([128, BL], f32, tag="mstp")
            nc.tensor.transpose(mstp1[0:72, :], maskSc[:, 128:200], ident[0:BL, 0:BL])
            mst1 = pre.tile([72, BL], bf16, tag="mst1", bufs=1)
            nc.scalar.copy(mst1[:, :], mstp1[0:72, :])

            itn = pre.tile([BL, D], f32, tag="itn")
            nc.sync.dma_start(itn[:, :], item[:, :])
            itp = pps.tile([D, BL], f32, tag="small_t")
            nc.tensor.transpose(itp[:, :], itn[:, :], ident[0:BL, 0:BL])
            nc.scalar.copy(qT[:, :], itp[:, :])
            usn = pre.tile([BL, D], f32, tag="itn")
            nc.sync.dma_start(usn[:, :], user[:, :])
            usp = pps.tile([D, BL], f32, tag="small_t")
            nc.tensor.transpose(usp[:, :], usn[:, :], ident[0:BL, 0:BL])
            nc.scalar.copy(userT[:, :], usp[:, :])
            b2p = pps.tile([BL, 1], f32, tag="small_t")
            _mm(nc, b2p[:, :], ones_f[0:1, 0:BL], b2v, start=True, stop=True)
            nc.scalar.copy(b2col[:, :], b2p[:, :])
            for s in range(CH):
                nc.scalar.copy(qRep[:, s * BL:(s + 1) * BL], qT[:, :])

            # X^T build + masked history sum. First-half (t<128) tiles for
            # ALL b come first: GRU chunk 0 only depends on those, so the
            # second half and the history matmuls overlap the running GRU.
            histp0 = hps.tile([D, BL], f32, tag="histp0")
            histp1 = hps.tile([D, BL], f32, tag="histp1")
            XT3 = XT.rearrange("p (t b) -> p t b", b=BL)
            for h, (t0, tn) in enumerate(((0, 128), (128, 72))):
                histp = histp0 if h == 0 else histp1
                for b in range(BL):
                    xn = pre.tile([128, D], bf16, tag="xn")
                    nc.sync.dma_start(xn[0:tn, :], hist[b, t0:t0 + tn, :])
                    xtp = pps.tile([D, 128], bf16, tag="xtp")
                    nc.tensor.transpose(xtp[:, 0:tn], xn[0:tn, :],
                                        identB128[0:tn, 0:tn])
                    dst = XT3[:, t0:t0 + tn, b]
                    if b % 2 == 0:
                        nc.scalar.copy(dst, xtp[:, 0:tn])
                    else:
                        nc.vector.tensor_copy(dst, xtp[:, 0:tn])
                    mcol = (mst0 if h == 0 else mst1)[0:tn, b:b + 1]
                    _mm(nc, histp[:, b:b + 1], xn[0:tn, :], mcol,
                        start=True, stop=True)
            nc.scalar.copy(histF[:, :], histp0[:, :])
            nc.vector.tensor_add(histF[:, :], histF[:, :], histp1[:, :])
            nc.scalar.copy(histT[:, :], histF[:, :])

        # ---------------- GRU + interleaved attention -------------------
        gruT3 = gruT.rearrange("p (t b) -> p t b", b=BL)
        SIG, TANH = AF.Sigmoid, AF.Tanh
        wgt = pp.tile([BL, T], f32)

        with tc.tile_pool(name="gip", bufs=2, space="PSUM") as gip, \
             tc.tile_pool(name="hnp", bufs=1, space="PSUM") as hnp, \
             tc.tile_pool(name="gt", bufs=6) as gt, \
             tc.tile_pool(name="aps", bufs=1, space="PSUM") as aps, \
             tc.tile_pool(name="sps", bufs=1, space="PSUM") as sps, \
             tc.tile_pool(name="at", bufs=3) as at:
            scp = sps.tile([BL, T], f32)

            def attention_pieces(a):
                """Attention for gru chunks [2a, 2a+1] as 8 emission slots;
                each slot is small PE/DVE/Pool work sized to fill one GRU
                step's engine-idle window without head-of-line-blocking the
                recurrence chain's FIFO queues."""
                gc = gruT[:, a * W:(a + 1) * W]
                r2 = at.tile([D, W], bf16, tag="r2")
                y0p = aps.tile([80, W], f32, tag="y0")
                y0 = at.tile([80, W], bf16, tag="y0s")
                y1p = aps.tile([40, W], f32, tag="y1")
                y1 = at.tile([40, W], bf16, tag="y1s")
                HW = 256

                def relu0(q):
                    nc.vector.tensor_scalar(y0[:, q * 128:(q + 1) * 128],
                                            y0p[:, q * 128:(q + 1) * 128],
                                            b0v[:, 0:1], 0.0, ALU.add, ALU.max)
                def relu1(h):
                    nc.vector.tensor_scalar(y1[:, h * HW:(h + 1) * HW],
                                            y1p[:, h * HW:(h + 1) * HW],
                                            b1v[:, 0:1], 0.0, ALU.add, ALU.max)
                def score(s):
                    t = a * CH + s
                    _mm(nc, scp[:, t:t + 1], y1[:, s * BL:(s + 1) * BL],
                        W2T, start=True, stop=True)

                def p0():
                    nc.gpsimd.tensor_mul(r2[:, :], gc, qRep[:, :])
                    _mm(nc, y0p[:, 0:HW], W0T[:, 0:80], gc[:, 0:HW],
                        start=True, stop=False)
                def p1():
                    _mm(nc, y0p[:, HW:W], W0T[:, 0:80], gc[:, HW:W],
                        start=True, stop=False)
                    _mm(nc, y0p[:, 0:HW], W0T[:, 80:160], qRep[:, 0:HW],
                        start=False, stop=False)
                def p2():
                    _mm(nc, y0p[:, HW:W], W0T[:, 80:160], qRep[:, HW:W],
                        start=False, stop=False)
                    _mm(nc, y0p[:, 0:HW], W0T[:, 160:240], r2[:, 0:HW],
                        start=False, stop=True)
                def p3():
                    _mm(nc, y0p[:, HW:W], W0T[:, 160:240], r2[:, HW:W],
                        start=False, stop=True)
                    relu0(0)
                def p4():
                    relu0(1)
                    relu0(2)
                def p5():
                    relu0(3)
                    _mm(nc, y1p[:, 0:HW], W1T, y0[:, 0:HW],
                        start=True, stop=True)
                def p6():
                    _mm(nc, y1p[:, HW:W], W1T, y0[:, HW:W],
                        start=True, stop=True)
                    relu1(0)
                    for s in range(2):
                        score(s)
                def p7():
                    relu1(1)
                    for s in range(2, CH):
                        score(s)
                return [p0, p1, p2, p3, p4, p5, p6, p7]

            pending = []          # attention pieces awaiting emission
            WhT_r, WhT_z, WhT_n = (WhhT[:, 0:D], WhhT[:, D:2 * D],
                                   WhhT[:, 2 * D:G3])
            gslot = [None, None]  # (prz, prz4, pn3) per psum buf

            def gru_proj_pieces(c):
                XTc = XT[:, c * WG:(c + 1) * WG]
                prz = gip.tile([D, CG * 128], f32, tag="girz")
                pn = gip.tile([D, CG * BL], f32, tag="gin")
                gslot[c % 2] = (
                    prz,
                    prz.rearrange("p (g s c) -> p g s c", g=2, c=BL),
                    pn.rearrange("p (s g) -> p s g", g=BL),
                )
                return [
                    lambda: _mm(nc, prz[:, 0:WG], WihT[:, 0:D], XTc,
                                start=True, stop=True),
                    lambda: _mm(nc, prz[:, WG:2 * WG], WihT[:, D:2 * D], XTc,
                                start=True, stop=True),
                    lambda: _mm(nc, pn[:, :], WihT[:, 2 * D:G3], XTc,
                                start=True, stop=True),
                ]

            for p in gru_proj_pieces(0):
                p()
            zh_prev = [None]      # z_{t-1} * h_{t-2}
            omzn_prev = [None]    # (1-z_{t-1}) * n_{t-1}
            for c in range(NCG):
                prz, prz4, pn3 = gslot[c % 2]
                for s in range(CG):
                    t = c * CG + s
                    # --- PE block A: recurrent matmuls feeding step t ----
                    hx = hnp.tile([D, 2 * BL], f32, tag="hn2")
                    hn, tmp2 = hx[:, 0:BL], hx[:, BL:2 * BL]
                    if t == 1:
                        h0 = gruT3[:, 0, :]
                        _mm(nc, prz[:, s * BL:(s + 1) * BL], WhT_r, h0,
                            start=False, stop=True, skip=True)
                        _mm(nc, prz[:, WG + s * BL:WG + (s + 1) * BL],
                            WhT_z, h0, start=False, stop=True, skip=True)
                        _mm(nc, hn[:, :], WhT_n, h0, start=True, stop=True)
                    elif t > 1:
                        rsl = prz[:, s * BL:(s + 1) * BL]
                        _mm(nc, rsl, WhT_r, zh_prev[0],
                            start=False, stop=False, skip=True)
                        _mm(nc, rsl, WhT_r, omzn_prev[0],
                            start=False, stop=True, skip=True)
                        hprev = gruT3[:, t - 1, :]
                        _mm(nc, prz[:, WG + s * BL:WG + (s + 1) * BL],
                            WhT_z, hprev, start=False, stop=True, skip=True)
                        _mm(nc, hn[:, :], WhT_n, hprev, start=True, stop=True)
                    # --- gate activations (biases ride ACT) --------------
                    r = gt.tile([D, BL], bf16, tag="r")
                    nc.scalar.activation(r[:, :], prz4[:, 0, s, :], SIG,
                                         bias=brz[:, 0:1])
                    z = gt.tile([D, BL], bf16, tag="z")
                    nc.scalar.activation(z[:, :], prz4[:, 1, s, :], SIG,
                                         bias=brz[:, 1:2])
                    omz = gt.tile([D, BL], bf16, tag="omz")
                    nc.gpsimd.tensor_scalar(omz[:, :], z[:, :], -1.0, 1.0,
                                            ALU.mult, ALU.add)
                    tmp = gt.tile([D, BL], bf16, tag="tmp")
                    if t > 0:
                        nc.vector.scalar_tensor_tensor(
                            tmp[:, :], hn[:, :], b_hn, r[:, :],
                            ALU.add, ALU.mult)
                    else:
                        nc.vector.tensor_scalar_mul(tmp[:, :], r[:, :], b_hn)
                    nc.vector.tensor_add(tmp2[:, :], tmp[:, :], pn3[:, s, :])
                    if t > 0:
                        zh = gt.tile([D, BL], bf16, tag="zh")
                        nc.vector.tensor_mul(zh[:, :], z[:, :],
                                             gruT3[:, t - 1, :])
                    n = gt.tile([D, BL], bf16, tag="n")
                    nc.scalar.activation(n[:, :], tmp2[:, :], TANH, bias=b_in)
                    hdst = gruT3[:, t, :]
                    if t > 0:
                        omzn = gt.tile([D, BL], bf16, tag="omzn")
                        nc.vector.tensor_mul(omzn[:, :], omz[:, :], n[:, :])
                        nc.vector.tensor_add(hdst, omzn[:, :], zh[:, :])
                        zh_prev[0], omzn_prev[0] = zh, omzn
                    else:
                        nc.vector.tensor_mul(hdst, omz[:, :], n[:, :])
                    # --- PE block B: hoisted chunk/attention work --------
                    if s == 1 and c + 1 < NCG:
                        proj_pieces = gru_proj_pieces(c + 1)
                    if s >= 1 and c + 1 < NCG:
                        proj_pieces[s - 1]()
                    if pending:
                        pending.pop(0)()
                if c % 2 == 1:
                    pending = attention_pieces(c // 2)
            for p in pending:
                p()

            # masked softmax over t (b-major); last MLP layer ReLU'd w/ b2
            rawr = at.tile([BL, T], f32, tag="rawr")
            nc.scalar.activation(rawr[:, :], scp[:, :], AF.Relu,
                                 bias=b2col[:, 0:1])
            rawm = at.tile([BL, T], f32, tag="rawm")
            nc.vector.tensor_mul(rawm[:, :], rawr[:, :], maskS[:, :])
            mxn = at.tile([BL, 1], f32, tag="mxn")
            nc.vector.tensor_reduce(mxn[:, :], rawm[:, :],
                                    axis=mybir.AxisListType.X,
                                    op=ALU.max, negate=True)
            ex = at.tile([BL, T], f32, tag="ex")
            nc.scalar.activation(ex[:, :], rawr[:, :], AF.Exp, bias=mxn[:, 0:1])
            em = at.tile([BL, T], f32, tag="em")
            nc.vector.tensor_mul(em[:, :], ex[:, :], maskS[:, :])
            sm = at.tile([BL, 1], f32, tag="sm")
            nc.vector.tensor_reduce(sm[:, :], em[:, :],
                                    axis=mybir.AxisListType.X, op=ALU.add)
            rs = at.tile([BL, 1], f32, tag="rs")
            nc.vector.reciprocal(rs[:, :], sm[:, :])
            nc.vector.tensor_scalar_mul(wgt[:, :], em[:, :], rs[:, 0:1])

        # attention weights go to partition-0 bf16 rows per AUGRU chunk
        # (the row transposes + broadcasts overlap the AUGRU itself)
        wgt_bf = pp.tile([BL, T], bf16)
        nc.scalar.copy(wgt_bf[:, :], wgt[:, :])

        # ---------------- AUGRU ----------------------------------------
        hA = [None]
        with tc.tile_pool(name="axp", bufs=2, space="PSUM") as axp, \
             tc.tile_pool(name="abp", bufs=2, space="PSUM") as abp, \
             tc.tile_pool(name="rtp", bufs=2, space="PSUM") as rtp, \
             tc.tile_pool(name="ut", bufs=6) as ut:
            aslot = [None, None]  # (pru, pru4, pxh3, pab) per psum buf

            def aug_proj_pieces(c):
                gc = gruT[:, c * WG:(c + 1) * WG]
                rowp = rtp.tile([1, WG], f32, tag="rowp")
                rows = ut.tile([1, WG], bf16, tag="rows")
                pru = axp.tile([D, CG * 128], f32, tag="pxru")
                pxh = axp.tile([D, CG * BL], f32, tag="pxh")
                pab = abp.tile([D, WG], f32, tag="pab")
                aslot[c % 2] = (
                    pru,
                    pru.rearrange("p (g s c) -> p g s c", g=2, c=BL),
                    pxh.rearrange("p (s g) -> p s g", g=BL),
                    pab,
                )

                def pa():
                    for s in range(CG):
                        t = c * CG + s
                        _mm(nc, rowp[0:1, s * BL:(s + 1) * BL],
                            wgt_bf[:, t:t + 1], identB, start=True, stop=True)
                    nc.vector.tensor_copy(rows[0:1, :], rowp[0:1, :])
                def pb():
                    _mm(nc, pru[:, 0:WG], WrxT, gc, start=True, stop=True)
                    _mm(nc, pab[:, :], ones1, rows[0:1, :],
                        start=True, stop=True)
                def pc():
                    _mm(nc, pru[:, WG:2 * WG], WuxT, gc, start=True, stop=True)
                def pd():
                    _mm(nc, pxh[:, :], WaxT, gc, start=True, stop=True)
                return [pa, pb, pc, pd]

            for p in aug_proj_pieces(0):
                p()
            hA_pp = [None]        # hA_{t-2}
            ud_prev = [None]      # ud_{t-1}
            for c in range(NCG):
                pru, pru4, pxh3, pab = aslot[c % 2]
                for s in range(CG):
                    t = c * CG + s
                    hAg = hA[0]
                    # --- PE block A: recurrent matmuls feeding step t ----
                    rsl = pru[:, s * BL:(s + 1) * BL]
                    usl = pru[:, WG + s * BL:WG + (s + 1) * BL]
                    if t == 1:
                        _mm(nc, rsl, WrhT, hAg, start=False, stop=True,
                            skip=True)
                        _mm(nc, usl, WuhT, hAg, start=False, stop=True,
                            skip=True)
                    elif t > 1:
                        _mm(nc, rsl, WrhT, hA_pp[0],
                            start=False, stop=False, skip=True)
                        _mm(nc, rsl, WrhT, ud_prev[0],
                            start=False, stop=True, skip=True)
                        _mm(nc, usl, WuhT, hA_pp[0],
                            start=False, stop=False, skip=True)
                        _mm(nc, usl, WuhT, ud_prev[0],
                            start=False, stop=True, skip=True)
                    r = ut.tile([D, BL], bf16, tag="ar")
                    nc.scalar.activation(r[:, :], pru4[:, 0, s, :], SIG,
                                         bias=ab_r)
                    u = ut.tile([D, BL], bf16, tag="au")
                    nc.scalar.activation(u[:, :], pru4[:, 1, s, :], SIG,
                                         bias=ab_u)
                    if t > 0:
                        hr = ut.tile([D, BL], bf16, tag="ahr")
                        nc.vector.tensor_mul(hr[:, :], hAg, r[:, :])
                        _mm(nc, pxh3[:, s, :], WahT, hr,
                            start=False, stop=True, skip=True)
                    up = ut.tile([D, BL], f32, tag="aup")
                    nc.vector.tensor_mul(up[:, :], pab[:, s * BL:(s + 1) * BL],
                                         u[:, :])
                    hh = ut.tile([D, BL], bf16, tag="ahh")
                    nc.scalar.activation(hh[:, :], pxh3[:, s, :], TANH,
                                         bias=ab_h)
                    hnew = hp.tile([D, BL], bf16, tag="hA")
                    if t > 0:
                        dd = ut.tile([D, BL], bf16, tag="add")
                        nc.vector.tensor_sub(dd[:, :], hh[:, :], hAg)
                        ud = ut.tile([D, BL], bf16, tag="aud")
                        nc.vector.tensor_mul(ud[:, :], up[:, :], dd[:, :])
                        nc.gpsimd.tensor_add(hnew[:, :], hAg, ud[:, :])
                        hA_pp[0], ud_prev[0] = hAg, ud
                    else:
                        nc.vector.tensor_mul(hnew[:, :], up[:, :], hh[:, :])
                    hA[0] = hnew
                    # --- PE block B: hoisted next-chunk work -------------
                    if c + 1 < NCG:
                        if s == 0:
                            aproj_pieces = aug_proj_pieces(c + 1)
                        aproj_pieces[s]()

        # ---------------- output layer ---------------------------------
        with tc.tile_pool(name="ops", bufs=1, space="PSUM") as ops, \
             tc.tile_pool(name="ot", bufs=1) as ot:
            ih = ot.tile([D, BL], bf16)
            nc.vector.tensor_mul(ih[:, :], qT[:, :], histF[:, :])
            po = ops.tile([1, BL], f32)
            pieces = [userT, qT, histT, ih, hA[0]]
            for g, piece in enumerate(pieces):
                _mm(nc, po[:, :], outWT[:, g:g + 1], piece,
                    start=(g == 0), stop=(g == 4))
            outs = ot.tile([1, BL], f32)
            nc.scalar.activation(outs[:, :], po[:, :], AF.Identity,
                                 bias=outb[:, 0:1])
            nc.sync.dma_start(outd[:, :], outs[:, :])

    nc.finalize()
    return nc


_NC = None


def _get_nc():
    global _NC
    if _NC is None:
        _NC = build_nc()
    return _NC


def make_in_maps(inputs):
    """Slice full inputs into per-core input maps (host-side layout only)."""
    f = {k: np.asarray(v) for k, v in inputs.items()}
    WihT = np.ascontiguousarray(f["gru_Wih"].T)          # (128, 384)
    WhhT = np.ascontiguousarray(f["gru_Whh"].T)
    bihT = np.ascontiguousarray(f["gru_bih"].reshape(3, D).T)  # (128, 3)
    bhhT = np.ascontiguousarray(f["gru_bhh"].reshape(3, D).T)
    # fold the (q - fact) block into the fact/q blocks:
    # W0_f@f + W0_q@q + W0_fq@(f*q) + W0_qf@(q-f)
    #   = (W0_f - W0_qf)@f + (W0_q + W0_qf)@q + W0_fq@(f*q)
    W0 = f["attn_W0"]                                    # (80, 4D)
    W0f = W0[:, 0:D] - W0[:, 3 * D:4 * D]
    W0q = W0[:, D:2 * D] + W0[:, 3 * D:4 * D]
    W0fq = W0[:, 2 * D:3 * D]
    W0T = np.ascontiguousarray(
        np.concatenate([W0f.T, W0q.T, W0fq.T], axis=1))  # (D, 240)
    b0 = np.ascontiguousarray(f["attn_b0"].reshape(80, 1))
    W1T = np.ascontiguousarray(f["attn_W1"].T)           # (80, 40)
    b1 = np.ascontiguousarray(f["attn_b1"].reshape(40, 1))
    W2T = np.ascontiguousarray(f["attn_W2"].T)           # (40, 1)
    b2 = f["attn_b2"].reshape(1, 1)
    augW = np.concatenate(
        [np.ascontiguousarray(f[k][:, p * D:(p + 1) * D].T)
         for k in ("aug_Wr", "aug_Wu", "aug_Wh") for p in (0, 1)],
        axis=1)                                          # (128, 768)
    augb = np.stack([f["aug_br"], f["aug_bu"], f["aug_bh"]], axis=1)  # (128,3)
    outWT = np.ascontiguousarray(f["out_W"].reshape(5, D).T)          # (128,5)
    outb = f["out_b"].reshape(1, 1)

    shared_bf = dict(WihT=WihT, WhhT=WhhT, W0T=W0T, W1T=W1T, W2T=W2T,
                     augW=augW, outWT=outWT)
    shared = dict(bihT=bihT, bhhT=bhhT, b0=b0, b1=b1, b2=b2, augb=augb,
                  outb=outb)
    shared = {k: np.ascontiguousarray(v.astype(np.float32)) for k, v in
              shared.items()}
    shared.update({k: np.ascontiguousarray(v.astype(BF)) for k, v in
                   shared_bf.items()})

    in_maps = []
    for c in range(NCORES):
        s = slice(c * BL, (c + 1) * BL)
        m = dict(shared)
        m["hist"] = np.ascontiguousarray(
            f["item_historical_embedding"][s].astype(BF))
        m["item"] = np.ascontiguousarray(f["item_embedding"][s].astype(np.float32))
        m["user"] = np.ascontiguousarray(f["user_embedding"][s].astype(np.float32))
        m["maskd"] = np.ascontiguousarray(f["mask"][s].astype(np.float32))
        m["seqd"] = np.ascontiguousarray(
            f["sequential_length"][s].reshape(BL, 1).astype(np.int32))
        in_maps.append(m)
    return in_maps


def kernel(**inputs) -> np.ndarray:
    nc = _get_nc()
    in_maps = make_in_maps(inputs)
    res = run_bass_kernel_spmd(nc, in_maps, list(range(NCORES)))
    return np.concatenate(
        [np.asarray(res.results[c]["out"]).reshape(BL) for c in range(NCORES)])


# revision 24
# speedup vs baseline: 1.1699x; 1.0082x over previous
"""DIEN (GRU + attention + AUGRU) Trainium2 kernel.

Data-parallel over 8 NeuronCores: each core handles a batch slice of 64.
All on-chip state is feature-major (feature dim on SBUF partitions, batch
on the free dim), so the two sequential recurrences need no per-step
transposes.

Structure:
  - X^T (D, T*64) in bf16, built via PE transposes of natural-layout DMA
    tiles; the same stationary tiles also accumulate the masked history
    sum (f32) via N=1 matmuls against 1/len-scaled mask columns.
  - GRU input projections land in PSUM chunks laid out (128, 4, [r|z|n]).
    Gate sigmoids are split per-gate so the gate bias rides the ACT
    instruction (no DVE bias adds), and (1-z) comes from sigmoid(-x).
    The combine uses h' = (1-z)*n + z*h with z*h computed off the
    critical path.
  - Attention runs per 8-step chunk interleaved with the GRU loop; its
    elementwise prep (fact*q, q-fact) runs on the idle GPSIMD engine so
    the DVE stays free for the recurrence chain.
  - AUGRU consumes attention weights via rows pre-transposed once after
    the softmax; per chunk a single ones-weight load broadcasts 4 rows
    into PSUM. Masked steps have weight 0 (u'=0, h unchanged), so the
    final AUGRU state IS the gathered aug_out[b, len-1] - no gather.
  - Matmuls are bf16 (f32 PSUM accumulation); elementwise recurrence ops
    run bf16 where both operands sit in SBUF (2x DVE mode); the masked
    history mean and softmax stay f32.
"""

import sys

if "/opt/trn_rl_repo" not in sys.path:
    sys.path.insert(0, "/opt/trn_rl_repo")

from contextlib import ExitStack

import ml_dtypes
import numpy as np

import concourse.bacc as bacc
import concourse.bass as bass
import concourse.mybir as mybir
import concourse.tile as tile
from concourse.bass_utils import run_bass_kernel_spmd
from concourse.masks import make_identity

f32 = mybir.dt.float32
bf16 = mybir.dt.bfloat16
i32 = mybir.dt.int32
AF = mybir.ActivationFunctionType
ALU = mybir.AluOpType
BF = ml_dtypes.bfloat16

NCORES = 8
B, T, D = 512, 200, 128
BL = B // NCORES          # 64 batch rows per core
CG = 4                    # GRU/AUGRU psum chunk: steps per chunk
NCG = T // CG             # 50
WG = CG * BL              # 256
CH = 8                    # attention chunk: steps per chunk
NCHUNK = T // CH          # 25
W = CH * BL               # 512
G3 = 3 * D


def _mm(nc, out, lhsT, rhs, start, stop, skip=False):
    nc.tensor.matmul(out, lhsT, rhs, start=start, stop=stop,
                     skip_group_check=skip)


def build_nc():
    nc = bacc.Bacc("TRN2", target_bir_lowering=False)

    hist = nc.declare_dram_parameter("hist", [BL, T, D], bf16, isOutput=False)
    histTd = nc.declare_dram_parameter("histTd", [D, T * BL], bf16,
                                       isOutput=False)
    item = nc.declare_dram_parameter("item", [BL, D], f32, isOutput=False)
    user = nc.declare_dram_parameter("user", [BL, D], f32, isOutput=False)
    maskd = nc.declare_dram_parameter("maskd", [BL, T], f32, isOutput=False)
    seqd = nc.declare_dram_parameter("seqd", [BL, 1], i32, isOutput=False)
    WihTd = nc.declare_dram_parameter("WihT", [D, G3], bf16, isOutput=False)
    WhhTd = nc.declare_dram_parameter("WhhT", [D, G3], bf16, isOutput=False)
    bihTd = nc.declare_dram_parameter("bihT", [D, 3], f32, isOutput=False)
    bhhTd = nc.declare_dram_parameter("bhhT", [D, 3], f32, isOutput=False)
    W0Td = nc.declare_dram_parameter("W0T", [D, 240], bf16, isOutput=False)
    b0d = nc.declare_dram_parameter("b0", [80, 1], f32, isOutput=False)
    W1Td = nc.declare_dram_parameter("W1T", [80, 40], bf16, isOutput=False)
    b1d = nc.declare_dram_parameter("b1", [40, 1], f32, isOutput=False)
    W2Td = nc.declare_dram_parameter("W2T", [40, 1], bf16, isOutput=False)
    b2d = nc.declare_dram_parameter("b2", [1, 1], f32, isOutput=False)
    augWd = nc.declare_dram_parameter("augW", [D, 6 * D], bf16, isOutput=False)
    augbd = nc.declare_dram_parameter("augb", [D, 3], f32, isOutput=False)
    outWTd = nc.declare_dram_parameter("outWT", [D, 5], bf16, isOutput=False)
    outbd = nc.declare_dram_parameter("outb", [1, 1], f32, isOutput=False)
    outd = nc.declare_dram_parameter("out", [1, BL], f32, isOutput=True)

    with tile.TileContext(nc) as tc, ExitStack() as ctx:
        big = ctx.enter_context(tc.tile_pool(name="big", bufs=1))
        wp = ctx.enter_context(tc.tile_pool(name="wp", bufs=1))
        pp = ctx.enter_context(tc.tile_pool(name="pp", bufs=1))
        hp = ctx.enter_context(tc.tile_pool(name="hp", bufs=4))

        XT = big.tile([D, T * BL], bf16)
        gruT = big.tile([D, T * BL], bf16)

        ident = pp.tile([128, 128], f32)
        make_identity(nc, ident)
        identB = pp.tile([BL, BL], bf16)
        make_identity(nc, identB)
        identB128 = pp.tile([128, 128], bf16)
        make_identity(nc, identB128)

        # ------------- weights (bf16 arrive pre-converted via DMA) -------
        WihT = wp.tile([D, G3], bf16)
        nc.sync.dma_start(WihT[:, :], WihTd[:, :])
        WhhT = wp.tile([D, G3], bf16)
        nc.sync.dma_start(WhhT[:, :], WhhTd[:, :])
        W0T = wp.tile([D, 240], bf16)
        nc.sync.dma_start(W0T[:, :], W0Td[:, :])
        W1T = wp.tile([80, 40], bf16)
        nc.sync.dma_start(W1T[:, :], W1Td[:, :])
        W2T = wp.tile([40, 1], bf16)
        nc.sync.dma_start(W2T[:, :], W2Td[:, :])
        augW = wp.tile([D, 6 * D], bf16)
        nc.sync.dma_start(augW[:, :], augWd[:, :])
        outWT = wp.tile([D, 5], bf16)
        nc.sync.dma_start(outWT[:, :], outWTd[:, :])
        WrhT, WrxT = augW[:, 0:D], augW[:, D:2 * D]
        WuhT, WuxT = augW[:, 2 * D:3 * D], augW[:, 3 * D:4 * D]
        WahT, WaxT = augW[:, 4 * D:5 * D], augW[:, 5 * D:6 * D]

        bihT = wp.tile([D, 3], f32)
        nc.sync.dma_start(bihT[:, :], bihTd[:, :])
        bhhT = wp.tile([D, 3], f32)
        nc.sync.dma_start(bhhT[:, :], bhhTd[:, :])
        brz = wp.tile([D, 2], f32)
        nc.vector.tensor_add(brz[:, :], bihT[:, 0:2], bhhT[:, 0:2])
        nbz = wp.tile([D, 1], f32)
        nc.vector.tensor_scalar(nbz[:, :], brz[:, 1:2], -1.0, 0.0,
                                ALU.mult, ALU.add)
        b_hn, b_in = bhhT[:, 2:3], bihT[:, 2:3]

        b0v = wp.tile([80, 1], f32)
        nc.sync.dma_start(b0v[:, :], b0d[:, :])
        b1v = wp.tile([40, 1], f32)
        nc.sync.dma_start(b1v[:, :], b1d[:, :])
        b2v = wp.tile([1, 1], f32)
        nc.sync.dma_start(b2v[:, :], b2d[:, :])
        augb = wp.tile([D, 3], f32)
        nc.sync.dma_start(augb[:, :], augbd[:, :])
        outb = wp.tile([1, 1], f32)
        nc.sync.dma_start(outb[:, :], outbd[:, :])
        ab_r, ab_u, ab_h = augb[:, 0:1], augb[:, 1:2], augb[:, 2:3]

        ones_f = wp.tile([1, 128], f32)
        nc.vector.memset(ones_f[:, :], 1.0)
        ones1 = wp.tile([1, 128], bf16)
        nc.vector.memset(ones1[:, :], 1.0)

        # ---------------- preprocessing --------------------------------
        histT = pp.tile([D, BL], bf16)
        qT = pp.tile([D, BL], bf16)
        userT = pp.tile([D, BL], bf16)
        qRep = pp.tile([D, W], bf16)
        maskS = pp.tile([BL, T], f32)
        b2col = pp.tile([BL, 1], f32)
        histF = pp.tile([D, BL], f32)

        with tc.tile_pool(name="pre", bufs=8) as pre, \
             tc.tile_pool(name="pps", bufs=2, space="PSUM") as pps, \
             tc.tile_pool(name="hps", bufs=1, space="PSUM") as hps:
            # X^T arrives pre-transposed from the host; split the DMA so the
            # first GRU chunks can start as soon as their columns land.
            nc.sync.dma_start(XT[:, 0:4 * WG], histTd[:, 0:4 * WG])
            nc.sync.dma_start(XT[:, 4 * WG:25 * WG], histTd[:, 4 * WG:25 * WG])
            nc.sync.dma_start(XT[:, 25 * WG:T * BL], histTd[:, 25 * WG:T * BL])

            nc.sync.dma_start(maskS[:, :], maskd[:, :])
            seqi = pre.tile([BL, 1], i32, tag="seqi", bufs=1)
            nc.sync.dma_start(seqi[:, :], seqd[:, :])
            seqf = pre.tile([BL, 1], f32, tag="seqf", bufs=1)
            nc.vector.tensor_copy(seqf[:, :], seqi[:, :])
            rsec = pre.tile([BL, 1], f32, tag="rsec", bufs=1)
            nc.vector.reciprocal(rsec[:, :], seqf[:, :])
            maskSc = pre.tile([BL, T], f32, tag="maskSc", bufs=1)
            nc.vector.tensor_scalar_mul(maskSc[:, :], maskS[:, :], rsec[:, 0:1])

            mstp0 = pps.tile([128, BL], f32, tag="mstp")
            nc.tensor.transpose(mstp0[:, :], maskSc[:, 0:128], ident[0:BL, 0:BL])
            mst0 = pre.tile([128, BL], bf16, tag="mst0", bufs=1)
            nc.scalar.copy(mst0[:, :], mstp0[:, :])
            mstp1 = pps.tile([128, BL], f32, tag="mstp")
            nc.tensor.transpose(mstp1[0:72, :], maskSc[:, 128:200], ident[0:BL, 0:BL])
            mst1 = pre.tile([72, BL], bf16, tag="mst1", bufs=1)
            nc.scalar.copy(mst1[:, :], mstp1[0:72, :])

            itn = pre.tile([BL, D], f32, tag="itn", bufs=2)
            nc.sync.dma_start(itn[:, :], item[:, :])
            itp = pps.tile([D, BL], f32, tag="small_t")
            nc.tensor.transpose(itp[:, :], itn[:, :], ident[0:BL, 0:BL])
            nc.scalar.copy(qT[:, :], itp[:, :])
            usn = pre.tile([BL, D], f32, tag="itn", bufs=2)
            nc.sync.dma_start(usn[:, :], user[:, :])
            usp = pps.tile([D, BL], f32, tag="small_t")
            nc.tensor.transpose(usp[:, :], usn[:, :], ident[0:BL, 0:BL])
            nc.scalar.copy(userT[:, :], usp[:, :])
            b2p = pps.tile([BL, 1], f32, tag="small_t")
            _mm(nc, b2p[:, :], ones_f[0:1, 0:BL], b2v, start=True, stop=True)
            nc.scalar.copy(b2col[:, :], b2p[:, :])
            for s in range(CH):
                nc.scalar.copy(qRep[:, s * BL:(s + 1) * BL], qT[:, :])

            # masked history sum from natural-layout tiles; the matmuls run
            # while the big X^T DMA streams in.
            histp0 = hps.tile([D, BL], f32, tag="histp0")
            histp1 = hps.tile([D, BL], f32, tag="histp1")
            for h, (t0, tn) in enumerate(((0, 128), (128, 72))):
                histp = histp0 if h == 0 else histp1
                for b in range(BL):
                    xn = pre.tile([128, D], bf16, tag="xn")
                    nc.sync.dma_start(xn[0:tn, :], hist[b, t0:t0 + tn, :])
                    mcol = (mst0 if h == 0 else mst1)[0:tn, b:b + 1]
                    _mm(nc, histp[:, b:b + 1], xn[0:tn, :], mcol,
                        start=True, stop=True)
            nc.scalar.copy(histF[:, :], histp0[:, :])
            nc.vector.tensor_add(histF[:, :], histF[:, :], histp1[:, :])
            nc.scalar.copy(histT[:, :], histF[:, :])

        # ---------------- GRU + interleaved attention -------------------
        gruT3 = gruT.rearrange("p (t b) -> p t b", b=BL)
        SIG, TANH = AF.Sigmoid, AF.Tanh
        wgt = pp.tile([BL, T], f32)

        with tc.tile_pool(name="gip", bufs=2, space="PSUM") as gip, \
             tc.tile_pool(name="hnp", bufs=1, space="PSUM") as hnp, \
             tc.tile_pool(name="gt", bufs=6) as gt, \
             tc.tile_pool(name="aps", bufs=1, space="PSUM") as aps, \
             tc.tile_pool(name="sps", bufs=1, space="PSUM") as sps, \
             tc.tile_pool(name="at", bufs=3) as at:
            scp = sps.tile([BL, T], f32)

            def attention_pieces(a):
                """Attention for gru chunks [2a, 2a+1] as 8 emission slots;
                each slot is small PE/DVE/Pool work sized to fill one GRU
                step's engine-idle window without head-of-line-blocking the
                recurrence chain's FIFO queues."""
                gc = gruT[:, a * W:(a + 1) * W]
                r2 = at.tile([D, W], bf16, tag="r2")
                y0p = aps.tile([80, W], f32, tag="y0")
                y0 = at.tile([80, W], bf16, tag="y0s")
                y1p = aps.tile([40, W], f32, tag="y1")
                y1 = at.tile([40, W], bf16, tag="y1s")
                HW = 256

                def relu0(q):
                    nc.vector.tensor_scalar(y0[:, q * 128:(q + 1) * 128],
                                            y0p[:, q * 128:(q + 1) * 128],
                                            b0v[:, 0:1], 0.0, ALU.add, ALU.max)
                def relu1(h):
                    nc.vector.tensor_scalar(y1[:, h * HW:(h + 1) * HW],
                                            y1p[:, h * HW:(h + 1) * HW],
                                            b1v[:, 0:1], 0.0, ALU.add, ALU.max)
                def score(s):
                    t = a * CH + s
                    _mm(nc, scp[:, t:t + 1], y1[:, s * BL:(s + 1) * BL],
                        W2T, start=True, stop=True)

                def p0():
                    nc.gpsimd.tensor_mul(r2[:, 0:HW], gc[:, 0:HW],
                                         qRep[:, 0:HW])
                def p1():
                    nc.gpsimd.tensor_mul(r2[:, HW:W], gc[:, HW:W],
                                         qRep[:, HW:W])
                    _mm(nc, y0p[:, 0:HW], W0T[:, 0:80], gc[:, 0:HW],
                        start=True, stop=False)
                def p2():
                    _mm(nc, y0p[:, HW:W], W0T[:, 0:80], gc[:, HW:W],
                        start=True, stop=False)
                    _mm(nc, y0p[:, 0:HW], W0T[:, 80:160], qRep[:, 0:HW],
                        start=False, stop=False)
                def p3():
                    _mm(nc, y0p[:, HW:W], W0T[:, 80:160], qRep[:, HW:W],
                        start=False, stop=False)
                    _mm(nc, y0p[:, 0:HW], W0T[:, 160:240], r2[:, 0:HW],
                        start=False, stop=True)
                def p4():
                    _mm(nc, y0p[:, HW:W], W0T[:, 160:240], r2[:, HW:W],
                        start=False, stop=True)
                    relu0(0)
                def p5():
                    relu0(1)
                    relu0(2)
                def p6():
                    relu0(3)
                    _mm(nc, y1p[:, 0:HW], W1T, y0[:, 0:HW],
                        start=True, stop=True)
                def p7():
                    _mm(nc, y1p[:, HW:W], W1T, y0[:, HW:W],
                        start=True, stop=True)
                    relu1(0)
                def p8():
                    for s in range(4):
                        score(s)
                    relu1(1)
                def p9():
                    for s in range(4, CH):
                        score(s)
                return [p0, p1, p2, p3, p4, p5, p6, p7, p8, p9]

            pending = []          # attention pieces awaiting emission
            WhT_r, WhT_z, WhT_n = (WhhT[:, 0:D], WhhT[:, D:2 * D],
                                   WhhT[:, 2 * D:G3])
            gslot = [None, None]  # (prz, prz4, pn3) per psum buf

            def gru_proj_pieces(c):
                XTc = XT[:, c * WG:(c + 1) * WG]
                prz = gip.tile([D, CG * 128], f32, tag="girz")
                pn = gip.tile([D, CG * BL], f32, tag="gin")
                gslot[c % 2] = (
                    prz,
                    prz.rearrange("p (g s c) -> p g s c", g=2, c=BL),
                    pn.rearrange("p (s g) -> p s g", g=BL),
                )
                return [
                    lambda: _mm(nc, prz[:, 0:WG], WihT[:, 0:D], XTc,
                                start=True, stop=True),
                    lambda: _mm(nc, prz[:, WG:2 * WG], WihT[:, D:2 * D], XTc,
                                start=True, stop=True),
                    lambda: _mm(nc, pn[:, :], WihT[:, 2 * D:G3], XTc,
                                start=True, stop=True),
                ]

            for p in gru_proj_pieces(0):
                p()
            zh_prev = [None]      # z_{t-1} * h_{t-2}
            omzn_prev = [None]    # (1-z_{t-1}) * n_{t-1}
            for c in range(NCG):
                prz, prz4, pn3 = gslot[c % 2]
                for s in range(CG):
                    t = c * CG + s
                    # --- PE block A: recurrent matmuls feeding step t ----
                    hx = hnp.tile([D, 2 * BL], f32, tag="hn2")
                    hn, tmp2 = hx[:, 0:BL], hx[:, BL:2 * BL]
                    if t == 1:
                        h0 = gruT3[:, 0, :]
                        _mm(nc, prz[:, s * BL:(s + 1) * BL], WhT_r, h0,
                            start=False, stop=True, skip=True)
                        _mm(nc, prz[:, WG + s * BL:WG + (s + 1) * BL],
                            WhT_z, h0, start=False, stop=True, skip=True)
                        _mm(nc, hn[:, :], WhT_n, h0, start=True, stop=True)
                    elif t > 1:
                        rsl = prz[:, s * BL:(s + 1) * BL]
                        _mm(nc, rsl, WhT_r, zh_prev[0],
                            start=False, stop=False, skip=True)
                        _mm(nc, rsl, WhT_r, omzn_prev[0],
                            start=False, stop=True, skip=True)
                        hprev = gruT3[:, t - 1, :]
                        _mm(nc, prz[:, WG + s * BL:WG + (s + 1) * BL],
                            WhT_z, hprev, start=False, stop=True, skip=True)
                        _mm(nc, hn[:, :], WhT_n, hprev, start=True, stop=True)
                    # --- gate activations (biases ride ACT) --------------
                    r = gt.tile([D, BL], bf16, tag="r")
                    nc.scalar.activation(r[:, :], prz4[:, 0, s, :], SIG,
                                         bias=brz[:, 0:1])
                    z = gt.tile([D, BL], bf16, tag="z")
                    nc.scalar.activation(z[:, :], prz4[:, 1, s, :], SIG,
                                         bias=brz[:, 1:2])
                    omz = gt.tile([D, BL], bf16, tag="omz")
                    nc.gpsimd.tensor_scalar(omz[:, :], z[:, :], -1.0, 1.0,
                                            ALU.mult, ALU.add)
                    tmp = gt.tile([D, BL], bf16, tag="tmp")
                    if t > 0:
                        nc.vector.scalar_tensor_tensor(
                            tmp[:, :], hn[:, :], b_hn, r[:, :],
                            ALU.add, ALU.mult)
                    else:
                        nc.vector.tensor_scalar_mul(tmp[:, :], r[:, :], b_hn)
                    nc.vector.tensor_add(tmp2[:, :], tmp[:, :], pn3[:, s, :])
                    if t > 0:
                        zh = gt.tile([D, BL], bf16, tag="zh")
                        nc.vector.tensor_mul(zh[:, :], z[:, :],
                                             gruT3[:, t - 1, :])
                    n = gt.tile([D, BL], bf16, tag="n")
                    nc.scalar.activation(n[:, :], tmp2[:, :], TANH, bias=b_in)
                    hdst = gruT3[:, t, :]
                    if t > 0:
                        omzn = gt.tile([D, BL], bf16, tag="omzn")
                        nc.vector.tensor_mul(omzn[:, :], omz[:, :], n[:, :])
                        nc.vector.tensor_add(hdst, omzn[:, :], zh[:, :])
                        zh_prev[0], omzn_prev[0] = zh, omzn
                    else:
                        nc.vector.tensor_mul(hdst, omz[:, :], n[:, :])
                    # --- PE block B: hoisted chunk/attention work --------
                    if s == 1 and c + 1 < NCG:
                        proj_pieces = gru_proj_pieces(c + 1)
                    if s >= 1 and c + 1 < NCG:
                        proj_pieces[s - 1]()
                    if pending:
                        pending.pop(0)()
                    if pending and t % 8 in (2, 3, 4):
                        pending.pop(0)()
                if c % 2 == 1:
                    pending.extend(attention_pieces(c // 2))
            for p in pending:
                p()

            # masked softmax over t (b-major); last MLP layer ReLU'd w/ b2
            rawr = at.tile([BL, T], f32, tag="rawr")
            nc.scalar.activation(rawr[:, :], scp[:, :], AF.Relu,
                                 bias=b2col[:, 0:1])
            rawm = at.tile([BL, T], f32, tag="rawm")
            nc.vector.tensor_mul(rawm[:, :], rawr[:, :], maskS[:, :])
            mxn = at.tile([BL, 1], f32, tag="mxn")
            nc.vector.tensor_reduce(mxn[:, :], rawm[:, :],
                                    axis=mybir.AxisListType.X,
                                    op=ALU.max, negate=True)
            ex = at.tile([BL, T], f32, tag="ex")
            nc.scalar.activation(ex[:, :], rawr[:, :], AF.Exp, bias=mxn[:, 0:1])
            em = at.tile([BL, T], f32, tag="em")
            nc.vector.tensor_mul(em[:, :], ex[:, :], maskS[:, :])
            sm = at.tile([BL, 1], f32, tag="sm")
            nc.vector.tensor_reduce(sm[:, :], em[:, :],
                                    axis=mybir.AxisListType.X, op=ALU.add)
            rs = at.tile([BL, 1], f32, tag="rs")
            nc.vector.reciprocal(rs[:, :], sm[:, :])
            nc.vector.tensor_scalar_mul(wgt[:, :], em[:, :], rs[:, 0:1])

        # attention weights go to partition-0 bf16 rows per AUGRU chunk
        # (the row transposes + broadcasts overlap the AUGRU itself)
        wgt_bf = pp.tile([BL, T], bf16)
        nc.scalar.copy(wgt_bf[:, :], wgt[:, :])

        # ---------------- AUGRU ----------------------------------------
        hA = [None]
        with tc.tile_pool(name="axp", bufs=2, space="PSUM") as axp, \
             tc.tile_pool(name="abp", bufs=2, space="PSUM") as abp, \
             tc.tile_pool(name="rtp", bufs=2, space="PSUM") as rtp, \
             tc.tile_pool(name="ut", bufs=6) as ut:
            aslot = [None, None]  # (pru, pru4, pxh3, pab) per psum buf

            def aug_proj_pieces(c):
                gc = gruT[:, c * WG:(c + 1) * WG]
                rowp = rtp.tile([1, WG], f32, tag="rowp")
                rows = ut.tile([1, WG], bf16, tag="rows")
                pru = axp.tile([D, CG * 128], f32, tag="pxru")
                pxh = axp.tile([D, CG * BL], f32, tag="pxh")
                pab = abp.tile([D, WG], f32, tag="pab")
                aslot[c % 2] = (
                    pru,
                    pru.rearrange("p (g s c) -> p g s c", g=2, c=BL),
                    pxh.rearrange("p (s g) -> p s g", g=BL),
                    pab,
                )

                def pa():
                    for s in range(CG):
                        t = c * CG + s
                        _mm(nc, rowp[0:1, s * BL:(s + 1) * BL],
                            wgt_bf[:, t:t + 1], identB, start=True, stop=True)
                    nc.vector.tensor_copy(rows[0:1, :], rowp[0:1, :])
                def pb():
                    _mm(nc, pru[:, 0:WG], WrxT, gc, start=True, stop=True)
                    _mm(nc, pab[:, :], ones1, rows[0:1, :],
                        start=True, stop=True)
                def pc():
                    _mm(nc, pru[:, WG:2 * WG], WuxT, gc, start=True, stop=True)
                def pd():
                    _mm(nc, pxh[:, :], WaxT, gc, start=True, stop=True)
                return [pa, pb, pc, pd]

            for p in aug_proj_pieces(0):
                p()
            hA_pp = [None]        # hA_{t-2}
            ud_prev = [None]      # ud_{t-1}
            for c in range(NCG):
                pru, pru4, pxh3, pab = aslot[c % 2]
                for s in range(CG):
                    t = c * CG + s
                    hAg = hA[0]
                    # --- PE block A: recurrent matmuls feeding step t ----
                    rsl = pru[:, s * BL:(s + 1) * BL]
                    usl = pru[:, WG + s * BL:WG + (s + 1) * BL]
                    if t == 1:
                        _mm(nc, rsl, WrhT, hAg, start=False, stop=True,
                            skip=True)
                        _mm(nc, usl, WuhT, hAg, start=False, stop=True,
                            skip=True)
                    elif t > 1:
                        _mm(nc, rsl, WrhT, hA_pp[0],
                            start=False, stop=False, skip=True)
                        _mm(nc, rsl, WrhT, ud_prev[0],
                            start=False, stop=True, skip=True)
                        _mm(nc, usl, WuhT, hA_pp[0],
                            start=False, stop=False, skip=True)
                        _mm(nc, usl, WuhT, ud_prev[0],
                            start=False, stop=True, skip=True)
                    r = ut.tile([D, BL], bf16, tag="ar")
                    nc.scalar.activation(r[:, :], pru4[:, 0, s, :], SIG,
                                         bias=ab_r)
                    u = ut.tile([D, BL], bf16, tag="au")
                    nc.scalar.activation(u[:, :], pru4[:, 1, s, :], SIG,
                                         bias=ab_u)
                    if t > 0:
                        hr = ut.tile([D, BL], bf16, tag="ahr")
                        nc.vector.tensor_mul(hr[:, :], hAg, r[:, :])
                        _mm(nc, pxh3[:, s, :], WahT, hr,
                            start=False, stop=True, skip=True)
                    up = ut.tile([D, BL], f32, tag="aup")
                    nc.vector.tensor_mul(up[:, :], pab[:, s * BL:(s + 1) * BL],
                                         u[:, :])
                    hh = ut.tile([D, BL], bf16, tag="ahh")
                    nc.scalar.activation(hh[:, :], pxh3[:, s, :], TANH,
                                         bias=ab_h)
                    hnew = hp.tile([D, BL], bf16, tag="hA")
                    if t > 0:
                        dd = ut.tile([D, BL], bf16, tag="add")
                        nc.vector.tensor_sub(dd[:, :], hh[:, :], hAg)
                        ud = ut.tile([D, BL], bf16, tag="aud")
                        nc.vector.tensor_mul(ud[:, :], up[:, :], dd[:, :])
                        nc.gpsimd.tensor_add(hnew[:, :], hAg, ud[:, :])
                        hA_pp[0], ud_prev[0] = hAg, ud
                    else:
                        nc.vector.tensor_mul(hnew[:, :], up[:, :], hh[:, :])
                    hA[0] = hnew
                    # --- PE block B: hoisted next-chunk work -------------
                    if c + 1 < NCG:
                        if s == 0:
                            aproj_pieces = aug_proj_pieces(c + 1)
                        aproj_pieces[s]()

        # ---------------- output layer ---------------------------------
        with tc.tile_pool(name="ops", bufs=1, space="PSUM") as ops, \
             tc.tile_pool(name="ot", bufs=1) as ot:
            ih = ot.tile([D, BL], bf16)
            nc.vector.tensor_mul(ih[:, :], qT[:, :], histF[:, :])
            po = ops.tile([1, BL], f32)
            pieces = [userT, qT, histT, ih, hA[0]]
            for g, piece in enumerate(pieces):
                _mm(nc, po[:, :], outWT[:, g:g + 1], piece,
                    start=(g == 0), stop=(g == 4))
            outs = ot.tile([1, BL], f32)
            nc.scalar.activation(outs[:, :], po[:, :], AF.Identity,
                                 bias=outb[:, 0:1])
            nc.sync.dma_start(outd[:, :], outs[:, :])

    nc.finalize()
    return nc


_NC = None


def _get_nc():
    global _NC
    if _NC is None:
        _NC = build_nc()
    return _NC


def make_in_maps(inputs):
    """Slice full inputs into per-core input maps (host-side layout only)."""
    f = {k: np.asarray(v) for k, v in inputs.items()}
    WihT = np.ascontiguousarray(f["gru_Wih"].T)          # (128, 384)
    WhhT = np.ascontiguousarray(f["gru_Whh"].T)
    bihT = np.ascontiguousarray(f["gru_bih"].reshape(3, D).T)  # (128, 3)
    bhhT = np.ascontiguousarray(f["gru_bhh"].reshape(3, D).T)
    # fold the (q - fact) block into the fact/q blocks:
    # W0_f@f + W0_q@q + W0_fq@(f*q) + W0_qf@(q-f)
    #   = (W0_f - W0_qf)@f + (W0_q + W0_qf)@q + W0_fq@(f*q)
    W0 = f["attn_W0"]                                    # (80, 4D)
    W0f = W0[:, 0:D] - W0[:, 3 * D:4 * D]
    W0q = W0[:, D:2 * D] + W0[:, 3 * D:4 * D]
    W0fq = W0[:, 2 * D:3 * D]
    W0T = np.ascontiguousarray(
        np.concatenate([W0f.T, W0q.T, W0fq.T], axis=1))  # (D, 240)
    b0 = np.ascontiguousarray(f["attn_b0"].reshape(80, 1))
    W1T = np.ascontiguousarray(f["attn_W1"].T)           # (80, 40)
    b1 = np.ascontiguousarray(f["attn_b1"].reshape(40, 1))
    W2T = np.ascontiguousarray(f["attn_W2"].T)           # (40, 1)
    b2 = f["attn_b2"].reshape(1, 1)
    augW = np.concatenate(
        [np.ascontiguousarray(f[k][:, p * D:(p + 1) * D].T)
         for k in ("aug_Wr", "aug_Wu", "aug_Wh") for p in (0, 1)],
        axis=1)                                          # (128, 768)
    augb = np.stack([f["aug_br"], f["aug_bu"], f["aug_bh"]], axis=1)  # (128,3)
    outWT = np.ascontiguousarray(f["out_W"].reshape(5, D).T)          # (128,5)
    outb = f["out_b"].reshape(1, 1)

    shared_bf = dict(WihT=WihT, WhhT=WhhT, W0T=W0T, W1T=W1T, W2T=W2T,
                     augW=augW, outWT=outWT)
    shared = dict(bihT=bihT, bhhT=bhhT, b0=b0, b1=b1, b2=b2, augb=augb,
                  outb=outb)
    shared = {k: np.ascontiguousarray(v.astype(np.float32)) for k, v in
              shared.items()}
    shared.update({k: np.ascontiguousarray(v.astype(BF)) for k, v in
                   shared_bf.items()})

    in_maps = []
    for c in range(NCORES):
        s = slice(c * BL, (c + 1) * BL)
        m = dict(shared)
        hs = f["item_historical_embedding"][s].astype(BF)
        m["hist"] = np.ascontiguousarray(hs)
        m["histTd"] = np.ascontiguousarray(
            hs.transpose(2, 1, 0).reshape(D, T * BL))
        m["item"] = np.ascontiguousarray(f["item_embedding"][s].astype(np.float32))
        m["user"] = np.ascontiguousarray(f["user_embedding"][s].astype(np.float32))
        m["maskd"] = np.ascontiguousarray(f["mask"][s].astype(np.float32))
        m["seqd"] = np.ascontiguousarray(
            f["sequential_length"][s].reshape(BL, 1).astype(np.int32))
        in_maps.append(m)
    return in_maps


def kernel(**inputs) -> np.ndarray:
    nc = _get_nc()
    in_maps = make_in_maps(inputs)
    res = run_bass_kernel_spmd(nc, in_maps, list(range(NCORES)))
    return np.concatenate(
        [np.asarray(res.results[c]["out"]).reshape(BL) for c in range(NCORES)])


# revision 25
# speedup vs baseline: 1.2574x; 1.0748x over previous
"""DIEN (GRU + attention + AUGRU) Trainium2 kernel.

Data-parallel over 8 NeuronCores: each core handles a batch slice of 64.
All on-chip state is feature-major (feature dim on SBUF partitions, batch
on the free dim), so the two sequential recurrences need no per-step
transposes.

Structure:
  - X^T (D, T*64) in bf16, built via PE transposes of natural-layout DMA
    tiles; the same stationary tiles also accumulate the masked history
    sum (f32) via N=1 matmuls against 1/len-scaled mask columns.
  - GRU input projections land in PSUM chunks laid out (128, 4, [r|z|n]).
    Gate sigmoids are split per-gate so the gate bias rides the ACT
    instruction (no DVE bias adds), and (1-z) comes from sigmoid(-x).
    The combine uses h' = (1-z)*n + z*h with z*h computed off the
    critical path.
  - Attention runs per 8-step chunk interleaved with the GRU loop; its
    elementwise prep (fact*q, q-fact) runs on the idle GPSIMD engine so
    the DVE stays free for the recurrence chain.
  - AUGRU consumes attention weights via rows pre-transposed once after
    the softmax; per chunk a single ones-weight load broadcasts 4 rows
    into PSUM. Masked steps have weight 0 (u'=0, h unchanged), so the
    final AUGRU state IS the gathered aug_out[b, len-1] - no gather.
  - Matmuls are bf16 (f32 PSUM accumulation); elementwise recurrence ops
    run bf16 where both operands sit in SBUF (2x DVE mode); the masked
    history mean and softmax stay f32.
"""

import sys

if "/opt/trn_rl_repo" not in sys.path:
    sys.path.insert(0, "/opt/trn_rl_repo")

from contextlib import ExitStack

import ml_dtypes
import numpy as np

import concourse.bacc as bacc
import concourse.bass as bass
import concourse.mybir as mybir
import concourse.tile as tile
from concourse.bass_utils import run_bass_kernel_spmd
from concourse.masks import make_identity

f32 = mybir.dt.float32
bf16 = mybir.dt.bfloat16
i32 = mybir.dt.int32
AF = mybir.ActivationFunctionType
ALU = mybir.AluOpType
BF = ml_dtypes.bfloat16

NCORES = 8
B, T, D = 512, 200, 128
BL = B // NCORES          # 64 batch rows per core
CG = 4                    # GRU/AUGRU psum chunk: steps per chunk
NCG = T // CG             # 50
WG = CG * BL              # 256
CH = 8                    # attention chunk: steps per chunk
NCHUNK = T // CH          # 25
W = CH * BL               # 512
G3 = 3 * D


def _mm(nc, out, lhsT, rhs, start, stop, skip=False):
    nc.tensor.matmul(out, lhsT, rhs, start=start, stop=stop,
                     skip_group_check=skip)


def build_nc():
    nc = bacc.Bacc("TRN2", target_bir_lowering=False)

    hist = nc.declare_dram_parameter("hist", [BL, T, D], bf16, isOutput=False)
    histTd = nc.declare_dram_parameter("histTd", [D, T * BL], bf16,
                                       isOutput=False)
    item = nc.declare_dram_parameter("item", [BL, D], f32, isOutput=False)
    user = nc.declare_dram_parameter("user", [BL, D], f32, isOutput=False)
    maskd = nc.declare_dram_parameter("maskd", [BL, T], f32, isOutput=False)
    seqd = nc.declare_dram_parameter("seqd", [BL, 1], i32, isOutput=False)
    WihTd = nc.declare_dram_parameter("WihT", [D, G3], bf16, isOutput=False)
    WhhTd = nc.declare_dram_parameter("WhhT", [D, G3], bf16, isOutput=False)
    bihTd = nc.declare_dram_parameter("bihT", [D, 3], f32, isOutput=False)
    bhhTd = nc.declare_dram_parameter("bhhT", [D, 3], f32, isOutput=False)
    W0Td = nc.declare_dram_parameter("W0T", [D, 240], bf16, isOutput=False)
    b0d = nc.declare_dram_parameter("b0", [80, 1], f32, isOutput=False)
    W1Td = nc.declare_dram_parameter("W1T", [80, 40], bf16, isOutput=False)
    b1d = nc.declare_dram_parameter("b1", [40, 1], f32, isOutput=False)
    W2Td = nc.declare_dram_parameter("W2T", [40, 1], bf16, isOutput=False)
    b2d = nc.declare_dram_parameter("b2", [1, 1], f32, isOutput=False)
    augWd = nc.declare_dram_parameter("augW", [D, 6 * D], bf16, isOutput=False)
    augbd = nc.declare_dram_parameter("augb", [D, 3], f32, isOutput=False)
    outWTd = nc.declare_dram_parameter("outWT", [D, 5], bf16, isOutput=False)
    outbd = nc.declare_dram_parameter("outb", [1, 1], f32, isOutput=False)
    outd = nc.declare_dram_parameter("out", [1, BL], f32, isOutput=True)

    with tile.TileContext(nc) as tc, ExitStack() as ctx:
        big = ctx.enter_context(tc.tile_pool(name="big", bufs=1))
        wp = ctx.enter_context(tc.tile_pool(name="wp", bufs=1))
        pp = ctx.enter_context(tc.tile_pool(name="pp", bufs=1))
        hp = ctx.enter_context(tc.tile_pool(name="hp", bufs=4))

        XT = big.tile([D, T * BL], bf16)
        gruT = big.tile([D, T * BL], bf16)

        ident = pp.tile([128, 128], f32)
        make_identity(nc, ident)
        identB = pp.tile([BL, BL], bf16)
        make_identity(nc, identB)
        identB128 = pp.tile([128, 128], bf16)
        make_identity(nc, identB128)

        # ------------- weights (bf16 arrive pre-converted via DMA) -------
        WihT = wp.tile([D, G3], bf16)
        nc.sync.dma_start(WihT[:, :], WihTd[:, :])
        WhhT = wp.tile([D, G3], bf16)
        nc.sync.dma_start(WhhT[:, :], WhhTd[:, :])
        W0T = wp.tile([D, 240], bf16)
        nc.sync.dma_start(W0T[:, :], W0Td[:, :])
        W1T = wp.tile([80, 40], bf16)
        nc.sync.dma_start(W1T[:, :], W1Td[:, :])
        W2T = wp.tile([40, 1], bf16)
        nc.sync.dma_start(W2T[:, :], W2Td[:, :])
        augW = wp.tile([D, 6 * D], bf16)
        nc.sync.dma_start(augW[:, :], augWd[:, :])
        outWT = wp.tile([D, 5], bf16)
        nc.sync.dma_start(outWT[:, :], outWTd[:, :])
        WrhT, WrxT = augW[:, 0:D], augW[:, D:2 * D]
        WuhT, WuxT = augW[:, 2 * D:3 * D], augW[:, 3 * D:4 * D]
        WahT, WaxT = augW[:, 4 * D:5 * D], augW[:, 5 * D:6 * D]

        bihT = wp.tile([D, 3], f32)
        nc.sync.dma_start(bihT[:, :], bihTd[:, :])
        bhhT = wp.tile([D, 3], f32)
        nc.sync.dma_start(bhhT[:, :], bhhTd[:, :])
        brz = wp.tile([D, 2], f32)
        nc.vector.tensor_add(brz[:, :], bihT[:, 0:2], bhhT[:, 0:2])
        nbz = wp.tile([D, 1], f32)
        nc.vector.tensor_scalar(nbz[:, :], brz[:, 1:2], -1.0, 0.0,
                                ALU.mult, ALU.add)
        b_hn, b_in = bhhT[:, 2:3], bihT[:, 2:3]

        b0v = wp.tile([80, 1], f32)
        nc.sync.dma_start(b0v[:, :], b0d[:, :])
        b1v = wp.tile([40, 1], f32)
        nc.sync.dma_start(b1v[:, :], b1d[:, :])
        b2v = wp.tile([1, 1], f32)
        nc.sync.dma_start(b2v[:, :], b2d[:, :])
        augb = wp.tile([D, 3], f32)
        nc.sync.dma_start(augb[:, :], augbd[:, :])
        outb = wp.tile([1, 1], f32)
        nc.sync.dma_start(outb[:, :], outbd[:, :])
        ab_r, ab_u, ab_h = augb[:, 0:1], augb[:, 1:2], augb[:, 2:3]

        ones_f = wp.tile([1, 128], f32)
        nc.vector.memset(ones_f[:, :], 1.0)
        ones1 = wp.tile([1, 128], bf16)
        nc.vector.memset(ones1[:, :], 1.0)

        # ---------------- preprocessing --------------------------------
        histT = pp.tile([D, BL], bf16)
        qT = pp.tile([D, BL], bf16)
        userT = pp.tile([D, BL], bf16)
        qRep = pp.tile([D, W], bf16)
        maskS = pp.tile([BL, T], f32)
        b2col = pp.tile([BL, 1], f32)
        histF = pp.tile([D, BL], f32)

        with tc.tile_pool(name="pre", bufs=8) as pre, \
             tc.tile_pool(name="pps", bufs=2, space="PSUM") as pps, \
             tc.tile_pool(name="hps", bufs=1, space="PSUM") as hps:
            # X^T arrives pre-transposed from the host; split the DMA so the
            # first GRU chunks can start as soon as their columns land.
            nc.sync.dma_start(XT[:, 0:4 * WG], histTd[:, 0:4 * WG])
            nc.sync.dma_start(XT[:, 4 * WG:25 * WG], histTd[:, 4 * WG:25 * WG])
            nc.sync.dma_start(XT[:, 25 * WG:T * BL], histTd[:, 25 * WG:T * BL])

            nc.sync.dma_start(maskS[:, :], maskd[:, :])
            seqi = pre.tile([BL, 1], i32, tag="seqi", bufs=1)
            nc.sync.dma_start(seqi[:, :], seqd[:, :])
            seqf = pre.tile([BL, 1], f32, tag="seqf", bufs=1)
            nc.vector.tensor_copy(seqf[:, :], seqi[:, :])
            rsec = pre.tile([BL, 1], f32, tag="rsec", bufs=1)
            nc.vector.reciprocal(rsec[:, :], seqf[:, :])
            maskSc = pre.tile([BL, T], f32, tag="maskSc", bufs=1)
            nc.vector.tensor_scalar_mul(maskSc[:, :], maskS[:, :], rsec[:, 0:1])

            mstp0 = pps.tile([128, BL], f32, tag="mstp")
            nc.tensor.transpose(mstp0[:, :], maskSc[:, 0:128], ident[0:BL, 0:BL])
            mst0 = pre.tile([128, BL], bf16, tag="mst0", bufs=1)
            nc.scalar.copy(mst0[:, :], mstp0[:, :])
            mstp1 = pps.tile([128, BL], f32, tag="mstp")
            nc.tensor.transpose(mstp1[0:72, :], maskSc[:, 128:200], ident[0:BL, 0:BL])
            mst1 = pre.tile([72, BL], bf16, tag="mst1", bufs=1)
            nc.scalar.copy(mst1[:, :], mstp1[0:72, :])

            itn = pre.tile([BL, D], f32, tag="itn", bufs=2)
            nc.sync.dma_start(itn[:, :], item[:, :])
            itp = pps.tile([D, BL], f32, tag="small_t")
            nc.tensor.transpose(itp[:, :], itn[:, :], ident[0:BL, 0:BL])
            nc.scalar.copy(qT[:, :], itp[:, :])
            usn = pre.tile([BL, D], f32, tag="itn", bufs=2)
            nc.sync.dma_start(usn[:, :], user[:, :])
            usp = pps.tile([D, BL], f32, tag="small_t")
            nc.tensor.transpose(usp[:, :], usn[:, :], ident[0:BL, 0:BL])
            nc.scalar.copy(userT[:, :], usp[:, :])
            b2p = pps.tile([BL, 1], f32, tag="small_t")
            _mm(nc, b2p[:, :], ones_f[0:1, 0:BL], b2v, start=True, stop=True)
            nc.scalar.copy(b2col[:, :], b2p[:, :])
            for s in range(CH):
                nc.scalar.copy(qRep[:, s * BL:(s + 1) * BL], qT[:, :])

            # masked history sum from natural-layout tiles; batched DMAs
            # (8 batch rows per dma_start) while the big X^T DMA streams in.
            histp0 = hps.tile([D, BL], f32, tag="histp0")
            histp1 = hps.tile([D, BL], f32, tag="histp1")
            GB = 8
            for h, (t0, tn) in enumerate(((0, 128), (128, 72))):
                histp = histp0 if h == 0 else histp1
                for b0 in range(0, BL, GB):
                    xn = pre.tile([128, GB * D], bf16, tag="xn", bufs=3)
                    xn3 = xn.rearrange("t (b d) -> t b d", d=D)
                    nc.sync.dma_start(
                        xn3[0:tn, :, :],
                        hist[b0:b0 + GB, t0:t0 + tn, :]
                        .rearrange("b t d -> t b d"))
                    for bi in range(GB):
                        b = b0 + bi
                        mcol = (mst0 if h == 0 else mst1)[0:tn, b:b + 1]
                        _mm(nc, histp[:, b:b + 1],
                            xn3[0:tn, bi, :], mcol, start=True, stop=True)
            nc.scalar.copy(histF[:, :], histp0[:, :])
            nc.vector.tensor_add(histF[:, :], histF[:, :], histp1[:, :])
            nc.scalar.copy(histT[:, :], histF[:, :])

        # ---------------- GRU + interleaved attention -------------------
        gruT3 = gruT.rearrange("p (t b) -> p t b", b=BL)
        SIG, TANH = AF.Sigmoid, AF.Tanh
        wgt = pp.tile([BL, T], f32)

        with tc.tile_pool(name="gip", bufs=2, space="PSUM") as gip, \
             tc.tile_pool(name="hnp", bufs=1, space="PSUM") as hnp, \
             tc.tile_pool(name="gt", bufs=6) as gt, \
             tc.tile_pool(name="aps", bufs=1, space="PSUM") as aps, \
             tc.tile_pool(name="sps", bufs=1, space="PSUM") as sps, \
             tc.tile_pool(name="at", bufs=3) as at:
            scp = sps.tile([BL, T], f32)

            def attention_pieces(a):
                """Attention for gru chunks [2a, 2a+1] as 8 emission slots;
                each slot is small PE/DVE/Pool work sized to fill one GRU
                step's engine-idle window without head-of-line-blocking the
                recurrence chain's FIFO queues."""
                gc = gruT[:, a * W:(a + 1) * W]
                r2 = at.tile([D, W], bf16, tag="r2")
                y0p = aps.tile([80, W], f32, tag="y0")
                y0 = at.tile([80, W], bf16, tag="y0s")
                y1p = aps.tile([40, W], f32, tag="y1")
                y1 = at.tile([40, W], bf16, tag="y1s")
                HW = 256

                def relu0(q):
                    nc.vector.tensor_scalar(y0[:, q * 128:(q + 1) * 128],
                                            y0p[:, q * 128:(q + 1) * 128],
                                            b0v[:, 0:1], 0.0, ALU.add, ALU.max)
                def relu1(h):
                    nc.vector.tensor_scalar(y1[:, h * HW:(h + 1) * HW],
                                            y1p[:, h * HW:(h + 1) * HW],
                                            b1v[:, 0:1], 0.0, ALU.add, ALU.max)
                def score(s):
                    t = a * CH + s
                    _mm(nc, scp[:, t:t + 1], y1[:, s * BL:(s + 1) * BL],
                        W2T, start=True, stop=True)

                def p0():
                    nc.gpsimd.tensor_mul(r2[:, 0:HW], gc[:, 0:HW],
                                         qRep[:, 0:HW])
                def p1():
                    nc.gpsimd.tensor_mul(r2[:, HW:W], gc[:, HW:W],
                                         qRep[:, HW:W])
                    _mm(nc, y0p[:, 0:HW], W0T[:, 0:80], gc[:, 0:HW],
                        start=True, stop=False)
                def p2():
                    _mm(nc, y0p[:, HW:W], W0T[:, 0:80], gc[:, HW:W],
                        start=True, stop=False)
                    _mm(nc, y0p[:, 0:HW], W0T[:, 80:160], qRep[:, 0:HW],
                        start=False, stop=False)
                def p3():
                    _mm(nc, y0p[:, HW:W], W0T[:, 80:160], qRep[:, HW:W],
                        start=False, stop=False)
                    _mm(nc, y0p[:, 0:HW], W0T[:, 160:240], r2[:, 0:HW],
                        start=False, stop=True)
                def p4():
                    _mm(nc, y0p[:, HW:W], W0T[:, 160:240], r2[:, HW:W],
                        start=False, stop=True)
                    relu0(0)
                def p5():
                    relu0(1)
                    relu0(2)
                def p6():
                    relu0(3)
                    _mm(nc, y1p[:, 0:HW], W1T, y0[:, 0:HW],
                        start=True, stop=True)
                def p7():
                    _mm(nc, y1p[:, HW:W], W1T, y0[:, HW:W],
                        start=True, stop=True)
                    relu1(0)
                def p8():
                    for s in range(4):
                        score(s)
                    relu1(1)
                def p9():
                    for s in range(4, CH):
                        score(s)
                return [p0, p1, p2, p3, p4, p5, p6, p7, p8, p9]

            pending = []          # attention pieces awaiting emission
            WhT_r, WhT_z, WhT_n = (WhhT[:, 0:D], WhhT[:, D:2 * D],
                                   WhhT[:, 2 * D:G3])
            gslot = [None, None]  # (prz, prz4, pn3) per psum buf

            def gru_proj_pieces(c):
                XTc = XT[:, c * WG:(c + 1) * WG]
                prz = gip.tile([D, CG * 128], f32, tag="girz")
                pn = gip.tile([D, CG * BL], f32, tag="gin")
                gslot[c % 2] = (
                    prz,
                    prz.rearrange("p (g s c) -> p g s c", g=2, c=BL),
                    pn.rearrange("p (s g) -> p s g", g=BL),
                )
                return [
                    lambda: _mm(nc, prz[:, 0:WG], WihT[:, 0:D], XTc,
                                start=True, stop=True),
                    lambda: _mm(nc, prz[:, WG:2 * WG], WihT[:, D:2 * D], XTc,
                                start=True, stop=True),
                    lambda: _mm(nc, pn[:, :], WihT[:, 2 * D:G3], XTc,
                                start=True, stop=True),
                ]

            for p in gru_proj_pieces(0):
                p()
            zh_prev = [None]      # z_{t-1} * h_{t-2}
            omzn_prev = [None]    # (1-z_{t-1}) * n_{t-1}
            for c in range(NCG):
                prz, prz4, pn3 = gslot[c % 2]
                for s in range(CG):
                    t = c * CG + s
                    # --- PE block A: recurrent matmuls feeding step t ----
                    hx = hnp.tile([D, 2 * BL], f32, tag="hn2")
                    hn, tmp2 = hx[:, 0:BL], hx[:, BL:2 * BL]
                    if t == 1:
                        h0 = gruT3[:, 0, :]
                        _mm(nc, prz[:, s * BL:(s + 1) * BL], WhT_r, h0,
                            start=False, stop=True, skip=True)
                        _mm(nc, prz[:, WG + s * BL:WG + (s + 1) * BL],
                            WhT_z, h0, start=False, stop=True, skip=True)
                        _mm(nc, hn[:, :], WhT_n, h0, start=True, stop=True)
                    elif t > 1:
                        rsl = prz[:, s * BL:(s + 1) * BL]
                        _mm(nc, rsl, WhT_r, zh_prev[0],
                            start=False, stop=False, skip=True)
                        _mm(nc, rsl, WhT_r, omzn_prev[0],
                            start=False, stop=True, skip=True)
                        hprev = gruT3[:, t - 1, :]
                        _mm(nc, prz[:, WG + s * BL:WG + (s + 1) * BL],
                            WhT_z, hprev, start=False, stop=True, skip=True)
                        _mm(nc, hn[:, :], WhT_n, hprev, start=True, stop=True)
                    # --- gate activations (biases ride ACT) --------------
                    r = gt.tile([D, BL], bf16, tag="r")
                    nc.scalar.activation(r[:, :], prz4[:, 0, s, :], SIG,
                                         bias=brz[:, 0:1])
                    z = gt.tile([D, BL], bf16, tag="z")
                    nc.scalar.activation(z[:, :], prz4[:, 1, s, :], SIG,
                                         bias=brz[:, 1:2])
                    omz = gt.tile([D, BL], bf16, tag="omz")
                    nc.gpsimd.tensor_scalar(omz[:, :], z[:, :], -1.0, 1.0,
                                            ALU.mult, ALU.add)
                    tmp = gt.tile([D, BL], bf16, tag="tmp")
                    if t > 0:
                        nc.vector.scalar_tensor_tensor(
                            tmp[:, :], hn[:, :], b_hn, r[:, :],
                            ALU.add, ALU.mult)
                    else:
                        nc.vector.tensor_scalar_mul(tmp[:, :], r[:, :], b_hn)
                    nc.vector.tensor_add(tmp2[:, :], tmp[:, :], pn3[:, s, :])
                    if t > 0:
                        zh = gt.tile([D, BL], bf16, tag="zh")
                        nc.vector.tensor_mul(zh[:, :], z[:, :],
                                             gruT3[:, t - 1, :])
                    n = gt.tile([D, BL], bf16, tag="n")
                    nc.scalar.activation(n[:, :], tmp2[:, :], TANH, bias=b_in)
                    hdst = gruT3[:, t, :]
                    if t > 0:
                        omzn = gt.tile([D, BL], bf16, tag="omzn")
                        nc.vector.tensor_mul(omzn[:, :], omz[:, :], n[:, :])
                        nc.vector.tensor_add(hdst, omzn[:, :], zh[:, :])
                        zh_prev[0], omzn_prev[0] = zh, omzn
                    else:
                        nc.vector.tensor_mul(hdst, omz[:, :], n[:, :])
                    # --- PE block B: hoisted chunk/attention work --------
                    if s == 1 and c + 1 < NCG:
                        proj_pieces = gru_proj_pieces(c + 1)
                    if s >= 1 and c + 1 < NCG:
                        proj_pieces[s - 1]()
                    if pending:
                        pending.pop(0)()
                    if pending and t % 8 in (2, 3, 4):
                        pending.pop(0)()
                if c % 2 == 1:
                    pending.extend(attention_pieces(c // 2))
            for p in pending:
                p()

            # masked softmax over t (b-major); last MLP layer ReLU'd w/ b2
            rawr = at.tile([BL, T], f32, tag="rawr")
            nc.scalar.activation(rawr[:, :], scp[:, :], AF.Relu,
                                 bias=b2col[:, 0:1])
            rawm = at.tile([BL, T], f32, tag="rawm")
            nc.vector.tensor_mul(rawm[:, :], rawr[:, :], maskS[:, :])
            mxn = at.tile([BL, 1], f32, tag="mxn")
            nc.vector.tensor_reduce(mxn[:, :], rawm[:, :],
                                    axis=mybir.AxisListType.X,
                                    op=ALU.max, negate=True)
            ex = at.tile([BL, T], f32, tag="ex")
            nc.scalar.activation(ex[:, :], rawr[:, :], AF.Exp, bias=mxn[:, 0:1])
            em = at.tile([BL, T], f32, tag="em")
            nc.vector.tensor_mul(em[:, :], ex[:, :], maskS[:, :])
            sm = at.tile([BL, 1], f32, tag="sm")
            nc.vector.tensor_reduce(sm[:, :], em[:, :],
                                    axis=mybir.AxisListType.X, op=ALU.add)
            rs = at.tile([BL, 1], f32, tag="rs")
            nc.vector.reciprocal(rs[:, :], sm[:, :])
            nc.vector.tensor_scalar_mul(wgt[:, :], em[:, :], rs[:, 0:1])

        # attention weights go to partition-0 bf16 rows per AUGRU chunk
        # (the row transposes + broadcasts overlap the AUGRU itself)
        wgt_bf = pp.tile([BL, T], bf16)
        nc.scalar.copy(wgt_bf[:, :], wgt[:, :])

        # ---------------- AUGRU ----------------------------------------
        hA = [None]
        with tc.tile_pool(name="axp", bufs=2, space="PSUM") as axp, \
             tc.tile_pool(name="abp", bufs=2, space="PSUM") as abp, \
             tc.tile_pool(name="rtp", bufs=2, space="PSUM") as rtp, \
             tc.tile_pool(name="ut", bufs=6) as ut:
            aslot = [None, None]  # (pru, pru4, pxh3, pab) per psum buf

            def aug_proj_pieces(c):
                gc = gruT[:, c * WG:(c + 1) * WG]
                rowp = rtp.tile([1, WG], f32, tag="rowp")
                rows = ut.tile([1, WG], bf16, tag="rows")
                pru = axp.tile([D, CG * 128], f32, tag="pxru")
                pxh = axp.tile([D, CG * BL], f32, tag="pxh")
                pab = abp.tile([D, WG], f32, tag="pab")
                aslot[c % 2] = (
                    pru,
                    pru.rearrange("p (g s c) -> p g s c", g=2, c=BL),
                    pxh.rearrange("p (s g) -> p s g", g=BL),
                    pab,
                )

                def pa():
                    for s in range(CG):
                        t = c * CG + s
                        _mm(nc, rowp[0:1, s * BL:(s + 1) * BL],
                            wgt_bf[:, t:t + 1], identB, start=True, stop=True)
                    nc.vector.tensor_copy(rows[0:1, :], rowp[0:1, :])
                def pb():
                    _mm(nc, pru[:, 0:WG], WrxT, gc, start=True, stop=True)
                    _mm(nc, pab[:, :], ones1, rows[0:1, :],
                        start=True, stop=True)
                def pc():
                    _mm(nc, pru[:, WG:2 * WG], WuxT, gc, start=True, stop=True)
                def pd():
                    _mm(nc, pxh[:, :], WaxT, gc, start=True, stop=True)
                return [pa, pb, pc, pd]

            for p in aug_proj_pieces(0):
                p()
            hA_pp = [None]        # hA_{t-2}
            ud_prev = [None]      # ud_{t-1}
            for c in range(NCG):
                pru, pru4, pxh3, pab = aslot[c % 2]
                for s in range(CG):
                    t = c * CG + s
                    hAg = hA[0]
                    # --- PE block A: recurrent matmuls feeding step t ----
                    rsl = pru[:, s * BL:(s + 1) * BL]
                    usl = pru[:, WG + s * BL:WG + (s + 1) * BL]
                    if t == 1:
                        _mm(nc, rsl, WrhT, hAg, start=False, stop=True,
                            skip=True)
                        _mm(nc, usl, WuhT, hAg, start=False, stop=True,
                            skip=True)
                    elif t > 1:
                        _mm(nc, rsl, WrhT, hA_pp[0],
                            start=False, stop=False, skip=True)
                        _mm(nc, rsl, WrhT, ud_prev[0],
                            start=False, stop=True, skip=True)
                        _mm(nc, usl, WuhT, hA_pp[0],
                            start=False, stop=False, skip=True)
                        _mm(nc, usl, WuhT, ud_prev[0],
                            start=False, stop=True, skip=True)
                    r = ut.tile([D, BL], bf16, tag="ar")
                    nc.scalar.activation(r[:, :], pru4[:, 0, s, :], SIG,
                                         bias=ab_r)
                    u = ut.tile([D, BL], bf16, tag="au")
                    nc.scalar.activation(u[:, :], pru4[:, 1, s, :], SIG,
                                         bias=ab_u)
                    if t > 0:
                        hr = ut.tile([D, BL], bf16, tag="ahr")
                        nc.vector.tensor_mul(hr[:, :], hAg, r[:, :])
                        _mm(nc, pxh3[:, s, :], WahT, hr,
                            start=False, stop=True, skip=True)
                    up = ut.tile([D, BL], f32, tag="aup")
                    nc.vector.tensor_mul(up[:, :], pab[:, s * BL:(s + 1) * BL],
                                         u[:, :])
                    hh = ut.tile([D, BL], bf16, tag="ahh")
                    nc.scalar.activation(hh[:, :], pxh3[:, s, :], TANH,
                                         bias=ab_h)
                    hnew = hp.tile([D, BL], bf16, tag="hA")
                    if t > 0:
                        dd = ut.tile([D, BL], bf16, tag="add")
                        nc.vector.tensor_sub(dd[:, :], hh[:, :], hAg)
                        ud = ut.tile([D, BL], bf16, tag="aud")
                        nc.vector.tensor_mul(ud[:, :], up[:, :], dd[:, :])
                        nc.gpsimd.tensor_add(hnew[:, :], hAg, ud[:, :])
                        hA_pp[0], ud_prev[0] = hAg, ud
                    else:
                        nc.vector.tensor_mul(hnew[:, :], up[:, :], hh[:, :])
                    hA[0] = hnew
                    # --- PE block B: hoisted next-chunk work -------------
                    if c + 1 < NCG:
                        if s == 0:
                            aproj_pieces = aug_proj_pieces(c + 1)
                        aproj_pieces[s]()

        # ---------------- output layer ---------------------------------
        with tc.tile_pool(name="ops", bufs=1, space="PSUM") as ops, \
             tc.tile_pool(name="ot", bufs=1) as ot:
            ih = ot.tile([D, BL], bf16)
            nc.vector.tensor_mul(ih[:, :], qT[:, :], histF[:, :])
            po = ops.tile([1, BL], f32)
            pieces = [userT, qT, histT, ih, hA[0]]
            for g, piece in enumerate(pieces):
                _mm(nc, po[:, :], outWT[:, g:g + 1], piece,
                    start=(g == 0), stop=(g == 4))
            outs = ot.tile([1, BL], f32)
            nc.scalar.activation(outs[:, :], po[:, :], AF.Identity,
                                 bias=outb[:, 0:1])
            nc.sync.dma_start(outd[:, :], outs[:, :])

    nc.finalize()
    return nc


_NC = None


def _get_nc():
    global _NC
    if _NC is None:
        _NC = build_nc()
    return _NC


def make_in_maps(inputs):
    """Slice full inputs into per-core input maps (host-side layout only)."""
    f = {k: np.asarray(v) for k, v in inputs.items()}
    WihT = np.ascontiguousarray(f["gru_Wih"].T)          # (128, 384)
    WhhT = np.ascontiguousarray(f["gru_Whh"].T)
    bihT = np.ascontiguousarray(f["gru_bih"].reshape(3, D).T)  # (128, 3)
    bhhT = np.ascontiguousarray(f["gru_bhh"].reshape(3, D).T)
    # fold the (q - fact) block into the fact/q blocks:
    # W0_f@f + W0_q@q + W0_fq@(f*q) + W0_qf@(q-f)
    #   = (W0_f - W0_qf)@f + (W0_q + W0_qf)@q + W0_fq@(f*q)
    W0 = f["attn_W0"]                                    # (80, 4D)
    W0f = W0[:, 0:D] - W0[:, 3 * D:4 * D]
    W0q = W0[:, D:2 * D] + W0[:, 3 * D:4 * D]
    W0fq = W0[:, 2 * D:3 * D]
    W0T = np.ascontiguousarray(
        np.concatenate([W0f.T, W0q.T, W0fq.T], axis=1))  # (D, 240)
    b0 = np.ascontiguousarray(f["attn_b0"].reshape(80, 1))
    W1T = np.ascontiguousarray(f["attn_W1"].T)           # (80, 40)
    b1 = np.ascontiguousarray(f["attn_b1"].reshape(40, 1))
    W2T = np.ascontiguousarray(f["attn_W2"].T)           # (40, 1)
    b2 = f["attn_b2"].reshape(1, 1)
    augW = np.concatenate(
        [np.ascontiguousarray(f[k][:, p * D:(p + 1) * D].T)
         for k in ("aug_Wr", "aug_Wu", "aug_Wh") for p in (0, 1)],
        axis=1)                                          # (128, 768)
    augb = np.stack([f["aug_br"], f["aug_bu"], f["aug_bh"]], axis=1)  # (128,3)
    outWT = np.ascontiguousarray(f["out_W"].reshape(5, D).T)          # (128,5)
    outb = f["out_b"].reshape(1, 1)

    shared_bf = dict(WihT=WihT, WhhT=WhhT, W0T=W0T, W1T=W1T, W2T=W2T,
                     augW=augW, outWT=outWT)
    shared = dict(bihT=bihT, bhhT=bhhT, b0=b0, b1=b1, b2=b2, augb=augb,
                  outb=outb)
    shared = {k: np.ascontiguousarray(v.astype(np.float32)) for k, v in
              shared.items()}
    shared.update({k: np.ascontiguousarray(v.astype(BF)) for k, v in
                   shared_bf.items()})

    in_maps = []
    for c in range(NCORES):
        s = slice(c * BL, (c + 1) * BL)
        m = dict(shared)
        hs = f["item_historical_embedding"][s].astype(BF)
        m["hist"] = np.ascontiguousarray(hs)
        m["histTd"] = np.ascontiguousarray(
            hs.transpose(2, 1, 0).reshape(D, T * BL))
        m["item"] = np.ascontiguousarray(f["item_embedding"][s].astype(np.float32))
        m["user"] = np.ascontiguousarray(f["user_embedding"][s].astype(np.float32))
        m["maskd"] = np.ascontiguousarray(f["mask"][s].astype(np.float32))
        m["seqd"] = np.ascontiguousarray(
            f["sequential_length"][s].reshape(BL, 1).astype(np.int32))
        in_maps.append(m)
    return in_maps


def kernel(**inputs) -> np.ndarray:
    nc = _get_nc()
    in_maps = make_in_maps(inputs)
    res = run_bass_kernel_spmd(nc, in_maps, list(range(NCORES)))
    return np.concatenate(
        [np.asarray(res.results[c]["out"]).reshape(BL) for c in range(NCORES)])


# revision 29
# speedup vs baseline: 1.2658x; 1.0067x over previous
"""DIEN (GRU + attention + AUGRU) Trainium2 kernel.

Data-parallel over 8 NeuronCores: each core handles a batch slice of 64.
All on-chip state is feature-major (feature dim on SBUF partitions, batch
on the free dim), so the two sequential recurrences need no per-step
transposes.

Structure:
  - X^T (D, T*64) in bf16, built via PE transposes of natural-layout DMA
    tiles; the same stationary tiles also accumulate the masked history
    sum (f32) via N=1 matmuls against 1/len-scaled mask columns.
  - GRU input projections land in PSUM chunks laid out (128, 4, [r|z|n]).
    Gate sigmoids are split per-gate so the gate bias rides the ACT
    instruction (no DVE bias adds), and (1-z) comes from sigmoid(-x).
    The combine uses h' = (1-z)*n + z*h with z*h computed off the
    critical path.
  - Attention runs per 8-step chunk interleaved with the GRU loop; its
    elementwise prep (fact*q, q-fact) runs on the idle GPSIMD engine so
    the DVE stays free for the recurrence chain.
  - AUGRU consumes attention weights via rows pre-transposed once after
    the softmax; per chunk a single ones-weight load broadcasts 4 rows
    into PSUM. Masked steps have weight 0 (u'=0, h unchanged), so the
    final AUGRU state IS the gathered aug_out[b, len-1] - no gather.
  - Matmuls are bf16 (f32 PSUM accumulation); elementwise recurrence ops
    run bf16 where both operands sit in SBUF (2x DVE mode); the masked
    history mean and softmax stay f32.
"""

import sys

if "/opt/trn_rl_repo" not in sys.path:
    sys.path.insert(0, "/opt/trn_rl_repo")

from contextlib import ExitStack

import ml_dtypes
import numpy as np

import concourse.bacc as bacc
import concourse.bass as bass
import concourse.mybir as mybir
import concourse.tile as tile
from concourse.bass_utils import run_bass_kernel_spmd
from concourse.masks import make_identity

f32 = mybir.dt.float32
bf16 = mybir.dt.bfloat16
i32 = mybir.dt.int32
AF = mybir.ActivationFunctionType
ALU = mybir.AluOpType
BF = ml_dtypes.bfloat16

NCORES = 8
B, T, D = 512, 200, 128
BL = B // NCORES          # 64 batch rows per core
CG = 4                    # GRU/AUGRU psum chunk: steps per chunk
NCG = T // CG             # 50
WG = CG * BL              # 256
CH = 8                    # attention chunk: steps per chunk
NCHUNK = T // CH          # 25
W = CH * BL               # 512
G3 = 3 * D


def _mm(nc, out, lhsT, rhs, start, stop, skip=False):
    nc.tensor.matmul(out, lhsT, rhs, start=start, stop=stop,
                     skip_group_check=skip)


def build_nc():
    nc = bacc.Bacc("TRN2", target_bir_lowering=False)

    hist = nc.declare_dram_parameter("hist", [BL, T, D], bf16, isOutput=False)
    histTd = nc.declare_dram_parameter("histTd", [D, T * BL], bf16,
                                       isOutput=False)
    item = nc.declare_dram_parameter("item", [BL, D], f32, isOutput=False)
    user = nc.declare_dram_parameter("user", [BL, D], f32, isOutput=False)
    maskd = nc.declare_dram_parameter("maskd", [BL, T], f32, isOutput=False)
    seqd = nc.declare_dram_parameter("seqd", [BL, 1], i32, isOutput=False)
    WihTd = nc.declare_dram_parameter("WihT", [D, G3], bf16, isOutput=False)
    WhhTd = nc.declare_dram_parameter("WhhT", [D, G3], bf16, isOutput=False)
    bihTd = nc.declare_dram_parameter("bihT", [D, 3], f32, isOutput=False)
    bhhTd = nc.declare_dram_parameter("bhhT", [D, 3], f32, isOutput=False)
    W0Td = nc.declare_dram_parameter("W0T", [D, 240], bf16, isOutput=False)
    b0d = nc.declare_dram_parameter("b0", [80, 1], f32, isOutput=False)
    W1Td = nc.declare_dram_parameter("W1T", [80, 40], bf16, isOutput=False)
    b1d = nc.declare_dram_parameter("b1", [40, 1], f32, isOutput=False)
    W2Td = nc.declare_dram_parameter("W2T", [40, 1], bf16, isOutput=False)
    b2d = nc.declare_dram_parameter("b2", [1, 1], f32, isOutput=False)
    augWd = nc.declare_dram_parameter("augW", [D, 6 * D], bf16, isOutput=False)
    augbd = nc.declare_dram_parameter("augb", [D, 3], f32, isOutput=False)
    outWTd = nc.declare_dram_parameter("outWT", [D, 5], bf16, isOutput=False)
    outbd = nc.declare_dram_parameter("outb", [1, 1], f32, isOutput=False)
    outd = nc.declare_dram_parameter("out", [1, BL], f32, isOutput=True)

    with tile.TileContext(nc) as tc, ExitStack() as ctx:
        big = ctx.enter_context(tc.tile_pool(name="big", bufs=1))
        wp = ctx.enter_context(tc.tile_pool(name="wp", bufs=1))
        pp = ctx.enter_context(tc.tile_pool(name="pp", bufs=1))
        hp = ctx.enter_context(tc.tile_pool(name="hp", bufs=4))

        XT = big.tile([D, T * BL], bf16)
        gruT = big.tile([D, T * BL], bf16)

        ident = pp.tile([128, 128], f32)
        make_identity(nc, ident)
        identB = pp.tile([BL, BL], bf16)
        make_identity(nc, identB)
        identB128 = pp.tile([128, 128], bf16)
        make_identity(nc, identB128)

        # ------------- weights (bf16 arrive pre-converted via DMA) -------
        WihT = wp.tile([D, G3], bf16)
        nc.sync.dma_start(WihT[:, :], WihTd[:, :])
        WhhT = wp.tile([D, G3], bf16)
        nc.sync.dma_start(WhhT[:, :], WhhTd[:, :])
        W0T = wp.tile([D, 240], bf16)
        nc.sync.dma_start(W0T[:, :], W0Td[:, :])
        W1T = wp.tile([80, 40], bf16)
        nc.sync.dma_start(W1T[:, :], W1Td[:, :])
        W2T = wp.tile([40, 1], bf16)
        nc.sync.dma_start(W2T[:, :], W2Td[:, :])
        augW = wp.tile([D, 6 * D], bf16)
        nc.sync.dma_start(augW[:, :], augWd[:, :])
        outWT = wp.tile([D, 5], bf16)
        nc.sync.dma_start(outWT[:, :], outWTd[:, :])
        WrhT, WrxT = augW[:, 0:D], augW[:, D:2 * D]
        WuhT, WuxT = augW[:, 2 * D:3 * D], augW[:, 3 * D:4 * D]
        WahT, WaxT = augW[:, 4 * D:5 * D], augW[:, 5 * D:6 * D]

        bihT = wp.tile([D, 3], f32)
        nc.sync.dma_start(bihT[:, :], bihTd[:, :])
        bhhT = wp.tile([D, 3], f32)
        nc.sync.dma_start(bhhT[:, :], bhhTd[:, :])
        brz = wp.tile([D, 2], f32)
        nc.vector.tensor_add(brz[:, :], bihT[:, 0:2], bhhT[:, 0:2])
        nbz = wp.tile([D, 1], f32)
        nc.vector.tensor_scalar(nbz[:, :], brz[:, 1:2], -1.0, 0.0,
                                ALU.mult, ALU.add)
        b_hn, b_in = bhhT[:, 2:3], bihT[:, 2:3]

        b0v = wp.tile([80, 1], f32)
        nc.sync.dma_start(b0v[:, :], b0d[:, :])
        b1v = wp.tile([40, 1], f32)
        nc.sync.dma_start(b1v[:, :], b1d[:, :])
        b2v = wp.tile([1, 1], f32)
        nc.sync.dma_start(b2v[:, :], b2d[:, :])
        augb = wp.tile([D, 3], f32)
        nc.sync.dma_start(augb[:, :], augbd[:, :])
        outb = wp.tile([1, 1], f32)
        nc.sync.dma_start(outb[:, :], outbd[:, :])
        ab_r, ab_u, ab_h = augb[:, 0:1], augb[:, 1:2], augb[:, 2:3]

        ones_f = wp.tile([1, 128], f32)
        nc.vector.memset(ones_f[:, :], 1.0)
        ones1 = wp.tile([1, 128], bf16)
        nc.vector.memset(ones1[:, :], 1.0)

        # ---------------- preprocessing --------------------------------
        histT = pp.tile([D, BL], bf16)
        qT = pp.tile([D, BL], bf16)
        userT = pp.tile([D, BL], bf16)
        qRep = pp.tile([D, W], bf16)
        maskS = pp.tile([BL, T], f32)
        b2col = pp.tile([BL, 1], f32)
        histF = pp.tile([D, BL], f32)

        with tc.tile_pool(name="pre", bufs=8) as pre, \
             tc.tile_pool(name="pps", bufs=2, space="PSUM") as pps, \
             tc.tile_pool(name="hps", bufs=1, space="PSUM") as hps:
            # X^T arrives pre-transposed from the host; split the DMA so the
            # first GRU chunks can start as soon as their columns land.
            nc.sync.dma_start(XT[:, 0:4 * WG], histTd[:, 0:4 * WG])
            nc.sync.dma_start(XT[:, 4 * WG:25 * WG], histTd[:, 4 * WG:25 * WG])
            nc.sync.dma_start(XT[:, 25 * WG:T * BL], histTd[:, 25 * WG:T * BL])

            nc.sync.dma_start(maskS[:, :], maskd[:, :])
            seqi = pre.tile([BL, 1], i32, tag="seqi", bufs=1)
            nc.sync.dma_start(seqi[:, :], seqd[:, :])
            seqf = pre.tile([BL, 1], f32, tag="seqf", bufs=1)
            nc.vector.tensor_copy(seqf[:, :], seqi[:, :])
            rsec = pre.tile([BL, 1], f32, tag="rsec", bufs=1)
            nc.vector.reciprocal(rsec[:, :], seqf[:, :])
            maskSc = pre.tile([BL, T], f32, tag="maskSc", bufs=1)
            nc.vector.tensor_scalar_mul(maskSc[:, :], maskS[:, :], rsec[:, 0:1])

            mstp0 = pps.tile([128, BL], f32, tag="mstp")
            nc.tensor.transpose(mstp0[:, :], maskSc[:, 0:128], ident[0:BL, 0:BL])
            mst0 = pre.tile([128, BL], bf16, tag="mst0", bufs=1)
            nc.scalar.copy(mst0[:, :], mstp0[:, :])
            mstp1 = pps.tile([128, BL], f32, tag="mstp")
            nc.tensor.transpose(mstp1[0:72, :], maskSc[:, 128:200], ident[0:BL, 0:BL])
            mst1 = pre.tile([72, BL], bf16, tag="mst1", bufs=1)
            nc.scalar.copy(mst1[:, :], mstp1[0:72, :])

            itn = pre.tile([BL, D], f32, tag="itn", bufs=2)
            nc.sync.dma_start(itn[:, :], item[:, :])
            itp = pps.tile([D, BL], f32, tag="small_t")
            nc.tensor.transpose(itp[:, :], itn[:, :], ident[0:BL, 0:BL])
            nc.scalar.copy(qT[:, :], itp[:, :])
            usn = pre.tile([BL, D], f32, tag="itn", bufs=2)
            nc.sync.dma_start(usn[:, :], user[:, :])
            usp = pps.tile([D, BL], f32, tag="small_t")
            nc.tensor.transpose(usp[:, :], usn[:, :], ident[0:BL, 0:BL])
            nc.scalar.copy(userT[:, :], usp[:, :])
            b2p = pps.tile([BL, 1], f32, tag="small_t")
            _mm(nc, b2p[:, :], ones_f[0:1, 0:BL], b2v, start=True, stop=True)
            nc.scalar.copy(b2col[:, :], b2p[:, :])
            for s in range(CH):
                nc.scalar.copy(qRep[:, s * BL:(s + 1) * BL], qT[:, :])

            # masked history sum from natural-layout tiles; batched DMAs
            # (8 batch rows per dma_start) while the big X^T DMA streams in.
            histp0 = hps.tile([D, BL], f32, tag="histp0")
            histp1 = hps.tile([D, BL], f32, tag="histp1")
            GB = 8
            for h, (t0, tn) in enumerate(((0, 128), (128, 72))):
                histp = histp0 if h == 0 else histp1
                for b0 in range(0, BL, GB):
                    xn = pre.tile([128, GB * D], bf16, tag="xn", bufs=3)
                    xn3 = xn.rearrange("t (b d) -> t b d", d=D)
                    nc.sync.dma_start(
                        xn3[0:tn, :, :],
                        hist[b0:b0 + GB, t0:t0 + tn, :]
                        .rearrange("b t d -> t b d"))
                    for bi in range(GB):
                        b = b0 + bi
                        mcol = (mst0 if h == 0 else mst1)[0:tn, b:b + 1]
                        _mm(nc, histp[:, b:b + 1],
                            xn3[0:tn, bi, :], mcol, start=True, stop=True)
            nc.scalar.copy(histF[:, :], histp0[:, :])
            nc.vector.tensor_add(histF[:, :], histF[:, :], histp1[:, :])
            nc.scalar.copy(histT[:, :], histF[:, :])

        # ---------------- GRU + interleaved attention -------------------
        gruT3 = gruT.rearrange("p (t b) -> p t b", b=BL)
        SIG, TANH = AF.Sigmoid, AF.Tanh
        wgt = pp.tile([BL, T], f32)

        with tc.tile_pool(name="gip", bufs=2, space="PSUM") as gip, \
             tc.tile_pool(name="hnp", bufs=1, space="PSUM") as hnp, \
             tc.tile_pool(name="gt", bufs=6) as gt, \
             tc.tile_pool(name="aps", bufs=1, space="PSUM") as aps, \
             tc.tile_pool(name="sps", bufs=1, space="PSUM") as sps, \
             tc.tile_pool(name="at", bufs=3) as at:
            scp = sps.tile([BL, T], f32)

            def attention_pieces(a):
                """Attention for gru chunks [2a, 2a+1] as 8 emission slots;
                each slot is small PE/DVE/Pool work sized to fill one GRU
                step's engine-idle window without head-of-line-blocking the
                recurrence chain's FIFO queues."""
                gc = gruT[:, a * W:(a + 1) * W]
                r2 = at.tile([D, W], bf16, tag="r2")
                y0p = aps.tile([80, W], f32, tag="y0")
                y0 = at.tile([80, W], bf16, tag="y0s")
                y1p = aps.tile([40, W], f32, tag="y1")
                y1 = at.tile([40, W], bf16, tag="y1s")
                HW = 256

                def relu0(h, eng):
                    if eng is nc.scalar:
                        nc.scalar.activation(y0[:, h * HW:(h + 1) * HW],
                                             y0p[:, h * HW:(h + 1) * HW],
                                             AF.Relu, bias=b0v[:, 0:1])
                    else:
                        eng.tensor_scalar(y0[:, h * HW:(h + 1) * HW],
                                          y0p[:, h * HW:(h + 1) * HW],
                                          b0v[:, 0:1], 0.0, ALU.add, ALU.max)
                def relu1(h, eng):
                    if eng is nc.scalar:
                        nc.scalar.activation(y1[:, h * HW:(h + 1) * HW],
                                             y1p[:, h * HW:(h + 1) * HW],
                                             AF.Relu, bias=b1v[:, 0:1])
                    else:
                        eng.tensor_scalar(y1[:, h * HW:(h + 1) * HW],
                                          y1p[:, h * HW:(h + 1) * HW],
                                          b1v[:, 0:1], 0.0, ALU.add, ALU.max)
                def score(s):
                    t = a * CH + s
                    _mm(nc, scp[:, t:t + 1], y1[:, s * BL:(s + 1) * BL],
                        W2T, start=True, stop=True)

                def p0():
                    nc.gpsimd.tensor_mul(r2[:, 0:HW], gc[:, 0:HW],
                                         qRep[:, 0:HW])
                def p1():
                    nc.gpsimd.tensor_mul(r2[:, HW:W], gc[:, HW:W],
                                         qRep[:, HW:W])
                    _mm(nc, y0p[:, 0:HW], W0T[:, 0:80], gc[:, 0:HW],
                        start=True, stop=False)
                def p2():
                    _mm(nc, y0p[:, HW:W], W0T[:, 0:80], gc[:, HW:W],
                        start=True, stop=False)
                    _mm(nc, y0p[:, 0:HW], W0T[:, 80:160], qRep[:, 0:HW],
                        start=False, stop=False)
                def p3():
                    _mm(nc, y0p[:, HW:W], W0T[:, 80:160], qRep[:, HW:W],
                        start=False, stop=False)
                    _mm(nc, y0p[:, 0:HW], W0T[:, 160:240], r2[:, 0:HW],
                        start=False, stop=True)
                def p4():
                    _mm(nc, y0p[:, HW:W], W0T[:, 160:240], r2[:, HW:W],
                        start=False, stop=True)
                    relu0(0, nc.scalar)
                def p5():
                    relu0(1, nc.vector)
                def p6():
                    _mm(nc, y1p[:, 0:HW], W1T, y0[:, 0:HW],
                        start=True, stop=True)
                    relu1(0, nc.scalar)
                    for s in range(4):
                        score(s)
                def p7():
                    _mm(nc, y1p[:, HW:W], W1T, y0[:, HW:W],
                        start=True, stop=True)
                    relu1(1, nc.vector)
                    for s in range(4, CH):
                        score(s)
                return [p0, p1, p2, p3, p4, p5, p6, p7]

            pending = []          # attention pieces awaiting emission
            WhT_r, WhT_z, WhT_n = (WhhT[:, 0:D], WhhT[:, D:2 * D],
                                   WhhT[:, 2 * D:G3])
            gslot = [None, None]  # (prz, prz4, pn3) per psum buf

            def gru_proj_pieces(c):
                XTc = XT[:, c * WG:(c + 1) * WG]
                prz = gip.tile([D, CG * 128], f32, tag="girz")
                pn = gip.tile([D, CG * BL], f32, tag="gin")
                gslot[c % 2] = (
                    prz,
                    prz.rearrange("p (g s c) -> p g s c", g=2, c=BL),
                    pn.rearrange("p (s g) -> p s g", g=BL),
                )
                return [
                    lambda: _mm(nc, prz[:, 0:WG], WihT[:, 0:D], XTc,
                                start=True, stop=True),
                    lambda: _mm(nc, prz[:, WG:2 * WG], WihT[:, D:2 * D], XTc,
                                start=True, stop=True),
                    lambda: _mm(nc, pn[:, :], WihT[:, 2 * D:G3], XTc,
                                start=True, stop=True),
                ]

            for p in gru_proj_pieces(0):
                p()
            zh_prev = [None]      # z_{t-1} * h_{t-2}
            omzn_prev = [None]    # (1-z_{t-1}) * n_{t-1}
            for c in range(NCG):
                prz, prz4, pn3 = gslot[c % 2]
                for s in range(CG):
                    t = c * CG + s
                    # --- PE block A: recurrent matmuls feeding step t ----
                    hx = hnp.tile([D, 2 * BL], f32, tag="hn2")
                    hn, tmp2 = hx[:, 0:BL], hx[:, BL:2 * BL]
                    if t == 1:
                        h0 = gruT3[:, 0, :]
                        _mm(nc, prz[:, s * BL:(s + 1) * BL], WhT_r, h0,
                            start=False, stop=True, skip=True)
                        _mm(nc, prz[:, WG + s * BL:WG + (s + 1) * BL],
                            WhT_z, h0, start=False, stop=True, skip=True)
                        _mm(nc, hn[:, :], WhT_n, h0, start=True, stop=True)
                    elif t > 1:
                        rsl = prz[:, s * BL:(s + 1) * BL]
                        _mm(nc, rsl, WhT_r, zh_prev[0],
                            start=False, stop=False, skip=True)
                        _mm(nc, rsl, WhT_r, omzn_prev[0],
                            start=False, stop=True, skip=True)
                        hprev = gruT3[:, t - 1, :]
                        _mm(nc, prz[:, WG + s * BL:WG + (s + 1) * BL],
                            WhT_z, hprev, start=False, stop=True, skip=True)
                        _mm(nc, hn[:, :], WhT_n, hprev, start=True, stop=True)
                    # --- gate activations (biases ride ACT) --------------
                    r = gt.tile([D, BL], bf16, tag="r")
                    nc.scalar.activation(r[:, :], prz4[:, 0, s, :], SIG,
                                         bias=brz[:, 0:1])
                    z = gt.tile([D, BL], bf16, tag="z")
                    nc.scalar.activation(z[:, :], prz4[:, 1, s, :], SIG,
                                         bias=brz[:, 1:2])
                    omz = gt.tile([D, BL], bf16, tag="omz")
                    nc.gpsimd.tensor_scalar(omz[:, :], z[:, :], -1.0, 1.0,
                                            ALU.mult, ALU.add)
                    tmp = gt.tile([D, BL], bf16, tag="tmp")
                    if t > 0:
                        nc.vector.scalar_tensor_tensor(
                            tmp[:, :], hn[:, :], b_hn, r[:, :],
                            ALU.add, ALU.mult)
                    else:
                        nc.vector.tensor_scalar_mul(tmp[:, :], r[:, :], b_hn)
                    nc.vector.tensor_add(tmp2[:, :], tmp[:, :], pn3[:, s, :])
                    if t > 0:
                        zh = gt.tile([D, BL], bf16, tag="zh")
                        nc.vector.tensor_mul(zh[:, :], z[:, :],
                                             gruT3[:, t - 1, :])
                    n = gt.tile([D, BL], bf16, tag="n")
                    nc.scalar.activation(n[:, :], tmp2[:, :], TANH, bias=b_in)
                    hdst = gruT3[:, t, :]
                    if t > 0:
                        omzn = gt.tile([D, BL], bf16, tag="omzn")
                        nc.vector.tensor_mul(omzn[:, :], omz[:, :], n[:, :])
                        nc.vector.tensor_add(hdst, omzn[:, :], zh[:, :])
                        zh_prev[0], omzn_prev[0] = zh, omzn
                    else:
                        nc.vector.tensor_mul(hdst, omz[:, :], n[:, :])
                    # --- PE block B: hoisted chunk/attention work --------
                    if s == 1 and c + 1 < NCG:
                        proj_pieces = gru_proj_pieces(c + 1)
                    if s >= 1 and c + 1 < NCG:
                        proj_pieces[s - 1]()
                    if pending:
                        pending.pop(0)()
                if c % 2 == 1:
                    pending.extend(attention_pieces(c // 2))
            for p in pending:
                p()

            # masked softmax over t (b-major); last MLP layer ReLU'd w/ b2
            rawr = at.tile([BL, T], f32, tag="rawr")
            nc.scalar.activation(rawr[:, :], scp[:, :], AF.Relu,
                                 bias=b2col[:, 0:1])
            rawm = at.tile([BL, T], f32, tag="rawm")
            nc.vector.tensor_mul(rawm[:, :], rawr[:, :], maskS[:, :])
            mxn = at.tile([BL, 1], f32, tag="mxn")
            nc.vector.tensor_reduce(mxn[:, :], rawm[:, :],
                                    axis=mybir.AxisListType.X,
                                    op=ALU.max, negate=True)
            ex = at.tile([BL, T], f32, tag="ex")
            nc.scalar.activation(ex[:, :], rawr[:, :], AF.Exp, bias=mxn[:, 0:1])
            em = at.tile([BL, T], f32, tag="em")
            nc.vector.tensor_mul(em[:, :], ex[:, :], maskS[:, :])
            sm = at.tile([BL, 1], f32, tag="sm")
            nc.vector.tensor_reduce(sm[:, :], em[:, :],
                                    axis=mybir.AxisListType.X, op=ALU.add)
            rs = at.tile([BL, 1], f32, tag="rs")
            nc.vector.reciprocal(rs[:, :], sm[:, :])
            nc.vector.tensor_scalar_mul(wgt[:, :], em[:, :], rs[:, 0:1])

        # attention weights go to partition-0 bf16 rows per AUGRU chunk
        # (the row transposes + broadcasts overlap the AUGRU itself)
        wgt_bf = pp.tile([BL, T], bf16)
        nc.scalar.copy(wgt_bf[:, :], wgt[:, :])

        # ---------------- AUGRU ----------------------------------------
        hA = [None]
        with tc.tile_pool(name="axp", bufs=2, space="PSUM") as axp, \
             tc.tile_pool(name="abp", bufs=2, space="PSUM") as abp, \
             tc.tile_pool(name="rtp", bufs=2, space="PSUM") as rtp, \
             tc.tile_pool(name="ut", bufs=6) as ut:
            aslot = [None, None]  # (pru, pru4, pxh3, pab) per psum buf

            def aug_proj_pieces(c):
                gc = gruT[:, c * WG:(c + 1) * WG]
                rowp = rtp.tile([1, WG], f32, tag="rowp")
                rows = ut.tile([1, WG], bf16, tag="rows")
                pru = axp.tile([D, CG * 128], f32, tag="pxru")
                pxh = axp.tile([D, CG * BL], f32, tag="pxh")
                pab = abp.tile([D, WG], f32, tag="pab")
                aslot[c % 2] = (
                    pru,
                    pru.rearrange("p (g s c) -> p g s c", g=2, c=BL),
                    pxh.rearrange("p (s g) -> p s g", g=BL),
                    pab,
                )

                def pa():
                    for s in range(CG):
                        t = c * CG + s
                        _mm(nc, rowp[0:1, s * BL:(s + 1) * BL],
                            wgt_bf[:, t:t + 1], identB, start=True, stop=True)
                    nc.vector.tensor_copy(rows[0:1, 0:BL * 2],
                                          rowp[0:1, 0:BL * 2])
                def pb():
                    nc.vector.tensor_copy(rows[0:1, BL * 2:WG],
                                          rowp[0:1, BL * 2:WG])
                    _mm(nc, pru[:, 0:WG], WrxT, gc, start=True, stop=True)
                def pc():
                    _mm(nc, pab[:, :], ones1, rows[0:1, :],
                        start=True, stop=True)
                    _mm(nc, pru[:, WG:2 * WG], WuxT, gc, start=True, stop=True)
                def pd():
                    _mm(nc, pxh[:, :], WaxT, gc, start=True, stop=True)
                return [pa, pb, pc, pd]

            for p in aug_proj_pieces(0):
                p()
            hA_pp = [None]        # hA_{t-2}
            ud_prev = [None]      # ud_{t-1}
            for c in range(NCG):
                pru, pru4, pxh3, pab = aslot[c % 2]
                for s in range(CG):
                    t = c * CG + s
                    hAg = hA[0]
                    # --- PE block A: recurrent matmuls feeding step t ----
                    rsl = pru[:, s * BL:(s + 1) * BL]
                    usl = pru[:, WG + s * BL:WG + (s + 1) * BL]
                    if t == 1:
                        _mm(nc, rsl, WrhT, hAg, start=False, stop=True,
                            skip=True)
                        _mm(nc, usl, WuhT, hAg, start=False, stop=True,
                            skip=True)
                    elif t > 1:
                        _mm(nc, rsl, WrhT, hA_pp[0],
                            start=False, stop=False, skip=True)
                        _mm(nc, rsl, WrhT, ud_prev[0],
                            start=False, stop=True, skip=True)
                        _mm(nc, usl, WuhT, hA_pp[0],
                            start=False, stop=False, skip=True)
                        _mm(nc, usl, WuhT, ud_prev[0],
                            start=False, stop=True, skip=True)
                    r = ut.tile([D, BL], bf16, tag="ar")
                    nc.scalar.activation(r[:, :], pru4[:, 0, s, :], SIG,
                                         bias=ab_r)
                    u = ut.tile([D, BL], bf16, tag="au")
                    nc.scalar.activation(u[:, :], pru4[:, 1, s, :], SIG,
                                         bias=ab_u)
                    if t > 0:
                        hr = ut.tile([D, BL], bf16, tag="ahr")
                        nc.vector.tensor_mul(hr[:, :], hAg, r[:, :])
                        _mm(nc, pxh3[:, s, :], WahT, hr,
                            start=False, stop=True, skip=True)
                    up = ut.tile([D, BL], f32, tag="aup")
                    nc.vector.tensor_mul(up[:, :], pab[:, s * BL:(s + 1) * BL],
                                         u[:, :])
                    hh = ut.tile([D, BL], bf16, tag="ahh")
                    nc.scalar.activation(hh[:, :], pxh3[:, s, :], TANH,
                                         bias=ab_h)
                    hnew = hp.tile([D, BL], bf16, tag="hA")
                    if t > 0:
                        dd = ut.tile([D, BL], bf16, tag="add")
                        nc.vector.tensor_sub(dd[:, :], hh[:, :], hAg)
                        ud = ut.tile([D, BL], bf16, tag="aud")
                        nc.vector.tensor_mul(ud[:, :], up[:, :], dd[:, :])
                        nc.gpsimd.tensor_add(hnew[:, :], hAg, ud[:, :])
                        hA_pp[0], ud_prev[0] = hAg, ud
                    else:
                        nc.vector.tensor_mul(hnew[:, :], up[:, :], hh[:, :])
                    hA[0] = hnew
                    # --- PE block B: hoisted next-chunk work -------------
                    if c + 1 < NCG:
                        if s == 0:
                            aproj_pieces = aug_proj_pieces(c + 1)
                        aproj_pieces[s]()

        # ---------------- output layer ---------------------------------
        with tc.tile_pool(name="ops", bufs=1, space="PSUM") as ops, \
             tc.tile_pool(name="ot", bufs=1) as ot:
            ih = ot.tile([D, BL], bf16)
            nc.vector.tensor_mul(ih[:, :], qT[:, :], histF[:, :])
            po = ops.tile([1, BL], f32)
            pieces = [userT, qT, histT, ih, hA[0]]
            for g, piece in enumerate(pieces):
                _mm(nc, po[:, :], outWT[:, g:g + 1], piece,
                    start=(g == 0), stop=(g == 4))
            outs = ot.tile([1, BL], f32)
            nc.scalar.activation(outs[:, :], po[:, :], AF.Identity,
                                 bias=outb[:, 0:1])
            nc.sync.dma_start(outd[:, :], outs[:, :])

    nc.finalize()
    return nc


_NC = None


def _get_nc():
    global _NC
    if _NC is None:
        _NC = build_nc()
    return _NC


def make_in_maps(inputs):
    """Slice full inputs into per-core input maps (host-side layout only)."""
    f = {k: np.asarray(v) for k, v in inputs.items()}
    WihT = np.ascontiguousarray(f["gru_Wih"].T)          # (128, 384)
    WhhT = np.ascontiguousarray(f["gru_Whh"].T)
    bihT = np.ascontiguousarray(f["gru_bih"].reshape(3, D).T)  # (128, 3)
    bhhT = np.ascontiguousarray(f["gru_bhh"].reshape(3, D).T)
    # fold the (q - fact) block into the fact/q blocks:
    # W0_f@f + W0_q@q + W0_fq@(f*q) + W0_qf@(q-f)
    #   = (W0_f - W0_qf)@f + (W0_q + W0_qf)@q + W0_fq@(f*q)
    W0 = f["attn_W0"]                                    # (80, 4D)
    W0f = W0[:, 0:D] - W0[:, 3 * D:4 * D]
    W0q = W0[:, D:2 * D] + W0[:, 3 * D:4 * D]
    W0fq = W0[:, 2 * D:3 * D]
    W0T = np.ascontiguousarray(
        np.concatenate([W0f.T, W0q.T, W0fq.T], axis=1))  # (D, 240)
    b0 = np.ascontiguousarray(f["attn_b0"].reshape(80, 1))
    W1T = np.ascontiguousarray(f["attn_W1"].T)           # (80, 40)
    b1 = np.ascontiguousarray(f["attn_b1"].reshape(40, 1))
    W2T = np.ascontiguousarray(f["attn_W2"].T)           # (40, 1)
    b2 = f["attn_b2"].reshape(1, 1)
    augW = np.concatenate(
        [np.ascontiguousarray(f[k][:, p * D:(p + 1) * D].T)
         for k in ("aug_Wr", "aug_Wu", "aug_Wh") for p in (0, 1)],
        axis=1)                                          # (128, 768)
    augb = np.stack([f["aug_br"], f["aug_bu"], f["aug_bh"]], axis=1)  # (128,3)
    outWT = np.ascontiguousarray(f["out_W"].reshape(5, D).T)          # (128,5)
    outb = f["out_b"].reshape(1, 1)

    shared_bf = dict(WihT=WihT, WhhT=WhhT, W0T=W0T, W1T=W1T, W2T=W2T,
                     augW=augW, outWT=outWT)
    shared = dict(bihT=bihT, bhhT=bhhT, b0=b0, b1=b1, b2=b2, augb=augb,
                  outb=outb)
    shared = {k: np.ascontiguousarray(v.astype(np.float32)) for k, v in
              shared.items()}
    shared.update({k: np.ascontiguousarray(v.astype(BF)) for k, v in
                   shared_bf.items()})

    in_maps = []
    for c in range(NCORES):
        s = slice(c * BL, (c + 1) * BL)
        m = dict(shared)
        hs = f["item_historical_embedding"][s].astype(BF)
        m["hist"] = np.ascontiguousarray(hs)
        m["histTd"] = np.ascontiguousarray(
            hs.transpose(2, 1, 0).reshape(D, T * BL))
        m["item"] = np.ascontiguousarray(f["item_embedding"][s].astype(np.float32))
        m["user"] = np.ascontiguousarray(f["user_embedding"][s].astype(np.float32))
        m["maskd"] = np.ascontiguousarray(f["mask"][s].astype(np.float32))
        m["seqd"] = np.ascontiguousarray(
            f["sequential_length"][s].reshape(BL, 1).astype(np.int32))
        in_maps.append(m)
    return in_maps


def kernel(**inputs) -> np.ndarray:
    nc = _get_nc()
    in_maps = make_in_maps(inputs)
    res = run_bass_kernel_spmd(nc, in_maps, list(range(NCORES)))
    return np.concatenate(
        [np.asarray(res.results[c]["out"]).reshape(BL) for c in range(NCORES)])


# revision 32
# speedup vs baseline: 1.2747x; 1.0070x over previous
"""DIEN (GRU + attention + AUGRU) Trainium2 kernel.

Data-parallel over 8 NeuronCores: each core handles a batch slice of 64.
All on-chip state is feature-major (feature dim on SBUF partitions, batch
on the free dim), so the two sequential recurrences need no per-step
transposes.

Structure:
  - X^T (D, T*64) in bf16, built via PE transposes of natural-layout DMA
    tiles; the same stationary tiles also accumulate the masked history
    sum (f32) via N=1 matmuls against 1/len-scaled mask columns.
  - GRU input projections land in PSUM chunks laid out (128, 4, [r|z|n]).
    Gate sigmoids are split per-gate so the gate bias rides the ACT
    instruction (no DVE bias adds), and (1-z) comes from sigmoid(-x).
    The combine uses h' = (1-z)*n + z*h with z*h computed off the
    critical path.
  - Attention runs per 8-step chunk interleaved with the GRU loop; its
    elementwise prep (fact*q, q-fact) runs on the idle GPSIMD engine so
    the DVE stays free for the recurrence chain.
  - AUGRU consumes attention weights via rows pre-transposed once after
    the softmax; per chunk a single ones-weight load broadcasts 4 rows
    into PSUM. Masked steps have weight 0 (u'=0, h unchanged), so the
    final AUGRU state IS the gathered aug_out[b, len-1] - no gather.
  - Matmuls are bf16 (f32 PSUM accumulation); elementwise recurrence ops
    run bf16 where both operands sit in SBUF (2x DVE mode); the masked
    history mean and softmax stay f32.
"""

import sys

if "/opt/trn_rl_repo" not in sys.path:
    sys.path.insert(0, "/opt/trn_rl_repo")

from contextlib import ExitStack

import ml_dtypes
import numpy as np

import concourse.bacc as bacc
import concourse.bass as bass
import concourse.mybir as mybir
import concourse.tile as tile
from concourse.bass_utils import run_bass_kernel_spmd
from concourse.masks import make_identity

f32 = mybir.dt.float32
bf16 = mybir.dt.bfloat16
i32 = mybir.dt.int32
AF = mybir.ActivationFunctionType
ALU = mybir.AluOpType
BF = ml_dtypes.bfloat16

NCORES = 8
B, T, D = 512, 200, 128
BL = B // NCORES          # 64 batch rows per core
CG = 4                    # GRU/AUGRU psum chunk: steps per chunk
NCG = T // CG             # 50
WG = CG * BL              # 256
CH = 8                    # attention chunk: steps per chunk
NCHUNK = T // CH          # 25
W = CH * BL               # 512
G3 = 3 * D


def _mm(nc, out, lhsT, rhs, start, stop, skip=False):
    nc.tensor.matmul(out, lhsT, rhs, start=start, stop=stop,
                     skip_group_check=skip)


def build_nc():
    nc = bacc.Bacc("TRN2", target_bir_lowering=False)

    hist = nc.declare_dram_parameter("hist", [BL, T, D], bf16, isOutput=False)
    histTd = nc.declare_dram_parameter("histTd", [D, T * BL], bf16,
                                       isOutput=False)
    item = nc.declare_dram_parameter("item", [BL, D], f32, isOutput=False)
    user = nc.declare_dram_parameter("user", [BL, D], f32, isOutput=False)
    maskd = nc.declare_dram_parameter("maskd", [BL, T], f32, isOutput=False)
    seqd = nc.declare_dram_parameter("seqd", [BL, 1], i32, isOutput=False)
    WihTd = nc.declare_dram_parameter("WihT", [D, G3], bf16, isOutput=False)
    WhhTd = nc.declare_dram_parameter("WhhT", [D, G3], bf16, isOutput=False)
    bihTd = nc.declare_dram_parameter("bihT", [D, 3], f32, isOutput=False)
    bhhTd = nc.declare_dram_parameter("bhhT", [D, 3], f32, isOutput=False)
    W0Td = nc.declare_dram_parameter("W0T", [D, 240], bf16, isOutput=False)
    b0d = nc.declare_dram_parameter("b0", [80, 1], f32, isOutput=False)
    W1Td = nc.declare_dram_parameter("W1T", [80, 40], bf16, isOutput=False)
    b1d = nc.declare_dram_parameter("b1", [40, 1], f32, isOutput=False)
    W2Td = nc.declare_dram_parameter("W2T", [40, 1], bf16, isOutput=False)
    b2d = nc.declare_dram_parameter("b2", [1, 1], f32, isOutput=False)
    augWd = nc.declare_dram_parameter("augW", [D, 6 * D], bf16, isOutput=False)
    augbd = nc.declare_dram_parameter("augb", [D, 3], f32, isOutput=False)
    outWTd = nc.declare_dram_parameter("outWT", [D, 5], bf16, isOutput=False)
    outbd = nc.declare_dram_parameter("outb", [1, 1], f32, isOutput=False)
    outd = nc.declare_dram_parameter("out", [1, BL], f32, isOutput=True)

    with tile.TileContext(nc) as tc, ExitStack() as ctx:
        big = ctx.enter_context(tc.tile_pool(name="big", bufs=1))
        wp = ctx.enter_context(tc.tile_pool(name="wp", bufs=1))
        pp = ctx.enter_context(tc.tile_pool(name="pp", bufs=1))
        hp = ctx.enter_context(tc.tile_pool(name="hp", bufs=4))

        XT = big.tile([D, T * BL], bf16)
        gruT = big.tile([D, T * BL], bf16)

        ident = pp.tile([128, 128], f32)
        make_identity(nc, ident)
        identB = pp.tile([BL, BL], bf16)
        make_identity(nc, identB)
        identB128 = pp.tile([128, 128], bf16)
        make_identity(nc, identB128)

        # ------------- weights (bf16 arrive pre-converted via DMA) -------
        WihT = wp.tile([D, G3], bf16)
        nc.sync.dma_start(WihT[:, :], WihTd[:, :])
        WhhT = wp.tile([D, G3], bf16)
        nc.sync.dma_start(WhhT[:, :], WhhTd[:, :])
        W0T = wp.tile([D, 240], bf16)
        nc.sync.dma_start(W0T[:, :], W0Td[:, :])
        W1T = wp.tile([80, 40], bf16)
        nc.sync.dma_start(W1T[:, :], W1Td[:, :])
        W2T = wp.tile([40, 1], bf16)
        nc.sync.dma_start(W2T[:, :], W2Td[:, :])
        augW = wp.tile([D, 6 * D], bf16)
        nc.sync.dma_start(augW[:, :], augWd[:, :])
        outWT = wp.tile([D, 5], bf16)
        nc.sync.dma_start(outWT[:, :], outWTd[:, :])
        WrhT, WrxT = augW[:, 0:D], augW[:, D:2 * D]
        WuhT, WuxT = augW[:, 2 * D:3 * D], augW[:, 3 * D:4 * D]
        WahT, WaxT = augW[:, 4 * D:5 * D], augW[:, 5 * D:6 * D]

        bihT = wp.tile([D, 3], f32)
        nc.sync.dma_start(bihT[:, :], bihTd[:, :])
        bhhT = wp.tile([D, 3], f32)
        nc.sync.dma_start(bhhT[:, :], bhhTd[:, :])
        brz = wp.tile([D, 2], f32)
        nc.vector.tensor_add(brz[:, :], bihT[:, 0:2], bhhT[:, 0:2])
        nbz = wp.tile([D, 1], f32)
        nc.vector.tensor_scalar(nbz[:, :], brz[:, 1:2], -1.0, 0.0,
                                ALU.mult, ALU.add)
        b_hn, b_in = bhhT[:, 2:3], bihT[:, 2:3]

        b0v = wp.tile([80, 1], f32)
        nc.sync.dma_start(b0v[:, :], b0d[:, :])
        b1v = wp.tile([40, 1], f32)
        nc.sync.dma_start(b1v[:, :], b1d[:, :])
        b2v = wp.tile([1, 1], f32)
        nc.sync.dma_start(b2v[:, :], b2d[:, :])
        augb = wp.tile([D, 3], f32)
        nc.sync.dma_start(augb[:, :], augbd[:, :])
        outb = wp.tile([1, 1], f32)
        nc.sync.dma_start(outb[:, :], outbd[:, :])
        ab_r, ab_u, ab_h = augb[:, 0:1], augb[:, 1:2], augb[:, 2:3]

        ones_f = wp.tile([1, 128], f32)
        nc.vector.memset(ones_f[:, :], 1.0)
        ones1 = wp.tile([1, 128], bf16)
        nc.vector.memset(ones1[:, :], 1.0)

        # ---------------- preprocessing --------------------------------
        histT = pp.tile([D, BL], bf16)
        qT = pp.tile([D, BL], bf16)
        userT = pp.tile([D, BL], bf16)
        qRep = pp.tile([D, W], bf16)
        maskS = pp.tile([BL, T], f32)
        b2col = pp.tile([BL, 1], f32)
        histF = pp.tile([D, BL], f32)

        with tc.tile_pool(name="pre", bufs=8) as pre, \
             tc.tile_pool(name="pps", bufs=2, space="PSUM") as pps, \
             tc.tile_pool(name="hps", bufs=1, space="PSUM") as hps:
            # X^T arrives pre-transposed from the host; split the DMA so the
            # first GRU chunks can start as soon as their columns land.
            nc.sync.dma_start(XT[:, 0:4 * WG], histTd[:, 0:4 * WG])
            nc.sync.dma_start(XT[:, 4 * WG:25 * WG], histTd[:, 4 * WG:25 * WG])
            nc.sync.dma_start(XT[:, 25 * WG:T * BL], histTd[:, 25 * WG:T * BL])

            nc.sync.dma_start(maskS[:, :], maskd[:, :])
            seqi = pre.tile([BL, 1], i32, tag="seqi", bufs=1)
            nc.sync.dma_start(seqi[:, :], seqd[:, :])
            seqf = pre.tile([BL, 1], f32, tag="seqf", bufs=1)
            nc.vector.tensor_copy(seqf[:, :], seqi[:, :])
            rsec = pre.tile([BL, 1], f32, tag="rsec", bufs=1)
            nc.vector.reciprocal(rsec[:, :], seqf[:, :])
            maskSc = pre.tile([BL, T], f32, tag="maskSc", bufs=1)
            nc.vector.tensor_scalar_mul(maskSc[:, :], maskS[:, :], rsec[:, 0:1])

            mstp0 = pps.tile([128, BL], f32, tag="mstp")
            nc.tensor.transpose(mstp0[:, :], maskSc[:, 0:128], ident[0:BL, 0:BL])
            mst0 = pre.tile([128, BL], bf16, tag="mst0", bufs=1)
            nc.scalar.copy(mst0[:, :], mstp0[:, :])
            mstp1 = pps.tile([128, BL], f32, tag="mstp")
            nc.tensor.transpose(mstp1[0:72, :], maskSc[:, 128:200], ident[0:BL, 0:BL])
            mst1 = pre.tile([72, BL], bf16, tag="mst1", bufs=1)
            nc.scalar.copy(mst1[:, :], mstp1[0:72, :])

            itn = pre.tile([BL, D], f32, tag="itn", bufs=2)
            nc.sync.dma_start(itn[:, :], item[:, :])
            itp = pps.tile([D, BL], f32, tag="small_t")
            nc.tensor.transpose(itp[:, :], itn[:, :], ident[0:BL, 0:BL])
            nc.scalar.copy(qT[:, :], itp[:, :])
            usn = pre.tile([BL, D], f32, tag="itn", bufs=2)
            nc.sync.dma_start(usn[:, :], user[:, :])
            usp = pps.tile([D, BL], f32, tag="small_t")
            nc.tensor.transpose(usp[:, :], usn[:, :], ident[0:BL, 0:BL])
            nc.scalar.copy(userT[:, :], usp[:, :])
            b2p = pps.tile([BL, 1], f32, tag="small_t")
            _mm(nc, b2p[:, :], ones_f[0:1, 0:BL], b2v, start=True, stop=True)
            nc.scalar.copy(b2col[:, :], b2p[:, :])
            for s in range(CH):
                nc.scalar.copy(qRep[:, s * BL:(s + 1) * BL], qT[:, :])

            # masked history sum from natural-layout tiles; batched DMAs
            # (8 batch rows per dma_start) while the big X^T DMA streams in.
            histp0 = hps.tile([D, BL], f32, tag="histp0")
            histp1 = hps.tile([D, BL], f32, tag="histp1")
            GB = 8
            for h, (t0, tn) in enumerate(((0, 128), (128, 72))):
                histp = histp0 if h == 0 else histp1
                for b0 in range(0, BL, GB):
                    xn = pre.tile([128, GB * D], bf16, tag="xn", bufs=3)
                    xn3 = xn.rearrange("t (b d) -> t b d", d=D)
                    nc.sync.dma_start(
                        xn3[0:tn, :, :],
                        hist[b0:b0 + GB, t0:t0 + tn, :]
                        .rearrange("b t d -> t b d"))
                    for bi in range(GB):
                        b = b0 + bi
                        mcol = (mst0 if h == 0 else mst1)[0:tn, b:b + 1]
                        _mm(nc, histp[:, b:b + 1],
                            xn3[0:tn, bi, :], mcol, start=True, stop=True)
            nc.scalar.copy(histF[:, :], histp0[:, :])
            nc.vector.tensor_add(histF[:, :], histF[:, :], histp1[:, :])
            nc.scalar.copy(histT[:, :], histF[:, :])

        # ---------------- GRU + interleaved attention -------------------
        gruT3 = gruT.rearrange("p (t b) -> p t b", b=BL)
        SIG, TANH = AF.Sigmoid, AF.Tanh
        wgt = pp.tile([BL, T], f32)

        with tc.tile_pool(name="gip", bufs=2, space="PSUM") as gip, \
             tc.tile_pool(name="hnp", bufs=1, space="PSUM") as hnp, \
             tc.tile_pool(name="gt", bufs=6) as gt, \
             tc.tile_pool(name="aps", bufs=1, space="PSUM") as aps, \
             tc.tile_pool(name="sps", bufs=1, space="PSUM") as sps, \
             tc.tile_pool(name="at", bufs=3) as at:
            scp = sps.tile([BL, T], f32)

            def attention_pieces(a):
                """Attention for gru chunks [2a, 2a+1] as 8 emission slots;
                each slot is small PE/DVE/Pool work sized to fill one GRU
                step's engine-idle window without head-of-line-blocking the
                recurrence chain's FIFO queues."""
                gc = gruT[:, a * W:(a + 1) * W]
                r2 = at.tile([D, W], bf16, tag="r2")
                y0p = aps.tile([80, W], f32, tag="y0")
                y0 = at.tile([80, W], bf16, tag="y0s")
                y1p = aps.tile([40, W], f32, tag="y1")
                y1 = at.tile([40, W], bf16, tag="y1s")
                HW = 256

                def relu0(h, eng):
                    if eng is nc.scalar:
                        nc.scalar.activation(y0[:, h * HW:(h + 1) * HW],
                                             y0p[:, h * HW:(h + 1) * HW],
                                             AF.Relu, bias=b0v[:, 0:1])
                    else:
                        eng.tensor_scalar(y0[:, h * HW:(h + 1) * HW],
                                          y0p[:, h * HW:(h + 1) * HW],
                                          b0v[:, 0:1], 0.0, ALU.add, ALU.max)
                def relu1(h, eng):
                    if eng is nc.scalar:
                        nc.scalar.activation(y1[:, h * HW:(h + 1) * HW],
                                             y1p[:, h * HW:(h + 1) * HW],
                                             AF.Relu, bias=b1v[:, 0:1])
                    else:
                        eng.tensor_scalar(y1[:, h * HW:(h + 1) * HW],
                                          y1p[:, h * HW:(h + 1) * HW],
                                          b1v[:, 0:1], 0.0, ALU.add, ALU.max)
                def score(s):
                    t = a * CH + s
                    _mm(nc, scp[:, t:t + 1], y1[:, s * BL:(s + 1) * BL],
                        W2T, start=True, stop=True)

                def p0():
                    nc.gpsimd.tensor_mul(r2[:, 0:HW], gc[:, 0:HW],
                                         qRep[:, 0:HW])
                def p1():
                    nc.gpsimd.tensor_mul(r2[:, HW:W], gc[:, HW:W],
                                         qRep[:, HW:W])
                    _mm(nc, y0p[:, 0:HW], W0T[:, 0:80], gc[:, 0:HW],
                        start=True, stop=False)
                def p2():
                    _mm(nc, y0p[:, HW:W], W0T[:, 0:80], gc[:, HW:W],
                        start=True, stop=False)
                    _mm(nc, y0p[:, 0:HW], W0T[:, 80:160], qRep[:, 0:HW],
                        start=False, stop=False)
                def p3():
                    _mm(nc, y0p[:, HW:W], W0T[:, 80:160], qRep[:, HW:W],
                        start=False, stop=False)
                    _mm(nc, y0p[:, 0:HW], W0T[:, 160:240], r2[:, 0:HW],
                        start=False, stop=True)
                def p4():
                    _mm(nc, y0p[:, HW:W], W0T[:, 160:240], r2[:, HW:W],
                        start=False, stop=True)
                    relu0(0, nc.scalar)
                def p5():
                    relu0(1, nc.vector)
                    _mm(nc, y1p[:, 0:HW], W1T, y0[:, 0:HW],
                        start=True, stop=True)
                def p6():
                    _mm(nc, y1p[:, HW:W], W1T, y0[:, HW:W],
                        start=True, stop=True)
                    relu1(0, nc.scalar)
                def p7():
                    relu1(1, nc.vector)
                    for s in range(4):
                        score(s)
                def p8():
                    for s in range(4, CH):
                        score(s)
                return [p0, p1, p2, p3, p4, p5, p6, p7, p8]

            pending = []          # attention pieces awaiting emission
            WhT_r, WhT_z, WhT_n = (WhhT[:, 0:D], WhhT[:, D:2 * D],
                                   WhhT[:, 2 * D:G3])
            gslot = [None, None]  # (prz, prz4, pn3) per psum buf

            def gru_proj_pieces(c):
                XTc = XT[:, c * WG:(c + 1) * WG]
                prz = gip.tile([D, CG * 128], f32, tag="girz")
                pn = gip.tile([D, CG * BL], f32, tag="gin")
                gslot[c % 2] = (
                    prz,
                    prz.rearrange("p (g s c) -> p g s c", g=2, c=BL),
                    pn.rearrange("p (s g) -> p s g", g=BL),
                )
                return [
                    lambda: _mm(nc, prz[:, 0:WG], WihT[:, 0:D], XTc,
                                start=True, stop=True),
                    lambda: _mm(nc, prz[:, WG:2 * WG], WihT[:, D:2 * D], XTc,
                                start=True, stop=True),
                    lambda: _mm(nc, pn[:, :], WihT[:, 2 * D:G3], XTc,
                                start=True, stop=True),
                ]

            for p in gru_proj_pieces(0):
                p()
            zh_prev = [None]      # z_{t-1} * h_{t-2}
            omzn_prev = [None]    # (1-z_{t-1}) * n_{t-1}
            for c in range(NCG):
                prz, prz4, pn3 = gslot[c % 2]
                for s in range(CG):
                    t = c * CG + s
                    # --- PE block A: recurrent matmuls feeding step t ----
                    hx = hnp.tile([D, 2 * BL], f32, tag="hn2")
                    hn, tmp2 = hx[:, 0:BL], hx[:, BL:2 * BL]
                    if t == 1:
                        h0 = gruT3[:, 0, :]
                        _mm(nc, prz[:, s * BL:(s + 1) * BL], WhT_r, h0,
                            start=False, stop=True, skip=True)
                        _mm(nc, prz[:, WG + s * BL:WG + (s + 1) * BL],
                            WhT_z, h0, start=False, stop=True, skip=True)
                        _mm(nc, hn[:, :], WhT_n, h0, start=True, stop=True)
                    elif t > 1:
                        rsl = prz[:, s * BL:(s + 1) * BL]
                        _mm(nc, rsl, WhT_r, zh_prev[0],
                            start=False, stop=False, skip=True)
                        _mm(nc, rsl, WhT_r, omzn_prev[0],
                            start=False, stop=True, skip=True)
                        hprev = gruT3[:, t - 1, :]
                        _mm(nc, prz[:, WG + s * BL:WG + (s + 1) * BL],
                            WhT_z, hprev, start=False, stop=True, skip=True)
                        _mm(nc, hn[:, :], WhT_n, hprev, start=True, stop=True)
                    # --- gate activations (biases ride ACT) --------------
                    r = gt.tile([D, BL], bf16, tag="r")
                    nc.scalar.activation(r[:, :], prz4[:, 0, s, :], SIG,
                                         bias=brz[:, 0:1])
                    z = gt.tile([D, BL], bf16, tag="z")
                    nc.scalar.activation(z[:, :], prz4[:, 1, s, :], SIG,
                                         bias=brz[:, 1:2])
                    omz = gt.tile([D, BL], bf16, tag="omz")
                    nc.gpsimd.tensor_scalar(omz[:, :], z[:, :], -1.0, 1.0,
                                            ALU.mult, ALU.add)
                    tmp = gt.tile([D, BL], bf16, tag="tmp")
                    if t > 0:
                        nc.vector.scalar_tensor_tensor(
                            tmp[:, :], hn[:, :], b_hn, r[:, :],
                            ALU.add, ALU.mult)
                    else:
                        nc.vector.tensor_scalar_mul(tmp[:, :], r[:, :], b_hn)
                    nc.vector.tensor_add(tmp2[:, :], tmp[:, :], pn3[:, s, :])
                    if t > 0:
                        zh = gt.tile([D, BL], bf16, tag="zh")
                        nc.vector.tensor_mul(zh[:, :], z[:, :],
                                             gruT3[:, t - 1, :])
                    n = gt.tile([D, BL], bf16, tag="n")
                    nc.scalar.activation(n[:, :], tmp2[:, :], TANH, bias=b_in)
                    hdst = gruT3[:, t, :]
                    if t > 0:
                        omzn = gt.tile([D, BL], bf16, tag="omzn")
                        nc.vector.tensor_mul(omzn[:, :], omz[:, :], n[:, :])
                        nc.vector.tensor_add(hdst, omzn[:, :], zh[:, :])
                        zh_prev[0], omzn_prev[0] = zh, omzn
                    else:
                        nc.vector.tensor_mul(hdst, omz[:, :], n[:, :])
                    # --- PE block B: hoisted chunk/attention work --------
                    if s == 1 and c + 1 < NCG:
                        proj_pieces = gru_proj_pieces(c + 1)
                    if s >= 1 and c + 1 < NCG:
                        proj_pieces[s - 1]()
                    if pending:
                        pending.pop(0)()
                    if pending and t % 8 == 2:
                        pending.pop(0)()
                if c % 2 == 1:
                    pending.extend(attention_pieces(c // 2))
            for p in pending:
                p()

            # masked softmax over t (b-major); last MLP layer ReLU'd w/ b2
            rawr = at.tile([BL, T], f32, tag="rawr")
            nc.scalar.activation(rawr[:, :], scp[:, :], AF.Relu,
                                 bias=b2col[:, 0:1])
            rawm = at.tile([BL, T], f32, tag="rawm")
            nc.vector.tensor_mul(rawm[:, :], rawr[:, :], maskS[:, :])
            mxn = at.tile([BL, 1], f32, tag="mxn")
            nc.vector.tensor_reduce(mxn[:, :], rawm[:, :],
                                    axis=mybir.AxisListType.X,
                                    op=ALU.max, negate=True)
            ex = at.tile([BL, T], f32, tag="ex")
            nc.scalar.activation(ex[:, :], rawr[:, :], AF.Exp, bias=mxn[:, 0:1])
            em = at.tile([BL, T], f32, tag="em")
            nc.vector.tensor_mul(em[:, :], ex[:, :], maskS[:, :])
            sm = at.tile([BL, 1], f32, tag="sm")
            nc.vector.tensor_reduce(sm[:, :], em[:, :],
                                    axis=mybir.AxisListType.X, op=ALU.add)
            rs = at.tile([BL, 1], f32, tag="rs")
            nc.vector.reciprocal(rs[:, :], sm[:, :])
            nc.vector.tensor_scalar_mul(wgt[:, :], em[:, :], rs[:, 0:1])

        # attention weights go to partition-0 bf16 rows per AUGRU chunk
        # (the row transposes + broadcasts overlap the AUGRU itself)
        wgt_bf = pp.tile([BL, T], bf16)
        nc.scalar.copy(wgt_bf[:, :], wgt[:, :])

        # ---------------- AUGRU ----------------------------------------
        hA = [None]
        with tc.tile_pool(name="axp", bufs=2, space="PSUM") as axp, \
             tc.tile_pool(name="abp", bufs=2, space="PSUM") as abp, \
             tc.tile_pool(name="rtp", bufs=2, space="PSUM") as rtp, \
             tc.tile_pool(name="ut", bufs=6) as ut:
            aslot = [None, None]  # (pru, pru4, pxh3, pab) per psum buf

            def aug_proj_pieces(c):
                gc = gruT[:, c * WG:(c + 1) * WG]
                rowp = rtp.tile([1, WG], f32, tag="rowp")
                rows = ut.tile([1, WG], bf16, tag="rows")
                pru = axp.tile([D, CG * 128], f32, tag="pxru")
                pxh = axp.tile([D, CG * BL], f32, tag="pxh")
                pab = abp.tile([D, WG], f32, tag="pab")
                aslot[c % 2] = (
                    pru,
                    pru.rearrange("p (g s c) -> p g s c", g=2, c=BL),
                    pxh.rearrange("p (s g) -> p s g", g=BL),
                    pab,
                )

                def pa():
                    for s in range(CG):
                        t = c * CG + s
                        _mm(nc, rowp[0:1, s * BL:(s + 1) * BL],
                            wgt_bf[:, t:t + 1], identB, start=True, stop=True)
                    nc.scalar.copy(rows[0:1, 0:BL * 2], rowp[0:1, 0:BL * 2])
                def pb():
                    nc.scalar.copy(rows[0:1, BL * 2:WG], rowp[0:1, BL * 2:WG])
                    _mm(nc, pru[:, 0:WG], WrxT, gc, start=True, stop=True)
                    _mm(nc, pab[:, 0:BL], ones1, rows[0:1, 0:BL],
                        start=True, stop=True)
                def pc():
                    _mm(nc, pru[:, WG:2 * WG], WuxT, gc, start=True, stop=True)
                    _mm(nc, pab[:, BL:2 * BL], ones1, rows[0:1, BL:2 * BL],
                        start=True, stop=True)
                def pd():
                    _mm(nc, pxh[:, :], WaxT, gc, start=True, stop=True)
                    _mm(nc, pab[:, 2 * BL:WG], ones1, rows[0:1, 2 * BL:WG],
                        start=True, stop=True)
                return [pa, pb, pc, pd]

            for p in aug_proj_pieces(0):
                p()
            hA_pp = [None]        # hA_{t-2}
            ud_prev = [None]      # ud_{t-1}
            for c in range(NCG):
                pru, pru4, pxh3, pab = aslot[c % 2]
                for s in range(CG):
                    t = c * CG + s
                    hAg = hA[0]
                    # --- PE block A: recurrent matmuls feeding step t ----
                    rsl = pru[:, s * BL:(s + 1) * BL]
                    usl = pru[:, WG + s * BL:WG + (s + 1) * BL]
                    if t == 1:
                        _mm(nc, rsl, WrhT, hAg, start=False, stop=True,
                            skip=True)
                        _mm(nc, usl, WuhT, hAg, start=False, stop=True,
                            skip=True)
                    elif t > 1:
                        _mm(nc, rsl, WrhT, hA_pp[0],
                            start=False, stop=False, skip=True)
                        _mm(nc, rsl, WrhT, ud_prev[0],
                            start=False, stop=True, skip=True)
                        _mm(nc, usl, WuhT, hA_pp[0],
                            start=False, stop=False, skip=True)
                        _mm(nc, usl, WuhT, ud_prev[0],
                            start=False, stop=True, skip=True)
                    r = ut.tile([D, BL], bf16, tag="ar")
                    nc.scalar.activation(r[:, :], pru4[:, 0, s, :], SIG,
                                         bias=ab_r)
                    u = ut.tile([D, BL], bf16, tag="au")
                    nc.scalar.activation(u[:, :], pru4[:, 1, s, :], SIG,
                                         bias=ab_u)
                    if t > 0:
                        hr = ut.tile([D, BL], bf16, tag="ahr")
                        nc.vector.tensor_mul(hr[:, :], hAg, r[:, :])
                        _mm(nc, pxh3[:, s, :], WahT, hr,
                            start=False, stop=True, skip=True)
                    up = ut.tile([D, BL], f32, tag="aup")
                    nc.vector.tensor_mul(up[:, :], pab[:, s * BL:(s + 1) * BL],
                                         u[:, :])
                    hh = ut.tile([D, BL], bf16, tag="ahh")
                    nc.scalar.activation(hh[:, :], pxh3[:, s, :], TANH,
                                         bias=ab_h)
                    hnew = hp.tile([D, BL], bf16, tag="hA")
                    if t > 0:
                        dd = ut.tile([D, BL], bf16, tag="add")
                        nc.vector.tensor_sub(dd[:, :], hh[:, :], hAg)
                        ud = ut.tile([D, BL], bf16, tag="aud")
                        nc.vector.tensor_mul(ud[:, :], up[:, :], dd[:, :])
                        nc.gpsimd.tensor_add(hnew[:, :], hAg, ud[:, :])
                        hA_pp[0], ud_prev[0] = hAg, ud
                    else:
                        nc.vector.tensor_mul(hnew[:, :], up[:, :], hh[:, :])
                    hA[0] = hnew
                    # --- PE block B: hoisted next-chunk work -------------
                    if c + 1 < NCG:
                        if s == 0:
                            aproj_pieces = aug_proj_pieces(c + 1)
                        aproj_pieces[s]()

        # ---------------- output layer ---------------------------------
        with tc.tile_pool(name="ops", bufs=1, space="PSUM") as ops, \
             tc.tile_pool(name="ot", bufs=1) as ot:
            ih = ot.tile([D, BL], bf16)
            nc.vector.tensor_mul(ih[:, :], qT[:, :], histF[:, :])
            po = ops.tile([1, BL], f32)
            pieces = [userT, qT, histT, ih, hA[0]]
            for g, piece in enumerate(pieces):
                _mm(nc, po[:, :], outWT[:, g:g + 1], piece,
                    start=(g == 0), stop=(g == 4))
            outs = ot.tile([1, BL], f32)
            nc.scalar.activation(outs[:, :], po[:, :], AF.Identity,
                                 bias=outb[:, 0:1])
            nc.sync.dma_start(outd[:, :], outs[:, :])

    nc.finalize()
    return nc


_NC = None


def _get_nc():
    global _NC
    if _NC is None:
        _NC = build_nc()
    return _NC


def make_in_maps(inputs):
    """Slice full inputs into per-core input maps (host-side layout only)."""
    f = {k: np.asarray(v) for k, v in inputs.items()}
    WihT = np.ascontiguousarray(f["gru_Wih"].T)          # (128, 384)
    WhhT = np.ascontiguousarray(f["gru_Whh"].T)
    bihT = np.ascontiguousarray(f["gru_bih"].reshape(3, D).T)  # (128, 3)
    bhhT = np.ascontiguousarray(f["gru_bhh"].reshape(3, D).T)
    # fold the (q - fact) block into the fact/q blocks:
    # W0_f@f + W0_q@q + W0_fq@(f*q) + W0_qf@(q-f)
    #   = (W0_f - W0_qf)@f + (W0_q + W0_qf)@q + W0_fq@(f*q)
    W0 = f["attn_W0"]                                    # (80, 4D)
    W0f = W0[:, 0:D] - W0[:, 3 * D:4 * D]
    W0q = W0[:, D:2 * D] + W0[:, 3 * D:4 * D]
    W0fq = W0[:, 2 * D:3 * D]
    W0T = np.ascontiguousarray(
        np.concatenate([W0f.T, W0q.T, W0fq.T], axis=1))  # (D, 240)
    b0 = np.ascontiguousarray(f["attn_b0"].reshape(80, 1))
    W1T = np.ascontiguousarray(f["attn_W1"].T)           # (80, 40)
    b1 = np.ascontiguousarray(f["attn_b1"].reshape(40, 1))
    W2T = np.ascontiguousarray(f["attn_W2"].T)           # (40, 1)
    b2 = f["attn_b2"].reshape(1, 1)
    augW = np.concatenate(
        [np.ascontiguousarray(f[k][:, p * D:(p + 1) * D].T)
         for k in ("aug_Wr", "aug_Wu", "aug_Wh") for p in (0, 1)],
        axis=1)                                          # (128, 768)
    augb = np.stack([f["aug_br"], f["aug_bu"], f["aug_bh"]], axis=1)  # (128,3)
    outWT = np.ascontiguousarray(f["out_W"].reshape(5, D).T)          # (128,5)
    outb = f["out_b"].reshape(1, 1)

    shared_bf = dict(WihT=WihT, WhhT=WhhT, W0T=W0T, W1T=W1T, W2T=W2T,
                     augW=augW, outWT=outWT)
    shared = dict(bihT=bihT, bhhT=bhhT, b0=b0, b1=b1, b2=b2, augb=augb,
                  outb=outb)
    shared = {k: np.ascontiguousarray(v.astype(np.float32)) for k, v in
              shared.items()}
    shared.update({k: np.ascontiguousarray(v.astype(BF)) for k, v in
                   shared_bf.items()})

    in_maps = []
    for c in range(NCORES):
        s = slice(c * BL, (c + 1) * BL)
        m = dict(shared)
        hs = f["item_historical_embedding"][s].astype(BF)
        m["hist"] = np.ascontiguousarray(hs)
        m["histTd"] = np.ascontiguousarray(
            hs.transpose(2, 1, 0).reshape(D, T * BL))
        m["item"] = np.ascontiguousarray(f["item_embedding"][s].astype(np.float32))
        m["user"] = np.ascontiguousarray(f["user_embedding"][s].astype(np.float32))
        m["maskd"] = np.ascontiguousarray(f["mask"][s].astype(np.float32))
        m["seqd"] = np.ascontiguousarray(
            f["sequential_length"][s].reshape(BL, 1).astype(np.int32))
        in_maps.append(m)
    return in_maps


def kernel(**inputs) -> np.ndarray:
    nc = _get_nc()
    in_maps = make_in_maps(inputs)
    res = run_bass_kernel_spmd(nc, in_maps, list(range(NCORES)))
    return np.concatenate(
        [np.asarray(res.results[c]["out"]).reshape(BL) for c in range(NCORES)])


# revision 33
# speedup vs baseline: 1.2771x; 1.0019x over previous
"""DIEN (GRU + attention + AUGRU) Trainium2 kernel.

Data-parallel over 8 NeuronCores: each core handles a batch slice of 64.
All on-chip state is feature-major (feature dim on SBUF partitions, batch
on the free dim), so the two sequential recurrences need no per-step
transposes.

Structure:
  - X^T (D, T*64) in bf16, built via PE transposes of natural-layout DMA
    tiles; the same stationary tiles also accumulate the masked history
    sum (f32) via N=1 matmuls against 1/len-scaled mask columns.
  - GRU input projections land in PSUM chunks laid out (128, 4, [r|z|n]).
    Gate sigmoids are split per-gate so the gate bias rides the ACT
    instruction (no DVE bias adds), and (1-z) comes from sigmoid(-x).
    The combine uses h' = (1-z)*n + z*h with z*h computed off the
    critical path.
  - Attention runs per 8-step chunk interleaved with the GRU loop; its
    elementwise prep (fact*q, q-fact) runs on the idle GPSIMD engine so
    the DVE stays free for the recurrence chain.
  - AUGRU consumes attention weights via rows pre-transposed once after
    the softmax; per chunk a single ones-weight load broadcasts 4 rows
    into PSUM. Masked steps have weight 0 (u'=0, h unchanged), so the
    final AUGRU state IS the gathered aug_out[b, len-1] - no gather.
  - Matmuls are bf16 (f32 PSUM accumulation); elementwise recurrence ops
    run bf16 where both operands sit in SBUF (2x DVE mode); the masked
    history mean and softmax stay f32.
"""

import sys

if "/opt/trn_rl_repo" not in sys.path:
    sys.path.insert(0, "/opt/trn_rl_repo")

from contextlib import ExitStack

import ml_dtypes
import numpy as np

import concourse.bacc as bacc
import concourse.bass as bass
import concourse.mybir as mybir
import concourse.tile as tile
from concourse.bass_utils import run_bass_kernel_spmd
from concourse.masks import make_identity

f32 = mybir.dt.float32
bf16 = mybir.dt.bfloat16
i32 = mybir.dt.int32
AF = mybir.ActivationFunctionType
ALU = mybir.AluOpType
BF = ml_dtypes.bfloat16

NCORES = 8
B, T, D = 512, 200, 128
BL = B // NCORES          # 64 batch rows per core
CG = 4                    # GRU/AUGRU psum chunk: steps per chunk
NCG = T // CG             # 50
WG = CG * BL              # 256
CH = 8                    # attention chunk: steps per chunk
NCHUNK = T // CH          # 25
W = CH * BL               # 512
G3 = 3 * D


def _mm(nc, out, lhsT, rhs, start, stop, skip=False):
    nc.tensor.matmul(out, lhsT, rhs, start=start, stop=stop,
                     skip_group_check=skip)


def build_nc():
    nc = bacc.Bacc("TRN2", target_bir_lowering=False)

    hist = nc.declare_dram_parameter("hist", [BL, T, D], bf16, isOutput=False)
    histTd = nc.declare_dram_parameter("histTd", [D, T * BL], bf16,
                                       isOutput=False)
    item = nc.declare_dram_parameter("item", [BL, D], f32, isOutput=False)
    user = nc.declare_dram_parameter("user", [BL, D], f32, isOutput=False)
    maskd = nc.declare_dram_parameter("maskd", [BL, T], f32, isOutput=False)
    seqd = nc.declare_dram_parameter("seqd", [BL, 1], i32, isOutput=False)
    WihTd = nc.declare_dram_parameter("WihT", [D, G3], bf16, isOutput=False)
    WhhTd = nc.declare_dram_parameter("WhhT", [D, G3], bf16, isOutput=False)
    bihTd = nc.declare_dram_parameter("bihT", [D, 3], f32, isOutput=False)
    bhhTd = nc.declare_dram_parameter("bhhT", [D, 3], f32, isOutput=False)
    W0Td = nc.declare_dram_parameter("W0T", [D, 240], bf16, isOutput=False)
    b0d = nc.declare_dram_parameter("b0", [80, 1], f32, isOutput=False)
    W1Td = nc.declare_dram_parameter("W1T", [80, 40], bf16, isOutput=False)
    b1d = nc.declare_dram_parameter("b1", [40, 1], f32, isOutput=False)
    W2Td = nc.declare_dram_parameter("W2T", [40, 1], bf16, isOutput=False)
    b2d = nc.declare_dram_parameter("b2", [1, 1], f32, isOutput=False)
    augWd = nc.declare_dram_parameter("augW", [D, 6 * D], bf16, isOutput=False)
    augbd = nc.declare_dram_parameter("augb", [D, 3], f32, isOutput=False)
    outWTd = nc.declare_dram_parameter("outWT", [D, 5], bf16, isOutput=False)
    outbd = nc.declare_dram_parameter("outb", [1, 1], f32, isOutput=False)
    outd = nc.declare_dram_parameter("out", [1, BL], f32, isOutput=True)

    with tile.TileContext(nc) as tc, ExitStack() as ctx:
        big = ctx.enter_context(tc.tile_pool(name="big", bufs=1))
        wp = ctx.enter_context(tc.tile_pool(name="wp", bufs=1))
        pp = ctx.enter_context(tc.tile_pool(name="pp", bufs=1))
        hp = ctx.enter_context(tc.tile_pool(name="hp", bufs=4))

        XT = big.tile([D, T * BL], bf16)
        gruT = big.tile([D, T * BL], bf16)

        ident = pp.tile([128, 128], f32)
        make_identity(nc, ident)
        identB = pp.tile([BL, BL], bf16)
        make_identity(nc, identB)
        identB128 = pp.tile([128, 128], bf16)
        make_identity(nc, identB128)

        # ------------- weights (bf16 arrive pre-converted via DMA) -------
        WihT = wp.tile([D, G3], bf16)
        nc.sync.dma_start(WihT[:, :], WihTd[:, :])
        WhhT = wp.tile([D, G3], bf16)
        nc.sync.dma_start(WhhT[:, :], WhhTd[:, :])
        W0T = wp.tile([D, 240], bf16)
        nc.sync.dma_start(W0T[:, :], W0Td[:, :])
        W1T = wp.tile([80, 40], bf16)
        nc.sync.dma_start(W1T[:, :], W1Td[:, :])
        W2T = wp.tile([40, 1], bf16)
        nc.sync.dma_start(W2T[:, :], W2Td[:, :])
        augW = wp.tile([D, 6 * D], bf16)
        nc.sync.dma_start(augW[:, :], augWd[:, :])
        outWT = wp.tile([D, 5], bf16)
        nc.sync.dma_start(outWT[:, :], outWTd[:, :])
        WrhT, WrxT = augW[:, 0:D], augW[:, D:2 * D]
        WuhT, WuxT = augW[:, 2 * D:3 * D], augW[:, 3 * D:4 * D]
        WahT, WaxT = augW[:, 4 * D:5 * D], augW[:, 5 * D:6 * D]

        bihT = wp.tile([D, 3], f32)
        nc.sync.dma_start(bihT[:, :], bihTd[:, :])
        bhhT = wp.tile([D, 3], f32)
        nc.sync.dma_start(bhhT[:, :], bhhTd[:, :])
        brz = wp.tile([D, 2], f32)
        nc.vector.tensor_add(brz[:, :], bihT[:, 0:2], bhhT[:, 0:2])
        nbz = wp.tile([D, 1], f32)
        nc.vector.tensor_scalar(nbz[:, :], brz[:, 1:2], -1.0, 0.0,
                                ALU.mult, ALU.add)
        b_hn, b_in = bhhT[:, 2:3], bihT[:, 2:3]

        b0v = wp.tile([80, 1], f32)
        nc.sync.dma_start(b0v[:, :], b0d[:, :])
        b1v = wp.tile([40, 1], f32)
        nc.sync.dma_start(b1v[:, :], b1d[:, :])
        b2v = wp.tile([1, 1], f32)
        nc.sync.dma_start(b2v[:, :], b2d[:, :])
        augb = wp.tile([D, 3], f32)
        nc.sync.dma_start(augb[:, :], augbd[:, :])
        outb = wp.tile([1, 1], f32)
        nc.sync.dma_start(outb[:, :], outbd[:, :])
        ab_r, ab_u, ab_h = augb[:, 0:1], augb[:, 1:2], augb[:, 2:3]

        ones_f = wp.tile([1, 128], f32)
        nc.vector.memset(ones_f[:, :], 1.0)
        ones1 = wp.tile([1, 128], bf16)
        nc.vector.memset(ones1[:, :], 1.0)

        # ---------------- preprocessing --------------------------------
        histT = pp.tile([D, BL], bf16)
        qT = pp.tile([D, BL], bf16)
        userT = pp.tile([D, BL], bf16)
        qRep = pp.tile([D, W], bf16)
        maskS = pp.tile([BL, T], f32)
        b2col = pp.tile([BL, 1], f32)
        histF = pp.tile([D, BL], f32)

        with tc.tile_pool(name="pre", bufs=8) as pre, \
             tc.tile_pool(name="pps", bufs=2, space="PSUM") as pps, \
             tc.tile_pool(name="hps", bufs=1, space="PSUM") as hps:
            # X^T arrives pre-transposed from the host; split the DMA so the
            # first GRU chunks can start as soon as their columns land.
            nc.sync.dma_start(XT[:, 0:4 * WG], histTd[:, 0:4 * WG])
            nc.sync.dma_start(XT[:, 4 * WG:25 * WG], histTd[:, 4 * WG:25 * WG])
            nc.sync.dma_start(XT[:, 25 * WG:T * BL], histTd[:, 25 * WG:T * BL])

            nc.sync.dma_start(maskS[:, :], maskd[:, :])
            seqi = pre.tile([BL, 1], i32, tag="seqi", bufs=1)
            nc.sync.dma_start(seqi[:, :], seqd[:, :])
            seqf = pre.tile([BL, 1], f32, tag="seqf", bufs=1)
            nc.vector.tensor_copy(seqf[:, :], seqi[:, :])
            rsec = pre.tile([BL, 1], f32, tag="rsec", bufs=1)
            nc.vector.reciprocal(rsec[:, :], seqf[:, :])
            maskSc = pre.tile([BL, T], f32, tag="maskSc", bufs=1)
            nc.vector.tensor_scalar_mul(maskSc[:, :], maskS[:, :], rsec[:, 0:1])

            mstp0 = pps.tile([128, BL], f32, tag="mstp")
            nc.tensor.transpose(mstp0[:, :], maskSc[:, 0:128], ident[0:BL, 0:BL])
            mst0 = pre.tile([128, BL], bf16, tag="mst0", bufs=1)
            nc.scalar.copy(mst0[:, :], mstp0[:, :])
            mstp1 = pps.tile([128, BL], f32, tag="mstp")
            nc.tensor.transpose(mstp1[0:72, :], maskSc[:, 128:200], ident[0:BL, 0:BL])
            mst1 = pre.tile([72, BL], bf16, tag="mst1", bufs=1)
            nc.scalar.copy(mst1[:, :], mstp1[0:72, :])

            itn = pre.tile([BL, D], f32, tag="itn", bufs=2)
            nc.sync.dma_start(itn[:, :], item[:, :])
            itp = pps.tile([D, BL], f32, tag="small_t")
            nc.tensor.transpose(itp[:, :], itn[:, :], ident[0:BL, 0:BL])
            nc.scalar.copy(qT[:, :], itp[:, :])
            usn = pre.tile([BL, D], f32, tag="itn", bufs=2)
            nc.sync.dma_start(usn[:, :], user[:, :])
            usp = pps.tile([D, BL], f32, tag="small_t")
            nc.tensor.transpose(usp[:, :], usn[:, :], ident[0:BL, 0:BL])
            nc.scalar.copy(userT[:, :], usp[:, :])
            b2p = pps.tile([BL, 1], f32, tag="small_t")
            _mm(nc, b2p[:, :], ones_f[0:1, 0:BL], b2v, start=True, stop=True)
            nc.scalar.copy(b2col[:, :], b2p[:, :])
            for s in range(CH):
                nc.scalar.copy(qRep[:, s * BL:(s + 1) * BL], qT[:, :])

            # masked history sum from natural-layout tiles; batched DMAs
            # (8 batch rows per dma_start) while the big X^T DMA streams in.
            histp0 = hps.tile([D, BL], f32, tag="histp0")
            histp1 = hps.tile([D, BL], f32, tag="histp1")
            GB = 8
            for h, (t0, tn) in enumerate(((0, 128), (128, 72))):
                histp = histp0 if h == 0 else histp1
                for b0 in range(0, BL, GB):
                    xn = pre.tile([128, GB * D], bf16, tag="xn", bufs=3)
                    xn3 = xn.rearrange("t (b d) -> t b d", d=D)
                    nc.sync.dma_start(
                        xn3[0:tn, :, :],
                        hist[b0:b0 + GB, t0:t0 + tn, :]
                        .rearrange("b t d -> t b d"))
                    for bi in range(GB):
                        b = b0 + bi
                        mcol = (mst0 if h == 0 else mst1)[0:tn, b:b + 1]
                        _mm(nc, histp[:, b:b + 1],
                            xn3[0:tn, bi, :], mcol, start=True, stop=True)
            nc.scalar.copy(histF[:, :], histp0[:, :])
            nc.vector.tensor_add(histF[:, :], histF[:, :], histp1[:, :])
            nc.scalar.copy(histT[:, :], histF[:, :])

        # ---------------- GRU + interleaved attention -------------------
        gruT3 = gruT.rearrange("p (t b) -> p t b", b=BL)
        SIG, TANH = AF.Sigmoid, AF.Tanh
        wgt = pp.tile([BL, T], f32)

        with tc.tile_pool(name="gip", bufs=2, space="PSUM") as gip, \
             tc.tile_pool(name="hnp", bufs=1, space="PSUM") as hnp, \
             tc.tile_pool(name="gt", bufs=6) as gt, \
             tc.tile_pool(name="aps", bufs=1, space="PSUM") as aps, \
             tc.tile_pool(name="sps", bufs=1, space="PSUM") as sps, \
             tc.tile_pool(name="at", bufs=3) as at:
            scp = sps.tile([BL, T], f32)

            def attention_pieces(a):
                """Attention for gru chunks [2a, 2a+1] as 8 emission slots;
                each slot is small PE/DVE/Pool work sized to fill one GRU
                step's engine-idle window without head-of-line-blocking the
                recurrence chain's FIFO queues."""
                gc = gruT[:, a * W:(a + 1) * W]
                r2 = at.tile([D, W], bf16, tag="r2")
                y0p = aps.tile([80, W], f32, tag="y0")
                y0 = at.tile([80, W], bf16, tag="y0s")
                y1p = aps.tile([40, W], f32, tag="y1")
                y1 = at.tile([40, W], bf16, tag="y1s")
                HW = 256

                def relu0(h, eng):
                    if eng is nc.scalar:
                        nc.scalar.activation(y0[:, h * HW:(h + 1) * HW],
                                             y0p[:, h * HW:(h + 1) * HW],
                                             AF.Relu, bias=b0v[:, 0:1])
                    else:
                        eng.tensor_scalar(y0[:, h * HW:(h + 1) * HW],
                                          y0p[:, h * HW:(h + 1) * HW],
                                          b0v[:, 0:1], 0.0, ALU.add, ALU.max)
                def relu1(h, eng):
                    if eng is nc.scalar:
                        nc.scalar.activation(y1[:, h * HW:(h + 1) * HW],
                                             y1p[:, h * HW:(h + 1) * HW],
                                             AF.Relu, bias=b1v[:, 0:1])
                    else:
                        eng.tensor_scalar(y1[:, h * HW:(h + 1) * HW],
                                          y1p[:, h * HW:(h + 1) * HW],
                                          b1v[:, 0:1], 0.0, ALU.add, ALU.max)
                def score(s):
                    t = a * CH + s
                    _mm(nc, scp[:, t:t + 1], y1[:, s * BL:(s + 1) * BL],
                        W2T, start=True, stop=True)

                def p0():
                    nc.gpsimd.tensor_mul(r2[:, 0:HW], gc[:, 0:HW],
                                         qRep[:, 0:HW])
                def p1():
                    nc.gpsimd.tensor_mul(r2[:, HW:W], gc[:, HW:W],
                                         qRep[:, HW:W])
                    _mm(nc, y0p[:, 0:HW], W0T[:, 0:80], gc[:, 0:HW],
                        start=True, stop=False)
                def p2():
                    _mm(nc, y0p[:, HW:W], W0T[:, 0:80], gc[:, HW:W],
                        start=True, stop=False)
                    _mm(nc, y0p[:, 0:HW], W0T[:, 80:160], qRep[:, 0:HW],
                        start=False, stop=False)
                def p3():
                    _mm(nc, y0p[:, HW:W], W0T[:, 80:160], qRep[:, HW:W],
                        start=False, stop=False)
                    _mm(nc, y0p[:, 0:HW], W0T[:, 160:240], r2[:, 0:HW],
                        start=False, stop=True)
                def p4():
                    _mm(nc, y0p[:, HW:W], W0T[:, 160:240], r2[:, HW:W],
                        start=False, stop=True)
                    relu0(0, nc.scalar)
                def p5():
                    relu0(1, nc.vector)
                    _mm(nc, y1p[:, 0:HW], W1T, y0[:, 0:HW],
                        start=True, stop=True)
                def p6():
                    _mm(nc, y1p[:, HW:W], W1T, y0[:, HW:W],
                        start=True, stop=True)
                    relu1(0, nc.scalar)
                def p7():
                    relu1(1, nc.vector)
                    for s in range(4):
                        score(s)
                def p8():
                    for s in range(4, CH):
                        score(s)
                return [p0, p1, p2, p3, p4, p5, p6, p7, p8]

            pending = []          # attention pieces awaiting emission
            WhT_r, WhT_z, WhT_n = (WhhT[:, 0:D], WhhT[:, D:2 * D],
                                   WhhT[:, 2 * D:G3])
            gslot = [None, None]  # (prz, prz4, pn3) per psum buf

            def gru_proj_pieces(c):
                XTc = XT[:, c * WG:(c + 1) * WG]
                prz = gip.tile([D, CG * 128], f32, tag="girz")
                pn = gip.tile([D, CG * BL], f32, tag="gin")
                gslot[c % 2] = (
                    prz,
                    prz.rearrange("p (g s c) -> p g s c", g=2, c=BL),
                    pn.rearrange("p (s g) -> p s g", g=BL),
                )
                return [
                    lambda: _mm(nc, prz[:, 0:WG], WihT[:, 0:D], XTc,
                                start=True, stop=True),
                    lambda: _mm(nc, prz[:, WG:2 * WG], WihT[:, D:2 * D], XTc,
                                start=True, stop=True),
                    lambda: _mm(nc, pn[:, :], WihT[:, 2 * D:G3], XTc,
                                start=True, stop=True),
                ]

            for p in gru_proj_pieces(0):
                p()
            zh_prev = [None]      # z_{t-1} * h_{t-2}
            omzn_prev = [None]    # (1-z_{t-1}) * n_{t-1}
            for c in range(NCG):
                prz, prz4, pn3 = gslot[c % 2]
                for s in range(CG):
                    t = c * CG + s
                    # --- PE block A: recurrent matmuls feeding step t ----
                    hx = hnp.tile([D, 2 * BL], f32, tag="hn2")
                    hn, tmp2 = hx[:, 0:BL], hx[:, BL:2 * BL]
                    if t == 1:
                        h0 = gruT3[:, 0, :]
                        _mm(nc, prz[:, s * BL:(s + 1) * BL], WhT_r, h0,
                            start=False, stop=True, skip=True)
                        _mm(nc, prz[:, WG + s * BL:WG + (s + 1) * BL],
                            WhT_z, h0, start=False, stop=True, skip=True)
                        _mm(nc, hn[:, :], WhT_n, h0, start=True, stop=True)
                    elif t > 1:
                        rsl = prz[:, s * BL:(s + 1) * BL]
                        _mm(nc, rsl, WhT_r, zh_prev[0],
                            start=False, stop=False, skip=True)
                        _mm(nc, rsl, WhT_r, omzn_prev[0],
                            start=False, stop=True, skip=True)
                        hprev = gruT3[:, t - 1, :]
                        _mm(nc, prz[:, WG + s * BL:WG + (s + 1) * BL],
                            WhT_z, hprev, start=False, stop=True, skip=True)
                        _mm(nc, hn[:, :], WhT_n, hprev, start=True, stop=True)
                    # --- gate activations (biases ride ACT) --------------
                    r = gt.tile([D, BL], bf16, tag="r")
                    nc.scalar.activation(r[:, :], prz4[:, 0, s, :], SIG,
                                         bias=brz[:, 0:1])
                    z = gt.tile([D, BL], bf16, tag="z")
                    nc.scalar.activation(z[:, :], prz4[:, 1, s, :], SIG,
                                         bias=brz[:, 1:2])
                    omz = gt.tile([D, BL], bf16, tag="omz")
                    nc.gpsimd.tensor_scalar(omz[:, :], z[:, :], -1.0, 1.0,
                                            ALU.mult, ALU.add)
                    tmp = gt.tile([D, BL], bf16, tag="tmp")
                    if t > 0:
                        nc.vector.scalar_tensor_tensor(
                            tmp[:, :], hn[:, :], b_hn, r[:, :],
                            ALU.add, ALU.mult)
                    else:
                        nc.vector.tensor_scalar_mul(tmp[:, :], r[:, :], b_hn)
                    nc.vector.tensor_add(tmp2[:, :], tmp[:, :], pn3[:, s, :])
                    if t > 0:
                        zh = gt.tile([D, BL], bf16, tag="zh")
                        nc.vector.tensor_mul(zh[:, :], z[:, :],
                                             gruT3[:, t - 1, :])
                    n = gt.tile([D, BL], bf16, tag="n")
                    nc.scalar.activation(n[:, :], tmp2[:, :], TANH, bias=b_in)
                    hdst = gruT3[:, t, :]
                    if t > 0:
                        omzn = gt.tile([D, BL], bf16, tag="omzn")
                        nc.vector.tensor_mul(omzn[:, :], omz[:, :], n[:, :])
                        nc.vector.tensor_add(hdst, omzn[:, :], zh[:, :])
                        zh_prev[0], omzn_prev[0] = zh, omzn
                    else:
                        nc.vector.tensor_mul(hdst, omz[:, :], n[:, :])
                    # --- PE block B: hoisted chunk/attention work --------
                    if s == 1 and c + 1 < NCG:
                        proj_pieces = gru_proj_pieces(c + 1)
                    if s >= 1 and c + 1 < NCG:
                        proj_pieces[s - 1]()
                    if pending and t % 8 != 7:
                        pending.pop(0)()
                    if pending and t % 8 in (1, 2):
                        pending.pop(0)()
                if c % 2 == 1:
                    pending.extend(attention_pieces(c // 2))
            for p in pending:
                p()

            # masked softmax over t (b-major); last MLP layer ReLU'd w/ b2
            rawr = at.tile([BL, T], f32, tag="rawr")
            nc.scalar.activation(rawr[:, :], scp[:, :], AF.Relu,
                                 bias=b2col[:, 0:1])
            rawm = at.tile([BL, T], f32, tag="rawm")
            nc.vector.tensor_mul(rawm[:, :], rawr[:, :], maskS[:, :])
            mxn = at.tile([BL, 1], f32, tag="mxn")
            nc.vector.tensor_reduce(mxn[:, :], rawm[:, :],
                                    axis=mybir.AxisListType.X,
                                    op=ALU.max, negate=True)
            ex = at.tile([BL, T], f32, tag="ex")
            nc.scalar.activation(ex[:, :], rawr[:, :], AF.Exp, bias=mxn[:, 0:1])
            em = at.tile([BL, T], f32, tag="em")
            nc.vector.tensor_mul(em[:, :], ex[:, :], maskS[:, :])
            sm = at.tile([BL, 1], f32, tag="sm")
            nc.vector.tensor_reduce(sm[:, :], em[:, :],
                                    axis=mybir.AxisListType.X, op=ALU.add)
            rs = at.tile([BL, 1], f32, tag="rs")
            nc.vector.reciprocal(rs[:, :], sm[:, :])
            nc.vector.tensor_scalar_mul(wgt[:, :], em[:, :], rs[:, 0:1])

        # attention weights go to partition-0 bf16 rows per AUGRU chunk
        # (the row transposes + broadcasts overlap the AUGRU itself)
        wgt_bf = pp.tile([BL, T], bf16)
        nc.scalar.copy(wgt_bf[:, :], wgt[:, :])

        # ---------------- AUGRU ----------------------------------------
        hA = [None]
        with tc.tile_pool(name="axp", bufs=2, space="PSUM") as axp, \
             tc.tile_pool(name="abp", bufs=2, space="PSUM") as abp, \
             tc.tile_pool(name="rtp", bufs=2, space="PSUM") as rtp, \
             tc.tile_pool(name="ut", bufs=6) as ut:
            aslot = [None, None]  # (pru, pru4, pxh3, pab) per psum buf

            def aug_proj_pieces(c):
                gc = gruT[:, c * WG:(c + 1) * WG]
                rowp = rtp.tile([1, WG], f32, tag="rowp")
                rows = ut.tile([1, WG], bf16, tag="rows")
                pru = axp.tile([D, CG * 128], f32, tag="pxru")
                pxh = axp.tile([D, CG * BL], f32, tag="pxh")
                pab = abp.tile([D, WG], f32, tag="pab")
                aslot[c % 2] = (
                    pru,
                    pru.rearrange("p (g s c) -> p g s c", g=2, c=BL),
                    pxh.rearrange("p (s g) -> p s g", g=BL),
                    pab,
                )

                def pa():
                    for s in range(CG):
                        t = c * CG + s
                        _mm(nc, rowp[0:1, s * BL:(s + 1) * BL],
                            wgt_bf[:, t:t + 1], identB, start=True, stop=True)
                    nc.scalar.copy(rows[0:1, 0:BL * 2], rowp[0:1, 0:BL * 2])
                def pb():
                    nc.scalar.copy(rows[0:1, BL * 2:WG], rowp[0:1, BL * 2:WG])
                    _mm(nc, pru[:, 0:WG], WrxT, gc, start=True, stop=True)
                    _mm(nc, pab[:, 0:BL], ones1, rows[0:1, 0:BL],
                        start=True, stop=True)
                def pc():
                    _mm(nc, pru[:, WG:2 * WG], WuxT, gc, start=True, stop=True)
                    _mm(nc, pab[:, BL:2 * BL], ones1, rows[0:1, BL:2 * BL],
                        start=True, stop=True)
                def pd():
                    _mm(nc, pxh[:, :], WaxT, gc, start=True, stop=True)
                    _mm(nc, pab[:, 2 * BL:WG], ones1, rows[0:1, 2 * BL:WG],
                        start=True, stop=True)
                return [pa, pb, pc, pd]

            for p in aug_proj_pieces(0):
                p()
            hA_pp = [None]        # hA_{t-2}
            ud_prev = [None]      # ud_{t-1}
            for c in range(NCG):
                pru, pru4, pxh3, pab = aslot[c % 2]
                for s in range(CG):
                    t = c * CG + s
                    hAg = hA[0]
                    # --- PE block A: recurrent matmuls feeding step t ----
                    rsl = pru[:, s * BL:(s + 1) * BL]
                    usl = pru[:, WG + s * BL:WG + (s + 1) * BL]
                    if t == 1:
                        _mm(nc, rsl, WrhT, hAg, start=False, stop=True,
                            skip=True)
                        _mm(nc, usl, WuhT, hAg, start=False, stop=True,
                            skip=True)
                    elif t > 1:
                        _mm(nc, rsl, WrhT, hA_pp[0],
                            start=False, stop=False, skip=True)
                        _mm(nc, rsl, WrhT, ud_prev[0],
                            start=False, stop=True, skip=True)
                        _mm(nc, usl, WuhT, hA_pp[0],
                            start=False, stop=False, skip=True)
                        _mm(nc, usl, WuhT, ud_prev[0],
                            start=False, stop=True, skip=True)
                    r = ut.tile([D, BL], bf16, tag="ar")
                    nc.scalar.activation(r[:, :], pru4[:, 0, s, :], SIG,
                                         bias=ab_r)
                    u = ut.tile([D, BL], bf16, tag="au")
                    nc.scalar.activation(u[:, :], pru4[:, 1, s, :], SIG,
                                         bias=ab_u)
                    if t > 0:
                        hr = ut.tile([D, BL], bf16, tag="ahr")
                        nc.vector.tensor_mul(hr[:, :], hAg, r[:, :])
                        _mm(nc, pxh3[:, s, :], WahT, hr,
                            start=False, stop=True, skip=True)
                    up = ut.tile([D, BL], f32, tag="aup")
                    nc.vector.tensor_mul(up[:, :], pab[:, s * BL:(s + 1) * BL],
                                         u[:, :])
                    hh = ut.tile([D, BL], bf16, tag="ahh")
                    nc.scalar.activation(hh[:, :], pxh3[:, s, :], TANH,
                                         bias=ab_h)
                    hnew = hp.tile([D, BL], bf16, tag="hA")
                    if t > 0:
                        dd = ut.tile([D, BL], bf16, tag="add")
                        nc.vector.tensor_sub(dd[:, :], hh[:, :], hAg)
                        ud = ut.tile([D, BL], bf16, tag="aud")
                        nc.vector.tensor_mul(ud[:, :], up[:, :], dd[:, :])
                        nc.gpsimd.tensor_add(hnew[:, :], hAg, ud[:, :])
                        hA_pp[0], ud_prev[0] = hAg, ud
                    else:
                        nc.vector.tensor_mul(hnew[:, :], up[:, :], hh[:, :])
                    hA[0] = hnew
                    # --- PE block B: hoisted next-chunk work -------------
                    if c + 1 < NCG:
                        if s == 0:
                            aproj_pieces = aug_proj_pieces(c + 1)
                        aproj_pieces[s]()

        # ---------------- output layer ---------------------------------
        with tc.tile_pool(name="ops", bufs=1, space="PSUM") as ops, \
             tc.tile_pool(name="ot", bufs=1) as ot:
            ih = ot.tile([D, BL], bf16)
            nc.vector.tensor_mul(ih[:, :], qT[:, :], histF[:, :])
            po = ops.tile([1, BL], f32)
            pieces = [userT, qT, histT, ih, hA[0]]
            for g, piece in enumerate(pieces):
                _mm(nc, po[:, :], outWT[:, g:g + 1], piece,
                    start=(g == 0), stop=(g == 4))
            outs = ot.tile([1, BL], f32)
            nc.scalar.activation(outs[:, :], po[:, :], AF.Identity,
                                 bias=outb[:, 0:1])
            nc.sync.dma_start(outd[:, :], outs[:, :])

    nc.finalize()
    return nc


_NC = None


def _get_nc():
    global _NC
    if _NC is None:
        _NC = build_nc()
    return _NC


def make_in_maps(inputs):
    """Slice full inputs into per-core input maps (host-side layout only)."""
    f = {k: np.asarray(v) for k, v in inputs.items()}
    WihT = np.ascontiguousarray(f["gru_Wih"].T)          # (128, 384)
    WhhT = np.ascontiguousarray(f["gru_Whh"].T)
    bihT = np.ascontiguousarray(f["gru_bih"].reshape(3, D).T)  # (128, 3)
    bhhT = np.ascontiguousarray(f["gru_bhh"].reshape(3, D).T)
    # fold the (q - fact) block into the fact/q blocks:
    # W0_f@f + W0_q@q + W0_fq@(f*q) + W0_qf@(q-f)
    #   = (W0_f - W0_qf)@f + (W0_q + W0_qf)@q + W0_fq@(f*q)
    W0 = f["attn_W0"]                                    # (80, 4D)
    W0f = W0[:, 0:D] - W0[:, 3 * D:4 * D]
    W0q = W0[:, D:2 * D] + W0[:, 3 * D:4 * D]
    W0fq = W0[:, 2 * D:3 * D]
    W0T = np.ascontiguousarray(
        np.concatenate([W0f.T, W0q.T, W0fq.T], axis=1))  # (D, 240)
    b0 = np.ascontiguousarray(f["attn_b0"].reshape(80, 1))
    W1T = np.ascontiguousarray(f["attn_W1"].T)           # (80, 40)
    b1 = np.ascontiguousarray(f["attn_b1"].reshape(40, 1))
    W2T = np.ascontiguousarray(f["attn_W2"].T)           # (40, 1)
    b2 = f["attn_b2"].reshape(1, 1)
    augW = np.concatenate(
        [np.ascontiguousarray(f[k][:, p * D:(p + 1) * D].T)
         for k in ("aug_Wr", "aug_Wu", "aug_Wh") for p in (0, 1)],
        axis=1)                                          # (128, 768)
    augb = np.stack([f["aug_br"], f["aug_bu"], f["aug_bh"]], axis=1)  # (128,3)
    outWT = np.ascontiguousarray(f["out_W"].reshape(5, D).T)          # (128,5)
    outb = f["out_b"].reshape(1, 1)

    shared_bf = dict(WihT=WihT, WhhT=WhhT, W0T=W0T, W1T=W1T, W2T=W2T,
                     augW=augW, outWT=outWT)
    shared = dict(bihT=bihT, bhhT=bhhT, b0=b0, b1=b1, b2=b2, augb=augb,
                  outb=outb)
    shared = {k: np.ascontiguousarray(v.astype(np.float32)) for k, v in
              shared.items()}
    shared.update({k: np.ascontiguousarray(v.astype(BF)) for k, v in
                   shared_bf.items()})

    in_maps = []
    for c in range(NCORES):
        s = slice(c * BL, (c + 1) * BL)
        m = dict(shared)
        hs = f["item_historical_embedding"][s].astype(BF)
        m["hist"] = np.ascontiguousarray(hs)
        m["histTd"] = np.ascontiguousarray(
            hs.transpose(2, 1, 0).reshape(D, T * BL))
        m["item"] = np.ascontiguousarray(f["item_embedding"][s].astype(np.float32))
        m["user"] = np.ascontiguousarray(f["user_embedding"][s].astype(np.float32))
        m["maskd"] = np.ascontiguousarray(f["mask"][s].astype(np.float32))
        m["seqd"] = np.ascontiguousarray(
            f["sequential_length"][s].reshape(BL, 1).astype(np.int32))
        in_maps.append(m)
    return in_maps


def kernel(**inputs) -> np.ndarray:
    nc = _get_nc()
    in_maps = make_in_maps(inputs)
    res = run_bass_kernel_spmd(nc, in_maps, list(range(NCORES)))
    return np.concatenate(
        [np.asarray(res.results[c]["out"]).reshape(BL) for c in range(NCORES)])


# revision 34
# speedup vs baseline: 1.2831x; 1.0046x over previous
"""DIEN (GRU + attention + AUGRU) Trainium2 kernel.

Data-parallel over 8 NeuronCores: each core handles a batch slice of 64.
All on-chip state is feature-major (feature dim on SBUF partitions, batch
on the free dim), so the two sequential recurrences need no per-step
transposes.

Structure:
  - X^T (D, T*64) in bf16, built via PE transposes of natural-layout DMA
    tiles; the same stationary tiles also accumulate the masked history
    sum (f32) via N=1 matmuls against 1/len-scaled mask columns.
  - GRU input projections land in PSUM chunks laid out (128, 4, [r|z|n]).
    Gate sigmoids are split per-gate so the gate bias rides the ACT
    instruction (no DVE bias adds), and (1-z) comes from sigmoid(-x).
    The combine uses h' = (1-z)*n + z*h with z*h computed off the
    critical path.
  - Attention runs per 8-step chunk interleaved with the GRU loop; its
    elementwise prep (fact*q, q-fact) runs on the idle GPSIMD engine so
    the DVE stays free for the recurrence chain.
  - AUGRU consumes attention weights via rows pre-transposed once after
    the softmax; per chunk a single ones-weight load broadcasts 4 rows
    into PSUM. Masked steps have weight 0 (u'=0, h unchanged), so the
    final AUGRU state IS the gathered aug_out[b, len-1] - no gather.
  - Matmuls are bf16 (f32 PSUM accumulation); elementwise recurrence ops
    run bf16 where both operands sit in SBUF (2x DVE mode); the masked
    history mean and softmax stay f32.
"""

import sys

if "/opt/trn_rl_repo" not in sys.path:
    sys.path.insert(0, "/opt/trn_rl_repo")

from contextlib import ExitStack

import ml_dtypes
import numpy as np

import concourse.bacc as bacc
import concourse.bass as bass
import concourse.mybir as mybir
import concourse.tile as tile
from concourse.bass_utils import run_bass_kernel_spmd
from concourse.masks import make_identity

f32 = mybir.dt.float32
bf16 = mybir.dt.bfloat16
i32 = mybir.dt.int32
AF = mybir.ActivationFunctionType
ALU = mybir.AluOpType
BF = ml_dtypes.bfloat16

NCORES = 8
B, T, D = 512, 200, 128
BL = B // NCORES          # 64 batch rows per core
CG = 4                    # GRU/AUGRU psum chunk: steps per chunk
NCG = T // CG             # 50
WG = CG * BL              # 256
CH = 8                    # attention chunk: steps per chunk
NCHUNK = T // CH          # 25
W = CH * BL               # 512
G3 = 3 * D


def _mm(nc, out, lhsT, rhs, start, stop, skip=False):
    nc.tensor.matmul(out, lhsT, rhs, start=start, stop=stop,
                     skip_group_check=skip)


def build_nc():
    nc = bacc.Bacc("TRN2", target_bir_lowering=False)

    hist = nc.declare_dram_parameter("hist", [BL, T, D], bf16, isOutput=False)
    histTd = nc.declare_dram_parameter("histTd", [D, T * BL], bf16,
                                       isOutput=False)
    item = nc.declare_dram_parameter("item", [BL, D], f32, isOutput=False)
    user = nc.declare_dram_parameter("user", [BL, D], f32, isOutput=False)
    maskd = nc.declare_dram_parameter("maskd", [BL, T], f32, isOutput=False)
    seqd = nc.declare_dram_parameter("seqd", [BL, 1], i32, isOutput=False)
    WihTd = nc.declare_dram_parameter("WihT", [D, G3], bf16, isOutput=False)
    WhhTd = nc.declare_dram_parameter("WhhT", [D, G3], bf16, isOutput=False)
    bihTd = nc.declare_dram_parameter("bihT", [D, 3], f32, isOutput=False)
    bhhTd = nc.declare_dram_parameter("bhhT", [D, 3], f32, isOutput=False)
    W0Td = nc.declare_dram_parameter("W0T", [D, 240], bf16, isOutput=False)
    b0d = nc.declare_dram_parameter("b0", [80, 1], f32, isOutput=False)
    W1Td = nc.declare_dram_parameter("W1T", [80, 40], bf16, isOutput=False)
    b1d = nc.declare_dram_parameter("b1", [40, 1], f32, isOutput=False)
    W2Td = nc.declare_dram_parameter("W2T", [40, 1], bf16, isOutput=False)
    b2d = nc.declare_dram_parameter("b2", [1, 1], f32, isOutput=False)
    augWd = nc.declare_dram_parameter("augW", [D, 6 * D], bf16, isOutput=False)
    augbd = nc.declare_dram_parameter("augb", [D, 3], f32, isOutput=False)
    outWTd = nc.declare_dram_parameter("outWT", [D, 5], bf16, isOutput=False)
    outbd = nc.declare_dram_parameter("outb", [1, 1], f32, isOutput=False)
    outd = nc.declare_dram_parameter("out", [1, BL], f32, isOutput=True)

    with tile.TileContext(nc) as tc, ExitStack() as ctx:
        big = ctx.enter_context(tc.tile_pool(name="big", bufs=1))
        wp = ctx.enter_context(tc.tile_pool(name="wp", bufs=1))
        pp = ctx.enter_context(tc.tile_pool(name="pp", bufs=1))
        hp = ctx.enter_context(tc.tile_pool(name="hp", bufs=4))

        XT = big.tile([D, T * BL], bf16)
        gruT = big.tile([D, T * BL], bf16)

        ident = pp.tile([128, 128], f32)
        make_identity(nc, ident)
        identB = pp.tile([BL, BL], bf16)
        make_identity(nc, identB)
        identB128 = pp.tile([128, 128], bf16)
        make_identity(nc, identB128)

        # ------------- weights (bf16 arrive pre-converted via DMA) -------
        WihT = wp.tile([D, G3], bf16)
        nc.sync.dma_start(WihT[:, :], WihTd[:, :])
        WhhT = wp.tile([D, G3], bf16)
        nc.sync.dma_start(WhhT[:, :], WhhTd[:, :])
        W0T = wp.tile([D, 240], bf16)
        nc.sync.dma_start(W0T[:, :], W0Td[:, :])
        W1T = wp.tile([80, 40], bf16)
        nc.sync.dma_start(W1T[:, :], W1Td[:, :])
        W2T = wp.tile([40, 1], bf16)
        nc.sync.dma_start(W2T[:, :], W2Td[:, :])
        augW = wp.tile([D, 6 * D], bf16)
        nc.sync.dma_start(augW[:, :], augWd[:, :])
        outWT = wp.tile([D, 5], bf16)
        nc.sync.dma_start(outWT[:, :], outWTd[:, :])
        WrhT, WrxT = augW[:, 0:D], augW[:, D:2 * D]
        WuhT, WuxT = augW[:, 2 * D:3 * D], augW[:, 3 * D:4 * D]
        WahT, WaxT = augW[:, 4 * D:5 * D], augW[:, 5 * D:6 * D]

        bihT = wp.tile([D, 3], f32)
        nc.sync.dma_start(bihT[:, :], bihTd[:, :])
        bhhT = wp.tile([D, 3], f32)
        nc.sync.dma_start(bhhT[:, :], bhhTd[:, :])
        brz = wp.tile([D, 2], f32)
        nc.vector.tensor_add(brz[:, :], bihT[:, 0:2], bhhT[:, 0:2])
        nbz = wp.tile([D, 1], f32)
        nc.vector.tensor_scalar(nbz[:, :], brz[:, 1:2], -1.0, 0.0,
                                ALU.mult, ALU.add)
        b_hn, b_in = bhhT[:, 2:3], bihT[:, 2:3]

        b0v = wp.tile([80, 1], f32)
        nc.sync.dma_start(b0v[:, :], b0d[:, :])
        b1v = wp.tile([40, 1], f32)
        nc.sync.dma_start(b1v[:, :], b1d[:, :])
        b2v = wp.tile([1, 1], f32)
        nc.sync.dma_start(b2v[:, :], b2d[:, :])
        augb = wp.tile([D, 3], f32)
        nc.sync.dma_start(augb[:, :], augbd[:, :])
        outb = wp.tile([1, 1], f32)
        nc.sync.dma_start(outb[:, :], outbd[:, :])
        ab_r, ab_u, ab_h = augb[:, 0:1], augb[:, 1:2], augb[:, 2:3]

        ones_f = wp.tile([1, 128], f32)
        nc.vector.memset(ones_f[:, :], 1.0)
        ones1 = wp.tile([1, 128], bf16)
        nc.vector.memset(ones1[:, :], 1.0)

        # ---------------- preprocessing --------------------------------
        histT = pp.tile([D, BL], bf16)
        qT = pp.tile([D, BL], bf16)
        userT = pp.tile([D, BL], bf16)
        qRep = pp.tile([D, W], bf16)
        maskS = pp.tile([BL, T], f32)
        b2col = pp.tile([BL, 1], f32)
        histF = pp.tile([D, BL], f32)

        with tc.tile_pool(name="pre", bufs=8) as pre, \
             tc.tile_pool(name="pps", bufs=2, space="PSUM") as pps, \
             tc.tile_pool(name="hps", bufs=1, space="PSUM") as hps:
            # X^T arrives pre-transposed from the host; split the DMA so the
            # first GRU chunks can start as soon as their columns land. The
            # bulk halves go on the scalar-engine DGE queue so the sync
            # queue's hist tiles aren't stuck behind them.
            nc.sync.dma_start(XT[:, 0:4 * WG], histTd[:, 0:4 * WG])
            nc.scalar.dma_start(XT[:, 4 * WG:25 * WG],
                                histTd[:, 4 * WG:25 * WG])
            nc.scalar.dma_start(XT[:, 25 * WG:T * BL],
                                histTd[:, 25 * WG:T * BL])

            nc.sync.dma_start(maskS[:, :], maskd[:, :])
            seqi = pre.tile([BL, 1], i32, tag="seqi", bufs=1)
            nc.sync.dma_start(seqi[:, :], seqd[:, :])
            seqf = pre.tile([BL, 1], f32, tag="seqf", bufs=1)
            nc.vector.tensor_copy(seqf[:, :], seqi[:, :])
            rsec = pre.tile([BL, 1], f32, tag="rsec", bufs=1)
            nc.vector.reciprocal(rsec[:, :], seqf[:, :])
            maskSc = pre.tile([BL, T], f32, tag="maskSc", bufs=1)
            nc.vector.tensor_scalar_mul(maskSc[:, :], maskS[:, :], rsec[:, 0:1])

            mstp0 = pps.tile([128, BL], f32, tag="mstp")
            nc.tensor.transpose(mstp0[:, :], maskSc[:, 0:128], ident[0:BL, 0:BL])
            mst0 = pre.tile([128, BL], bf16, tag="mst0", bufs=1)
            nc.scalar.copy(mst0[:, :], mstp0[:, :])
            mstp1 = pps.tile([128, BL], f32, tag="mstp")
            nc.tensor.transpose(mstp1[0:72, :], maskSc[:, 128:200], ident[0:BL, 0:BL])
            mst1 = pre.tile([72, BL], bf16, tag="mst1", bufs=1)
            nc.scalar.copy(mst1[:, :], mstp1[0:72, :])

            itn = pre.tile([BL, D], f32, tag="itn", bufs=2)
            nc.sync.dma_start(itn[:, :], item[:, :])
            itp = pps.tile([D, BL], f32, tag="small_t")
            nc.tensor.transpose(itp[:, :], itn[:, :], ident[0:BL, 0:BL])
            nc.scalar.copy(qT[:, :], itp[:, :])
            usn = pre.tile([BL, D], f32, tag="itn", bufs=2)
            nc.sync.dma_start(usn[:, :], user[:, :])
            usp = pps.tile([D, BL], f32, tag="small_t")
            nc.tensor.transpose(usp[:, :], usn[:, :], ident[0:BL, 0:BL])
            nc.scalar.copy(userT[:, :], usp[:, :])
            b2p = pps.tile([BL, 1], f32, tag="small_t")
            _mm(nc, b2p[:, :], ones_f[0:1, 0:BL], b2v, start=True, stop=True)
            nc.scalar.copy(b2col[:, :], b2p[:, :])
            for s in range(CH):
                nc.scalar.copy(qRep[:, s * BL:(s + 1) * BL], qT[:, :])

            # masked history sum from natural-layout tiles; batched DMAs
            # (8 batch rows per dma_start) while the big X^T DMA streams in.
            histp0 = hps.tile([D, BL], f32, tag="histp0")
            histp1 = hps.tile([D, BL], f32, tag="histp1")
            GB = 8
            for h, (t0, tn) in enumerate(((0, 128), (128, 72))):
                histp = histp0 if h == 0 else histp1
                for b0 in range(0, BL, GB):
                    xn = pre.tile([128, GB * D], bf16, tag="xn", bufs=3)
                    xn3 = xn.rearrange("t (b d) -> t b d", d=D)
                    nc.sync.dma_start(
                        xn3[0:tn, :, :],
                        hist[b0:b0 + GB, t0:t0 + tn, :]
                        .rearrange("b t d -> t b d"))
                    for bi in range(GB):
                        b = b0 + bi
                        mcol = (mst0 if h == 0 else mst1)[0:tn, b:b + 1]
                        _mm(nc, histp[:, b:b + 1],
                            xn3[0:tn, bi, :], mcol, start=True, stop=True)
            nc.scalar.copy(histF[:, :], histp0[:, :])
            nc.vector.tensor_add(histF[:, :], histF[:, :], histp1[:, :])
            nc.scalar.copy(histT[:, :], histF[:, :])

        # ---------------- GRU + interleaved attention -------------------
        gruT3 = gruT.rearrange("p (t b) -> p t b", b=BL)
        SIG, TANH = AF.Sigmoid, AF.Tanh
        wgt = pp.tile([BL, T], f32)

        with tc.tile_pool(name="gip", bufs=2, space="PSUM") as gip, \
             tc.tile_pool(name="hnp", bufs=1, space="PSUM") as hnp, \
             tc.tile_pool(name="gt", bufs=6) as gt, \
             tc.tile_pool(name="aps", bufs=1, space="PSUM") as aps, \
             tc.tile_pool(name="sps", bufs=1, space="PSUM") as sps, \
             tc.tile_pool(name="at", bufs=3) as at:
            scp = sps.tile([BL, T], f32)

            def attention_pieces(a):
                """Attention for gru chunks [2a, 2a+1] as 8 emission slots;
                each slot is small PE/DVE/Pool work sized to fill one GRU
                step's engine-idle window without head-of-line-blocking the
                recurrence chain's FIFO queues."""
                gc = gruT[:, a * W:(a + 1) * W]
                r2 = at.tile([D, W], bf16, tag="r2")
                y0p = aps.tile([80, W], f32, tag="y0")
                y0 = at.tile([80, W], bf16, tag="y0s")
                y1p = aps.tile([40, W], f32, tag="y1")
                y1 = at.tile([40, W], bf16, tag="y1s")
                HW = 256

                def relu0(h, eng):
                    if eng is nc.scalar:
                        nc.scalar.activation(y0[:, h * HW:(h + 1) * HW],
                                             y0p[:, h * HW:(h + 1) * HW],
                                             AF.Relu, bias=b0v[:, 0:1])
                    else:
                        eng.tensor_scalar(y0[:, h * HW:(h + 1) * HW],
                                          y0p[:, h * HW:(h + 1) * HW],
                                          b0v[:, 0:1], 0.0, ALU.add, ALU.max)
                def relu1(h, eng):
                    if eng is nc.scalar:
                        nc.scalar.activation(y1[:, h * HW:(h + 1) * HW],
                                             y1p[:, h * HW:(h + 1) * HW],
                                             AF.Relu, bias=b1v[:, 0:1])
                    else:
                        eng.tensor_scalar(y1[:, h * HW:(h + 1) * HW],
                                          y1p[:, h * HW:(h + 1) * HW],
                                          b1v[:, 0:1], 0.0, ALU.add, ALU.max)
                def score(s):
                    t = a * CH + s
                    _mm(nc, scp[:, t:t + 1], y1[:, s * BL:(s + 1) * BL],
                        W2T, start=True, stop=True)

                def p0():
                    nc.gpsimd.tensor_mul(r2[:, 0:HW], gc[:, 0:HW],
                                         qRep[:, 0:HW])
                def p1():
                    nc.gpsimd.tensor_mul(r2[:, HW:W], gc[:, HW:W],
                                         qRep[:, HW:W])
                    _mm(nc, y0p[:, 0:HW], W0T[:, 0:80], gc[:, 0:HW],
                        start=True, stop=False)
                def p2():
                    _mm(nc, y0p[:, HW:W], W0T[:, 0:80], gc[:, HW:W],
                        start=True, stop=False)
                    _mm(nc, y0p[:, 0:HW], W0T[:, 80:160], qRep[:, 0:HW],
                        start=False, stop=False)
                def p3():
                    _mm(nc, y0p[:, HW:W], W0T[:, 80:160], qRep[:, HW:W],
                        start=False, stop=False)
                    _mm(nc, y0p[:, 0:HW], W0T[:, 160:240], r2[:, 0:HW],
                        start=False, stop=True)
                def p4():
                    _mm(nc, y0p[:, HW:W], W0T[:, 160:240], r2[:, HW:W],
                        start=False, stop=True)
                    relu0(0, nc.scalar)
                def p5():
                    relu0(1, nc.vector)
                    _mm(nc, y1p[:, 0:HW], W1T, y0[:, 0:HW],
                        start=True, stop=True)
                def p6():
                    _mm(nc, y1p[:, HW:W], W1T, y0[:, HW:W],
                        start=True, stop=True)
                    relu1(0, nc.scalar)
                def p7():
                    relu1(1, nc.vector)
                    for s in range(4):
                        score(s)
                def p8():
                    for s in range(4, CH):
                        score(s)
                return [p0, p1, p2, p3, p4, p5, p6, p7, p8]

            pending = []          # attention pieces awaiting emission
            WhT_r, WhT_z, WhT_n = (WhhT[:, 0:D], WhhT[:, D:2 * D],
                                   WhhT[:, 2 * D:G3])
            gslot = [None, None]  # (prz, prz4, pn3) per psum buf

            def gru_proj_pieces(c):
                XTc = XT[:, c * WG:(c + 1) * WG]
                prz = gip.tile([D, CG * 128], f32, tag="girz")
                pn = gip.tile([D, CG * BL], f32, tag="gin")
                gslot[c % 2] = (
                    prz,
                    prz.rearrange("p (g s c) -> p g s c", g=2, c=BL),
                    pn.rearrange("p (s g) -> p s g", g=BL),
                )
                return [
                    lambda: _mm(nc, prz[:, 0:WG], WihT[:, 0:D], XTc,
                                start=True, stop=True),
                    lambda: _mm(nc, prz[:, WG:2 * WG], WihT[:, D:2 * D], XTc,
                                start=True, stop=True),
                    lambda: _mm(nc, pn[:, :], WihT[:, 2 * D:G3], XTc,
                                start=True, stop=True),
                ]

            for p in gru_proj_pieces(0):
                p()
            zh_prev = [None]      # z_{t-1} * h_{t-2}
            omzn_prev = [None]    # (1-z_{t-1}) * n_{t-1}
            for c in range(NCG):
                prz, prz4, pn3 = gslot[c % 2]
                for s in range(CG):
                    t = c * CG + s
                    # --- PE block A: recurrent matmuls feeding step t ----
                    hx = hnp.tile([D, 2 * BL], f32, tag="hn2")
                    hn, tmp2 = hx[:, 0:BL], hx[:, BL:2 * BL]
                    if t == 1:
                        h0 = gruT3[:, 0, :]
                        _mm(nc, prz[:, s * BL:(s + 1) * BL], WhT_r, h0,
                            start=False, stop=True, skip=True)
                        _mm(nc, prz[:, WG + s * BL:WG + (s + 1) * BL],
                            WhT_z, h0, start=False, stop=True, skip=True)
                        _mm(nc, hn[:, :], WhT_n, h0, start=True, stop=True)
                    elif t > 1:
                        rsl = prz[:, s * BL:(s + 1) * BL]
                        _mm(nc, rsl, WhT_r, zh_prev[0],
                            start=False, stop=False, skip=True)
                        _mm(nc, rsl, WhT_r, omzn_prev[0],
                            start=False, stop=True, skip=True)
                        hprev = gruT3[:, t - 1, :]
                        _mm(nc, prz[:, WG + s * BL:WG + (s + 1) * BL],
                            WhT_z, hprev, start=False, stop=True, skip=True)
                        _mm(nc, hn[:, :], WhT_n, hprev, start=True, stop=True)
                    # --- gate activations (biases ride ACT) --------------
                    r = gt.tile([D, BL], bf16, tag="r")
                    nc.scalar.activation(r[:, :], prz4[:, 0, s, :], SIG,
                                         bias=brz[:, 0:1])
                    z = gt.tile([D, BL], bf16, tag="z")
                    nc.scalar.activation(z[:, :], prz4[:, 1, s, :], SIG,
                                         bias=brz[:, 1:2])
                    omz = gt.tile([D, BL], bf16, tag="omz")
                    nc.gpsimd.tensor_scalar(omz[:, :], z[:, :], -1.0, 1.0,
                                            ALU.mult, ALU.add)
                    tmp = gt.tile([D, BL], bf16, tag="tmp")
                    if t > 0:
                        nc.vector.scalar_tensor_tensor(
                            tmp[:, :], hn[:, :], b_hn, r[:, :],
                            ALU.add, ALU.mult)
                    else:
                        nc.vector.tensor_scalar_mul(tmp[:, :], r[:, :], b_hn)
                    nc.vector.tensor_add(tmp2[:, :], tmp[:, :], pn3[:, s, :])
                    if t > 0:
                        zh = gt.tile([D, BL], bf16, tag="zh")
                        nc.vector.tensor_mul(zh[:, :], z[:, :],
                                             gruT3[:, t - 1, :])
                    n = gt.tile([D, BL], bf16, tag="n")
                    nc.scalar.activation(n[:, :], tmp2[:, :], TANH, bias=b_in)
                    hdst = gruT3[:, t, :]
                    if t > 0:
                        omzn = gt.tile([D, BL], bf16, tag="omzn")
                        nc.vector.tensor_mul(omzn[:, :], omz[:, :], n[:, :])
                        nc.vector.tensor_add(hdst, omzn[:, :], zh[:, :])
                        zh_prev[0], omzn_prev[0] = zh, omzn
                    else:
                        nc.vector.tensor_mul(hdst, omz[:, :], n[:, :])
                    # --- PE block B: hoisted chunk/attention work --------
                    if s == 1 and c + 1 < NCG:
                        proj_pieces = gru_proj_pieces(c + 1)
                    if s >= 1 and c + 1 < NCG:
                        proj_pieces[s - 1]()
                    if pending and t % 8 != 7:
                        pending.pop(0)()
                    if pending and t % 8 in (1, 2):
                        pending.pop(0)()
                if c % 2 == 1:
                    pending.extend(attention_pieces(c // 2))
            for p in pending:
                p()

            # masked softmax over t (b-major); last MLP layer ReLU'd w/ b2
            rawr = at.tile([BL, T], f32, tag="rawr")
            nc.scalar.activation(rawr[:, :], scp[:, :], AF.Relu,
                                 bias=b2col[:, 0:1])
            rawm = at.tile([BL, T], f32, tag="rawm")
            nc.vector.tensor_mul(rawm[:, :], rawr[:, :], maskS[:, :])
            mxn = at.tile([BL, 1], f32, tag="mxn")
            nc.vector.tensor_reduce(mxn[:, :], rawm[:, :],
                                    axis=mybir.AxisListType.X,
                                    op=ALU.max, negate=True)
            ex = at.tile([BL, T], f32, tag="ex")
            nc.scalar.activation(ex[:, :], rawr[:, :], AF.Exp, bias=mxn[:, 0:1])
            em = at.tile([BL, T], f32, tag="em")
            nc.vector.tensor_mul(em[:, :], ex[:, :], maskS[:, :])
            sm = at.tile([BL, 1], f32, tag="sm")
            nc.vector.tensor_reduce(sm[:, :], em[:, :],
                                    axis=mybir.AxisListType.X, op=ALU.add)
            rs = at.tile([BL, 1], f32, tag="rs")
            nc.vector.reciprocal(rs[:, :], sm[:, :])
            nc.vector.tensor_scalar_mul(wgt[:, :], em[:, :], rs[:, 0:1])

        # attention weights go to partition-0 bf16 rows per AUGRU chunk
        # (the row transposes + broadcasts overlap the AUGRU itself)
        wgt_bf = pp.tile([BL, T], bf16)
        nc.scalar.copy(wgt_bf[:, :], wgt[:, :])

        # ---------------- AUGRU ----------------------------------------
        hA = [None]
        with tc.tile_pool(name="axp", bufs=2, space="PSUM") as axp, \
             tc.tile_pool(name="abp", bufs=2, space="PSUM") as abp, \
             tc.tile_pool(name="rtp", bufs=2, space="PSUM") as rtp, \
             tc.tile_pool(name="ut", bufs=6) as ut:
            aslot = [None, None]  # (pru, pru4, pxh3, pab) per psum buf

            def aug_proj_pieces(c):
                gc = gruT[:, c * WG:(c + 1) * WG]
                rowp = rtp.tile([1, WG], f32, tag="rowp")
                rows = ut.tile([1, WG], bf16, tag="rows")
                pru = axp.tile([D, CG * 128], f32, tag="pxru")
                pxh = axp.tile([D, CG * BL], f32, tag="pxh")
                pab = abp.tile([D, WG], f32, tag="pab")
                aslot[c % 2] = (
                    pru,
                    pru.rearrange("p (g s c) -> p g s c", g=2, c=BL),
                    pxh.rearrange("p (s g) -> p s g", g=BL),
                    pab,
                )

                def pa():
                    for s in range(CG):
                        t = c * CG + s
                        _mm(nc, rowp[0:1, s * BL:(s + 1) * BL],
                            wgt_bf[:, t:t + 1], identB, start=True, stop=True)
                    nc.scalar.copy(rows[0:1, 0:BL * 2], rowp[0:1, 0:BL * 2])
                def pb():
                    nc.scalar.copy(rows[0:1, BL * 2:WG], rowp[0:1, BL * 2:WG])
                    _mm(nc, pru[:, 0:WG], WrxT, gc, start=True, stop=True)
                    _mm(nc, pab[:, 0:BL], ones1, rows[0:1, 0:BL],
                        start=True, stop=True)
                def pc():
                    _mm(nc, pru[:, WG:2 * WG], WuxT, gc, start=True, stop=True)
                    _mm(nc, pab[:, BL:2 * BL], ones1, rows[0:1, BL:2 * BL],
                        start=True, stop=True)
                def pd():
                    _mm(nc, pxh[:, :], WaxT, gc, start=True, stop=True)
                    _mm(nc, pab[:, 2 * BL:WG], ones1, rows[0:1, 2 * BL:WG],
                        start=True, stop=True)
                return [pa, pb, pc, pd]

            for p in aug_proj_pieces(0):
                p()
            hA_pp = [None]        # hA_{t-2}
            ud_prev = [None]      # ud_{t-1}
            for c in range(NCG):
                pru, pru4, pxh3, pab = aslot[c % 2]
                for s in range(CG):
                    t = c * CG + s
                    hAg = hA[0]
                    # --- PE block A: recurrent matmuls feeding step t ----
                    rsl = pru[:, s * BL:(s + 1) * BL]
                    usl = pru[:, WG + s * BL:WG + (s + 1) * BL]
                    if t == 1:
                        _mm(nc, rsl, WrhT, hAg, start=False, stop=True,
                            skip=True)
                        _mm(nc, usl, WuhT, hAg, start=False, stop=True,
                            skip=True)
                    elif t > 1:
                        _mm(nc, rsl, WrhT, hA_pp[0],
                            start=False, stop=False, skip=True)
                        _mm(nc, rsl, WrhT, ud_prev[0],
                            start=False, stop=True, skip=True)
                        _mm(nc, usl, WuhT, hA_pp[0],
                            start=False, stop=False, skip=True)
                        _mm(nc, usl, WuhT, ud_prev[0],
                            start=False, stop=True, skip=True)
                    r = ut.tile([D, BL], bf16, tag="ar")
                    nc.scalar.activation(r[:, :], pru4[:, 0, s, :], SIG,
                                         bias=ab_r)
                    u = ut.tile([D, BL], bf16, tag="au")
                    nc.scalar.activation(u[:, :], pru4[:, 1, s, :], SIG,
                                         bias=ab_u)
                    if t > 0:
                        hr = ut.tile([D, BL], bf16, tag="ahr")
                        nc.vector.tensor_mul(hr[:, :], hAg, r[:, :])
                        _mm(nc, pxh3[:, s, :], WahT, hr,
                            start=False, stop=True, skip=True)
                    up = ut.tile([D, BL], f32, tag="aup")
                    nc.vector.tensor_mul(up[:, :], pab[:, s * BL:(s + 1) * BL],
                                         u[:, :])
                    hh = ut.tile([D, BL], bf16, tag="ahh")
                    nc.scalar.activation(hh[:, :], pxh3[:, s, :], TANH,
                                         bias=ab_h)
                    hnew = hp.tile([D, BL], bf16, tag="hA")
                    if t > 0:
                        dd = ut.tile([D, BL], bf16, tag="add")
                        nc.vector.tensor_sub(dd[:, :], hh[:, :], hAg)
                        ud = ut.tile([D, BL], bf16, tag="aud")
                        nc.vector.tensor_mul(ud[:, :], up[:, :], dd[:, :])
                        nc.gpsimd.tensor_add(hnew[:, :], hAg, ud[:, :])
                        hA_pp[0], ud_prev[0] = hAg, ud
                    else:
                        nc.vector.tensor_mul(hnew[:, :], up[:, :], hh[:, :])
                    hA[0] = hnew
                    # --- PE block B: hoisted next-chunk work -------------
                    if c + 1 < NCG:
                        if s == 0:
                            aproj_pieces = aug_proj_pieces(c + 1)
                        aproj_pieces[s]()

        # ---------------- output layer ---------------------------------
        with tc.tile_pool(name="ops", bufs=1, space="PSUM") as ops, \
             tc.tile_pool(name="ot", bufs=1) as ot:
            ih = ot.tile([D, BL], bf16)
            nc.vector.tensor_mul(ih[:, :], qT[:, :], histF[:, :])
            po = ops.tile([1, BL], f32)
            pieces = [userT, qT, histT, ih, hA[0]]
            for g, piece in enumerate(pieces):
                _mm(nc, po[:, :], outWT[:, g:g + 1], piece,
                    start=(g == 0), stop=(g == 4))
            outs = ot.tile([1, BL], f32)
            nc.scalar.activation(outs[:, :], po[:, :], AF.Identity,
                                 bias=outb[:, 0:1])
            nc.sync.dma_start(outd[:, :], outs[:, :])

    nc.finalize()
    return nc


_NC = None


def _get_nc():
    global _NC
    if _NC is None:
        _NC = build_nc()
    return _NC


def make_in_maps(inputs):
    """Slice full inputs into per-core input maps (host-side layout only)."""
    f = {k: np.asarray(v) for k, v in inputs.items()}
    WihT = np.ascontiguousarray(f["gru_Wih"].T)          # (128, 384)
    WhhT = np.ascontiguousarray(f["gru_Whh"].T)
    bihT = np.ascontiguousarray(f["gru_bih"].reshape(3, D).T)  # (128, 3)
    bhhT = np.ascontiguousarray(f["gru_bhh"].reshape(3, D).T)
    # fold the (q - fact) block into the fact/q blocks:
    # W0_f@f + W0_q@q + W0_fq@(f*q) + W0_qf@(q-f)
    #   = (W0_f - W0_qf)@f + (W0_q + W0_qf)@q + W0_fq@(f*q)
    W0 = f["attn_W0"]                                    # (80, 4D)
    W0f = W0[:, 0:D] - W0[:, 3 * D:4 * D]
    W0q = W0[:, D:2 * D] + W0[:, 3 * D:4 * D]
    W0fq = W0[:, 2 * D:3 * D]
    W0T = np.ascontiguousarray(
        np.concatenate([W0f.T, W0q.T, W0fq.T], axis=1))  # (D, 240)
    b0 = np.ascontiguousarray(f["attn_b0"].reshape(80, 1))
    W1T = np.ascontiguousarray(f["attn_W1"].T)           # (80, 40)
    b1 = np.ascontiguousarray(f["attn_b1"].reshape(40, 1))
    W2T = np.ascontiguousarray(f["attn_W2"].T)           # (40, 1)
    b2 = f["attn_b2"].reshape(1, 1)
    augW = np.concatenate(
        [np.ascontiguousarray(f[k][:, p * D:(p + 1) * D].T)
         for k in ("aug_Wr", "aug_Wu", "aug_Wh") for p in (0, 1)],
        axis=1)                                          # (128, 768)
    augb = np.stack([f["aug_br"], f["aug_bu"], f["aug_bh"]], axis=1)  # (128,3)
    outWT = np.ascontiguousarray(f["out_W"].reshape(5, D).T)          # (128,5)
    outb = f["out_b"].reshape(1, 1)

    shared_bf = dict(WihT=WihT, WhhT=WhhT, W0T=W0T, W1T=W1T, W2T=W2T,
                     augW=augW, outWT=outWT)
    shared = dict(bihT=bihT, bhhT=bhhT, b0=b0, b1=b1, b2=b2, augb=augb,
                  outb=outb)
    shared = {k: np.ascontiguousarray(v.astype(np.float32)) for k, v in
              shared.items()}
    shared.update({k: np.ascontiguousarray(v.astype(BF)) for k, v in
                   shared_bf.items()})

    in_maps = []
    for c in range(NCORES):
        s = slice(c * BL, (c + 1) * BL)
        m = dict(shared)
        hs = f["item_historical_embedding"][s].astype(BF)
        m["hist"] = np.ascontiguousarray(hs)
        m["histTd"] = np.ascontiguousarray(
            hs.transpose(2, 1, 0).reshape(D, T * BL))
        m["item"] = np.ascontiguousarray(f["item_embedding"][s].astype(np.float32))
        m["user"] = np.ascontiguousarray(f["user_embedding"][s].astype(np.float32))
        m["maskd"] = np.ascontiguousarray(f["mask"][s].astype(np.float32))
        m["seqd"] = np.ascontiguousarray(
            f["sequential_length"][s].reshape(BL, 1).astype(np.int32))
        in_maps.append(m)
    return in_maps


def kernel(**inputs) -> np.ndarray:
    nc = _get_nc()
    in_maps = make_in_maps(inputs)
    res = run_bass_kernel_spmd(nc, in_maps, list(range(NCORES)))
    return np.concatenate(
        [np.asarray(res.results[c]["out"]).reshape(BL) for c in range(NCORES)])


# revision 35
# speedup vs baseline: 1.2833x; 1.0002x over previous
"""DIEN (GRU + attention + AUGRU) Trainium2 kernel.

Data-parallel over 8 NeuronCores: each core handles a batch slice of 64.
All on-chip state is feature-major (feature dim on SBUF partitions, batch
on the free dim), so the two sequential recurrences need no per-step
transposes.

Structure:
  - X^T (D, T*64) in bf16, built via PE transposes of natural-layout DMA
    tiles; the same stationary tiles also accumulate the masked history
    sum (f32) via N=1 matmuls against 1/len-scaled mask columns.
  - GRU input projections land in PSUM chunks laid out (128, 4, [r|z|n]).
    Gate sigmoids are split per-gate so the gate bias rides the ACT
    instruction (no DVE bias adds), and (1-z) comes from sigmoid(-x).
    The combine uses h' = (1-z)*n + z*h with z*h computed off the
    critical path.
  - Attention runs per 8-step chunk interleaved with the GRU loop; its
    elementwise prep (fact*q, q-fact) runs on the idle GPSIMD engine so
    the DVE stays free for the recurrence chain.
  - AUGRU consumes attention weights via rows pre-transposed once after
    the softmax; per chunk a single ones-weight load broadcasts 4 rows
    into PSUM. Masked steps have weight 0 (u'=0, h unchanged), so the
    final AUGRU state IS the gathered aug_out[b, len-1] - no gather.
  - Matmuls are bf16 (f32 PSUM accumulation); elementwise recurrence ops
    run bf16 where both operands sit in SBUF (2x DVE mode); the masked
    history mean and softmax stay f32.
"""

import sys

if "/opt/trn_rl_repo" not in sys.path:
    sys.path.insert(0, "/opt/trn_rl_repo")

from contextlib import ExitStack

import ml_dtypes
import numpy as np

import concourse.bacc as bacc
import concourse.bass as bass
import concourse.mybir as mybir
import concourse.tile as tile
from concourse.bass_utils import run_bass_kernel_spmd
from concourse.masks import make_identity

f32 = mybir.dt.float32
bf16 = mybir.dt.bfloat16
i32 = mybir.dt.int32
AF = mybir.ActivationFunctionType
ALU = mybir.AluOpType
BF = ml_dtypes.bfloat16

NCORES = 8
B, T, D = 512, 200, 128
BL = B // NCORES          # 64 batch rows per core
CG = 4                    # GRU/AUGRU psum chunk: steps per chunk
NCG = T // CG             # 50
WG = CG * BL              # 256
CH = 8                    # attention chunk: steps per chunk
NCHUNK = T // CH          # 25
W = CH * BL               # 512
G3 = 3 * D


def _mm(nc, out, lhsT, rhs, start, stop, skip=False):
    nc.tensor.matmul(out, lhsT, rhs, start=start, stop=stop,
                     skip_group_check=skip)


def build_nc():
    nc = bacc.Bacc("TRN2", target_bir_lowering=False)

    hist = nc.declare_dram_parameter("hist", [BL, T, D], bf16, isOutput=False)
    histTd = nc.declare_dram_parameter("histTd", [D, T * BL], bf16,
                                       isOutput=False)
    item = nc.declare_dram_parameter("item", [BL, D], f32, isOutput=False)
    user = nc.declare_dram_parameter("user", [BL, D], f32, isOutput=False)
    maskd = nc.declare_dram_parameter("maskd", [BL, T], f32, isOutput=False)
    seqd = nc.declare_dram_parameter("seqd", [BL, 1], i32, isOutput=False)
    WihTd = nc.declare_dram_parameter("WihT", [D, G3], bf16, isOutput=False)
    WhhTd = nc.declare_dram_parameter("WhhT", [D, G3], bf16, isOutput=False)
    bihTd = nc.declare_dram_parameter("bihT", [D, 3], f32, isOutput=False)
    bhhTd = nc.declare_dram_parameter("bhhT", [D, 3], f32, isOutput=False)
    W0Td = nc.declare_dram_parameter("W0T", [D, 240], bf16, isOutput=False)
    b0d = nc.declare_dram_parameter("b0", [80, 1], f32, isOutput=False)
    W1Td = nc.declare_dram_parameter("W1T", [80, 40], bf16, isOutput=False)
    b1d = nc.declare_dram_parameter("b1", [40, 1], f32, isOutput=False)
    W2Td = nc.declare_dram_parameter("W2T", [40, 1], bf16, isOutput=False)
    b2d = nc.declare_dram_parameter("b2", [1, 1], f32, isOutput=False)
    augWd = nc.declare_dram_parameter("augW", [D, 6 * D], bf16, isOutput=False)
    augbd = nc.declare_dram_parameter("augb", [D, 3], f32, isOutput=False)
    outWTd = nc.declare_dram_parameter("outWT", [D, 5], bf16, isOutput=False)
    outbd = nc.declare_dram_parameter("outb", [1, 1], f32, isOutput=False)
    outd = nc.declare_dram_parameter("out", [1, BL], f32, isOutput=True)

    with tile.TileContext(nc) as tc, ExitStack() as ctx:
        big = ctx.enter_context(tc.tile_pool(name="big", bufs=1))
        wp = ctx.enter_context(tc.tile_pool(name="wp", bufs=1))
        pp = ctx.enter_context(tc.tile_pool(name="pp", bufs=1))
        hp = ctx.enter_context(tc.tile_pool(name="hp", bufs=4))

        XT = big.tile([D, T * BL], bf16)
        gruT = big.tile([D, T * BL], bf16)

        ident = pp.tile([128, 128], f32)
        make_identity(nc, ident)
        identB = pp.tile([BL, BL], bf16)
        make_identity(nc, identB)
        identB128 = pp.tile([128, 128], bf16)
        make_identity(nc, identB128)

        # ------------- weights (bf16 arrive pre-converted via DMA) -------
        WihT = wp.tile([D, G3], bf16)
        nc.sync.dma_start(WihT[:, :], WihTd[:, :])
        WhhT = wp.tile([D, G3], bf16)
        nc.sync.dma_start(WhhT[:, :], WhhTd[:, :])
        W0T = wp.tile([D, 240], bf16)
        nc.sync.dma_start(W0T[:, :], W0Td[:, :])
        W1T = wp.tile([80, 40], bf16)
        nc.sync.dma_start(W1T[:, :], W1Td[:, :])
        W2T = wp.tile([40, 1], bf16)
        nc.sync.dma_start(W2T[:, :], W2Td[:, :])
        augW = wp.tile([D, 6 * D], bf16)
        nc.sync.dma_start(augW[:, :], augWd[:, :])
        outWT = wp.tile([D, 5], bf16)
        nc.sync.dma_start(outWT[:, :], outWTd[:, :])
        WrhT, WrxT = augW[:, 0:D], augW[:, D:2 * D]
        WuhT, WuxT = augW[:, 2 * D:3 * D], augW[:, 3 * D:4 * D]
        WahT, WaxT = augW[:, 4 * D:5 * D], augW[:, 5 * D:6 * D]

        bihT = wp.tile([D, 3], f32)
        nc.sync.dma_start(bihT[:, :], bihTd[:, :])
        bhhT = wp.tile([D, 3], f32)
        nc.sync.dma_start(bhhT[:, :], bhhTd[:, :])
        brz = wp.tile([D, 2], f32)
        nc.vector.tensor_add(brz[:, :], bihT[:, 0:2], bhhT[:, 0:2])
        nbz = wp.tile([D, 1], f32)
        nc.vector.tensor_scalar(nbz[:, :], brz[:, 1:2], -1.0, 0.0,
                                ALU.mult, ALU.add)
        b_hn, b_in = bhhT[:, 2:3], bihT[:, 2:3]

        b0v = wp.tile([80, 1], f32)
        nc.sync.dma_start(b0v[:, :], b0d[:, :])
        b1v = wp.tile([40, 1], f32)
        nc.sync.dma_start(b1v[:, :], b1d[:, :])
        b2v = wp.tile([1, 1], f32)
        nc.sync.dma_start(b2v[:, :], b2d[:, :])
        augb = wp.tile([D, 3], f32)
        nc.sync.dma_start(augb[:, :], augbd[:, :])
        outb = wp.tile([1, 1], f32)
        nc.sync.dma_start(outb[:, :], outbd[:, :])
        ab_r, ab_u, ab_h = augb[:, 0:1], augb[:, 1:2], augb[:, 2:3]

        ones_f = wp.tile([1, 128], f32)
        nc.vector.memset(ones_f[:, :], 1.0)
        ones1 = wp.tile([1, 128], bf16)
        nc.vector.memset(ones1[:, :], 1.0)

        # ---------------- preprocessing --------------------------------
        histT = pp.tile([D, BL], bf16)
        qT = pp.tile([D, BL], bf16)
        userT = pp.tile([D, BL], bf16)
        qRep = pp.tile([D, W], bf16)
        maskS = pp.tile([BL, T], f32)
        b2col = pp.tile([BL, 1], f32)
        histF = pp.tile([D, BL], f32)

        with tc.tile_pool(name="pre", bufs=8) as pre, \
             tc.tile_pool(name="pps", bufs=2, space="PSUM") as pps, \
             tc.tile_pool(name="hps", bufs=1, space="PSUM") as hps:
            # X^T arrives pre-transposed from the host; split the DMA so the
            # first GRU chunks can start as soon as their columns land. The
            # bulk halves go on the scalar-engine DGE queue so the sync
            # queue's hist tiles aren't stuck behind them.
            nc.sync.dma_start(XT[:, 0:4 * WG], histTd[:, 0:4 * WG])
            nc.scalar.dma_start(XT[:, 4 * WG:25 * WG],
                                histTd[:, 4 * WG:25 * WG])
            nc.scalar.dma_start(XT[:, 25 * WG:T * BL],
                                histTd[:, 25 * WG:T * BL])

            nc.sync.dma_start(maskS[:, :], maskd[:, :])
            seqi = pre.tile([BL, 1], i32, tag="seqi", bufs=1)
            nc.sync.dma_start(seqi[:, :], seqd[:, :])
            seqf = pre.tile([BL, 1], f32, tag="seqf", bufs=1)
            nc.vector.tensor_copy(seqf[:, :], seqi[:, :])
            rsec = pre.tile([BL, 1], f32, tag="rsec", bufs=1)
            nc.vector.reciprocal(rsec[:, :], seqf[:, :])
            maskSc = pre.tile([BL, T], f32, tag="maskSc", bufs=1)
            nc.vector.tensor_scalar_mul(maskSc[:, :], maskS[:, :], rsec[:, 0:1])

            mstp0 = pps.tile([128, BL], f32, tag="mstp")
            nc.tensor.transpose(mstp0[:, :], maskSc[:, 0:128], ident[0:BL, 0:BL])
            mst0 = pre.tile([128, BL], bf16, tag="mst0", bufs=1)
            nc.scalar.copy(mst0[:, :], mstp0[:, :])
            mstp1 = pps.tile([128, BL], f32, tag="mstp")
            nc.tensor.transpose(mstp1[0:72, :], maskSc[:, 128:200], ident[0:BL, 0:BL])
            mst1 = pre.tile([72, BL], bf16, tag="mst1", bufs=1)
            nc.scalar.copy(mst1[:, :], mstp1[0:72, :])

            itn = pre.tile([BL, D], f32, tag="itn", bufs=2)
            nc.sync.dma_start(itn[:, :], item[:, :])
            itp = pps.tile([D, BL], f32, tag="small_t")
            nc.tensor.transpose(itp[:, :], itn[:, :], ident[0:BL, 0:BL])
            nc.scalar.copy(qT[:, :], itp[:, :])
            usn = pre.tile([BL, D], f32, tag="itn", bufs=2)
            nc.sync.dma_start(usn[:, :], user[:, :])
            usp = pps.tile([D, BL], f32, tag="small_t")
            nc.tensor.transpose(usp[:, :], usn[:, :], ident[0:BL, 0:BL])
            nc.scalar.copy(userT[:, :], usp[:, :])
            b2p = pps.tile([BL, 1], f32, tag="small_t")
            _mm(nc, b2p[:, :], ones_f[0:1, 0:BL], b2v, start=True, stop=True)
            nc.scalar.copy(b2col[:, :], b2p[:, :])
            for s in range(CH):
                nc.scalar.copy(qRep[:, s * BL:(s + 1) * BL], qT[:, :])

            # masked history sum from natural-layout tiles; batched DMAs
            # (8 batch rows per dma_start) while the big X^T DMA streams in.
            histp0 = hps.tile([D, BL], f32, tag="histp0")
            histp1 = hps.tile([D, BL], f32, tag="histp1")
            GB = 8
            for h, (t0, tn) in enumerate(((0, 128), (128, 72))):
                histp = histp0 if h == 0 else histp1
                for b0 in range(0, BL, GB):
                    xn = pre.tile([128, GB * D], bf16, tag="xn", bufs=3)
                    xn3 = xn.rearrange("t (b d) -> t b d", d=D)
                    nc.sync.dma_start(
                        xn3[0:tn, :, :],
                        hist[b0:b0 + GB, t0:t0 + tn, :]
                        .rearrange("b t d -> t b d"))
                    for bi in range(GB):
                        b = b0 + bi
                        mcol = (mst0 if h == 0 else mst1)[0:tn, b:b + 1]
                        _mm(nc, histp[:, b:b + 1],
                            xn3[0:tn, bi, :], mcol, start=True, stop=True)
            nc.scalar.copy(histF[:, :], histp0[:, :])
            nc.vector.tensor_add(histF[:, :], histF[:, :], histp1[:, :])
            nc.scalar.copy(histT[:, :], histF[:, :])

        # ---------------- GRU + interleaved attention -------------------
        gruT3 = gruT.rearrange("p (t b) -> p t b", b=BL)
        SIG, TANH = AF.Sigmoid, AF.Tanh
        wgt = pp.tile([BL, T], f32)

        with tc.tile_pool(name="gip", bufs=2, space="PSUM") as gip, \
             tc.tile_pool(name="hnp", bufs=1, space="PSUM") as hnp, \
             tc.tile_pool(name="gt", bufs=6) as gt, \
             tc.tile_pool(name="aps", bufs=1, space="PSUM") as aps, \
             tc.tile_pool(name="sps", bufs=1, space="PSUM") as sps, \
             tc.tile_pool(name="at", bufs=3) as at:
            scp = sps.tile([BL, T], f32)

            def attention_pieces(a):
                """Attention for gru chunks [2a, 2a+1] as 8 emission slots;
                each slot is small PE/DVE/Pool work sized to fill one GRU
                step's engine-idle window without head-of-line-blocking the
                recurrence chain's FIFO queues."""
                gc = gruT[:, a * W:(a + 1) * W]
                r2 = at.tile([D, W], bf16, tag="r2")
                y0p = aps.tile([80, W], f32, tag="y0")
                y0 = at.tile([80, W], bf16, tag="y0s")
                y1p = aps.tile([40, W], f32, tag="y1")
                y1 = at.tile([40, W], bf16, tag="y1s")
                HW = 256

                def relu0(h, eng):
                    if eng is nc.scalar:
                        nc.scalar.activation(y0[:, h * HW:(h + 1) * HW],
                                             y0p[:, h * HW:(h + 1) * HW],
                                             AF.Relu, bias=b0v[:, 0:1])
                    else:
                        eng.tensor_scalar(y0[:, h * HW:(h + 1) * HW],
                                          y0p[:, h * HW:(h + 1) * HW],
                                          b0v[:, 0:1], 0.0, ALU.add, ALU.max)
                def relu1(h, eng):
                    if eng is nc.scalar:
                        nc.scalar.activation(y1[:, h * HW:(h + 1) * HW],
                                             y1p[:, h * HW:(h + 1) * HW],
                                             AF.Relu, bias=b1v[:, 0:1])
                    else:
                        eng.tensor_scalar(y1[:, h * HW:(h + 1) * HW],
                                          y1p[:, h * HW:(h + 1) * HW],
                                          b1v[:, 0:1], 0.0, ALU.add, ALU.max)
                def score(s):
                    t = a * CH + s
                    _mm(nc, scp[:, t:t + 1], y1[:, s * BL:(s + 1) * BL],
                        W2T, start=True, stop=True)

                def p0():
                    nc.gpsimd.tensor_mul(r2[:, 0:HW], gc[:, 0:HW],
                                         qRep[:, 0:HW])
                def p1():
                    nc.gpsimd.tensor_mul(r2[:, HW:W], gc[:, HW:W],
                                         qRep[:, HW:W])
                    _mm(nc, y0p[:, 0:HW], W0T[:, 0:80], gc[:, 0:HW],
                        start=True, stop=False)
                def p2():
                    _mm(nc, y0p[:, HW:W], W0T[:, 0:80], gc[:, HW:W],
                        start=True, stop=False)
                    _mm(nc, y0p[:, 0:HW], W0T[:, 80:160], qRep[:, 0:HW],
                        start=False, stop=False)
                def p3():
                    _mm(nc, y0p[:, HW:W], W0T[:, 80:160], qRep[:, HW:W],
                        start=False, stop=False)
                    _mm(nc, y0p[:, 0:HW], W0T[:, 160:240], r2[:, 0:HW],
                        start=False, stop=True)
                def p4():
                    _mm(nc, y0p[:, HW:W], W0T[:, 160:240], r2[:, HW:W],
                        start=False, stop=True)
                    relu0(0, nc.scalar)
                def p5():
                    relu0(1, nc.vector)
                    _mm(nc, y1p[:, 0:HW], W1T, y0[:, 0:HW],
                        start=True, stop=True)
                def p6():
                    _mm(nc, y1p[:, HW:W], W1T, y0[:, HW:W],
                        start=True, stop=True)
                    relu1(0, nc.scalar)
                def p7():
                    relu1(1, nc.vector)
                    for s in range(4):
                        score(s)
                def p8():
                    for s in range(4, CH):
                        score(s)
                return [p0, p1, p2, p3, p4, p5, p6, p7, p8]

            pending = []          # attention pieces awaiting emission
            WhT_r, WhT_z, WhT_n = (WhhT[:, 0:D], WhhT[:, D:2 * D],
                                   WhhT[:, 2 * D:G3])
            gslot = [None, None]  # (prz, prz4, pn3) per psum buf

            def gru_proj_pieces(c):
                XTc = XT[:, c * WG:(c + 1) * WG]
                prz = gip.tile([D, CG * 128], f32, tag="girz")
                pn = gip.tile([D, CG * BL], f32, tag="gin")
                gslot[c % 2] = (
                    prz,
                    prz.rearrange("p (g s c) -> p g s c", g=2, c=BL),
                    pn.rearrange("p (s g) -> p s g", g=BL),
                )
                return [
                    lambda: _mm(nc, prz[:, 0:WG], WihT[:, 0:D], XTc,
                                start=True, stop=True),
                    lambda: _mm(nc, prz[:, WG:2 * WG], WihT[:, D:2 * D], XTc,
                                start=True, stop=True),
                    lambda: _mm(nc, pn[:, :], WihT[:, 2 * D:G3], XTc,
                                start=True, stop=True),
                ]

            for p in gru_proj_pieces(0):
                p()
            zh_prev = [None]      # z_{t-1} * h_{t-2}
            omzn_prev = [None]    # (1-z_{t-1}) * n_{t-1}
            for c in range(NCG):
                prz, prz4, pn3 = gslot[c % 2]
                for s in range(CG):
                    t = c * CG + s
                    # --- PE block A: recurrent matmuls feeding step t ----
                    hx = hnp.tile([D, 2 * BL], f32, tag="hn2")
                    hn, tmp2 = hx[:, 0:BL], hx[:, BL:2 * BL]
                    if t == 1:
                        h0 = gruT3[:, 0, :]
                        _mm(nc, prz[:, s * BL:(s + 1) * BL], WhT_r, h0,
                            start=False, stop=True, skip=True)
                        _mm(nc, prz[:, WG + s * BL:WG + (s + 1) * BL],
                            WhT_z, h0, start=False, stop=True, skip=True)
                        _mm(nc, hn[:, :], WhT_n, h0, start=True, stop=True)
                    elif t > 1:
                        rsl = prz[:, s * BL:(s + 1) * BL]
                        _mm(nc, rsl, WhT_r, zh_prev[0],
                            start=False, stop=False, skip=True)
                        _mm(nc, rsl, WhT_r, omzn_prev[0],
                            start=False, stop=True, skip=True)
                        hprev = gruT3[:, t - 1, :]
                        _mm(nc, prz[:, WG + s * BL:WG + (s + 1) * BL],
                            WhT_z, hprev, start=False, stop=True, skip=True)
                        _mm(nc, hn[:, :], WhT_n, hprev, start=True, stop=True)
                    # --- gate activations (biases ride ACT) --------------
                    r = gt.tile([D, BL], bf16, tag="r")
                    nc.scalar.activation(r[:, :], prz4[:, 0, s, :], SIG,
                                         bias=brz[:, 0:1])
                    z = gt.tile([D, BL], bf16, tag="z")
                    nc.scalar.activation(z[:, :], prz4[:, 1, s, :], SIG,
                                         bias=brz[:, 1:2])
                    omz = gt.tile([D, BL], bf16, tag="omz")
                    nc.gpsimd.tensor_scalar(omz[:, :], z[:, :], -1.0, 1.0,
                                            ALU.mult, ALU.add)
                    tmp = gt.tile([D, BL], bf16, tag="tmp")
                    if t > 0:
                        nc.vector.scalar_tensor_tensor(
                            tmp[:, :], hn[:, :], b_hn, r[:, :],
                            ALU.add, ALU.mult)
                    else:
                        nc.vector.tensor_scalar_mul(tmp[:, :], r[:, :], b_hn)
                    nc.vector.tensor_add(tmp2[:, :], tmp[:, :], pn3[:, s, :])
                    if t > 0:
                        zh = gt.tile([D, BL], bf16, tag="zh")
                        nc.vector.tensor_mul(zh[:, :], z[:, :],
                                             gruT3[:, t - 1, :])
                    n = gt.tile([D, BL], bf16, tag="n")
                    nc.scalar.activation(n[:, :], tmp2[:, :], TANH, bias=b_in)
                    hdst = gruT3[:, t, :]
                    if t > 0:
                        omzn = gt.tile([D, BL], bf16, tag="omzn")
                        nc.vector.tensor_mul(omzn[:, :], omz[:, :], n[:, :])
                        nc.vector.tensor_add(hdst, omzn[:, :], zh[:, :])
                        zh_prev[0], omzn_prev[0] = zh, omzn
                    else:
                        nc.vector.tensor_mul(hdst, omz[:, :], n[:, :])
                    # --- PE block B: hoisted chunk/attention work --------
                    if s == 1 and c + 1 < NCG:
                        proj_pieces = gru_proj_pieces(c + 1)
                    if s >= 1 and c + 1 < NCG:
                        proj_pieces[s - 1]()
                    if pending and t % 8 != 7:
                        pending.pop(0)()
                    if pending and t % 8 in (1, 2):
                        pending.pop(0)()
                if c % 2 == 1:
                    pending.extend(attention_pieces(c // 2))
            for p in pending:
                p()

            # masked softmax over t (b-major); last MLP layer ReLU'd w/ b2
            rawr = at.tile([BL, T], f32, tag="rawr")
            nc.scalar.activation(rawr[:, :], scp[:, :], AF.Relu,
                                 bias=b2col[:, 0:1])
            rawm = at.tile([BL, T], f32, tag="rawm")
            nc.vector.tensor_mul(rawm[:, :], rawr[:, :], maskS[:, :])
            mxn = at.tile([BL, 1], f32, tag="mxn")
            nc.vector.tensor_reduce(mxn[:, :], rawm[:, :],
                                    axis=mybir.AxisListType.X,
                                    op=ALU.max, negate=True)
            ex = at.tile([BL, T], f32, tag="ex")
            nc.scalar.activation(ex[:, :], rawr[:, :], AF.Exp, bias=mxn[:, 0:1])
            em = at.tile([BL, T], f32, tag="em")
            nc.vector.tensor_mul(em[:, :], ex[:, :], maskS[:, :])
            sm = at.tile([BL, 1], f32, tag="sm")
            nc.vector.tensor_reduce(sm[:, :], em[:, :],
                                    axis=mybir.AxisListType.X, op=ALU.add)
            rs = at.tile([BL, 1], f32, tag="rs")
            nc.vector.reciprocal(rs[:, :], sm[:, :])
            nc.vector.tensor_scalar_mul(wgt[:, :], em[:, :], rs[:, 0:1])

        # attention weights go to partition-0 bf16 rows per AUGRU chunk
        # (the row transposes + broadcasts overlap the AUGRU itself)
        wgt_bf = pp.tile([BL, T], bf16)
        nc.scalar.copy(wgt_bf[:, :], wgt[:, :])

        # ---------------- AUGRU ----------------------------------------
        hA = [None]
        with tc.tile_pool(name="axp", bufs=2, space="PSUM") as axp, \
             tc.tile_pool(name="abp", bufs=2, space="PSUM") as abp, \
             tc.tile_pool(name="rtp", bufs=2, space="PSUM") as rtp, \
             tc.tile_pool(name="ut", bufs=6) as ut:
            aslot = [None, None]  # (pru, pru4, pxh3, pab) per psum buf

            def aug_proj_pieces(c):
                gc = gruT[:, c * WG:(c + 1) * WG]
                rowp = rtp.tile([1, WG], f32, tag="rowp")
                rows = ut.tile([1, WG], bf16, tag="rows")
                pru = axp.tile([D, CG * 128], f32, tag="pxru")
                pxh = axp.tile([D, CG * BL], f32, tag="pxh")
                pab = abp.tile([D, WG], f32, tag="pab")
                aslot[c % 2] = (
                    pru,
                    pru.rearrange("p (g s c) -> p g s c", g=2, c=BL),
                    pxh.rearrange("p (s g) -> p s g", g=BL),
                    pab,
                )

                def rowmm(s):
                    t = c * CG + s
                    _mm(nc, rowp[0:1, s * BL:(s + 1) * BL],
                        wgt_bf[:, t:t + 1], identB, start=True, stop=True)

                def pa():
                    rowmm(0)
                    rowmm(1)
                    nc.scalar.copy(rows[0:1, 0:BL * 2], rowp[0:1, 0:BL * 2])
                def pb():
                    rowmm(2)
                    rowmm(3)
                    _mm(nc, pru[:, 0:WG], WrxT, gc, start=True, stop=True)
                    _mm(nc, pab[:, 0:BL], ones1, rows[0:1, 0:BL],
                        start=True, stop=True)
                def pc():
                    nc.scalar.copy(rows[0:1, BL * 2:WG], rowp[0:1, BL * 2:WG])
                    _mm(nc, pru[:, WG:2 * WG], WuxT, gc, start=True, stop=True)
                    _mm(nc, pab[:, BL:2 * BL], ones1, rows[0:1, BL:2 * BL],
                        start=True, stop=True)
                def pd():
                    _mm(nc, pxh[:, :], WaxT, gc, start=True, stop=True)
                    _mm(nc, pab[:, 2 * BL:WG], ones1, rows[0:1, 2 * BL:WG],
                        start=True, stop=True)
                return [pa, pb, pc, pd]

            for p in aug_proj_pieces(0):
                p()
            hA_pp = [None]        # hA_{t-2}
            ud_prev = [None]      # ud_{t-1}
            for c in range(NCG):
                pru, pru4, pxh3, pab = aslot[c % 2]
                for s in range(CG):
                    t = c * CG + s
                    hAg = hA[0]
                    # --- PE block A: recurrent matmuls feeding step t ----
                    rsl = pru[:, s * BL:(s + 1) * BL]
                    usl = pru[:, WG + s * BL:WG + (s + 1) * BL]
                    if t == 1:
                        _mm(nc, rsl, WrhT, hAg, start=False, stop=True,
                            skip=True)
                        _mm(nc, usl, WuhT, hAg, start=False, stop=True,
                            skip=True)
                    elif t > 1:
                        _mm(nc, rsl, WrhT, hA_pp[0],
                            start=False, stop=False, skip=True)
                        _mm(nc, rsl, WrhT, ud_prev[0],
                            start=False, stop=True, skip=True)
                        _mm(nc, usl, WuhT, hA_pp[0],
                            start=False, stop=False, skip=True)
                        _mm(nc, usl, WuhT, ud_prev[0],
                            start=False, stop=True, skip=True)
                    r = ut.tile([D, BL], bf16, tag="ar")
                    nc.scalar.activation(r[:, :], pru4[:, 0, s, :], SIG,
                                         bias=ab_r)
                    u = ut.tile([D, BL], bf16, tag="au")
                    nc.scalar.activation(u[:, :], pru4[:, 1, s, :], SIG,
                                         bias=ab_u)
                    if t > 0:
                        hr = ut.tile([D, BL], bf16, tag="ahr")
                        nc.vector.tensor_mul(hr[:, :], hAg, r[:, :])
                        _mm(nc, pxh3[:, s, :], WahT, hr,
                            start=False, stop=True, skip=True)
                    up = ut.tile([D, BL], f32, tag="aup")
                    nc.vector.tensor_mul(up[:, :], pab[:, s * BL:(s + 1) * BL],
                                         u[:, :])
                    hh = ut.tile([D, BL], bf16, tag="ahh")
                    nc.scalar.activation(hh[:, :], pxh3[:, s, :], TANH,
                                         bias=ab_h)
                    hnew = hp.tile([D, BL], bf16, tag="hA")
                    if t > 0:
                        dd = ut.tile([D, BL], bf16, tag="add")
                        nc.vector.tensor_sub(dd[:, :], hh[:, :], hAg)
                        ud = ut.tile([D, BL], bf16, tag="aud")
                        nc.vector.tensor_mul(ud[:, :], up[:, :], dd[:, :])
                        nc.gpsimd.tensor_add(hnew[:, :], hAg, ud[:, :])
                        hA_pp[0], ud_prev[0] = hAg, ud
                    else:
                        nc.vector.tensor_mul(hnew[:, :], up[:, :], hh[:, :])
                    hA[0] = hnew
                    # --- PE block B: hoisted next-chunk work -------------
                    if c + 1 < NCG:
                        if s == 0:
                            aproj_pieces = aug_proj_pieces(c + 1)
                        aproj_pieces[s]()

        # ---------------- output layer ---------------------------------
        with tc.tile_pool(name="ops", bufs=1, space="PSUM") as ops, \
             tc.tile_pool(name="ot", bufs=1) as ot:
            ih = ot.tile([D, BL], bf16)
            nc.vector.tensor_mul(ih[:, :], qT[:, :], histF[:, :])
            po = ops.tile([1, BL], f32)
            pieces = [userT, qT, histT, ih, hA[0]]
            for g, piece in enumerate(pieces):
                _mm(nc, po[:, :], outWT[:, g:g + 1], piece,
                    start=(g == 0), stop=(g == 4))
            outs = ot.tile([1, BL], f32)
            nc.scalar.activation(outs[:, :], po[:, :], AF.Identity,
                                 bias=outb[:, 0:1])
            nc.sync.dma_start(outd[:, :], outs[:, :])

    nc.finalize()
    return nc


_NC = None


def _get_nc():
    global _NC
    if _NC is None:
        _NC = build_nc()
    return _NC


def make_in_maps(inputs):
    """Slice full inputs into per-core input maps (host-side layout only)."""
    f = {k: np.asarray(v) for k, v in inputs.items()}
    WihT = np.ascontiguousarray(f["gru_Wih"].T)          # (128, 384)
    WhhT = np.ascontiguousarray(f["gru_Whh"].T)
    bihT = np.ascontiguousarray(f["gru_bih"].reshape(3, D).T)  # (128, 3)
    bhhT = np.ascontiguousarray(f["gru_bhh"].reshape(3, D).T)
    # fold the (q - fact) block into the fact/q blocks:
    # W0_f@f + W0_q@q + W0_fq@(f*q) + W0_qf@(q-f)
    #   = (W0_f - W0_qf)@f + (W0_q + W0_qf)@q + W0_fq@(f*q)
    W0 = f["attn_W0"]                                    # (80, 4D)
    W0f = W0[:, 0:D] - W0[:, 3 * D:4 * D]
    W0q = W0[:, D:2 * D] + W0[:, 3 * D:4 * D]
    W0fq = W0[:, 2 * D:3 * D]
    W0T = np.ascontiguousarray(
        np.concatenate([W0f.T, W0q.T, W0fq.T], axis=1))  # (D, 240)
    b0 = np.ascontiguousarray(f["attn_b0"].reshape(80, 1))
    W1T = np.ascontiguousarray(f["attn_W1"].T)           # (80, 40)
    b1 = np.ascontiguousarray(f["attn_b1"].reshape(40, 1))
    W2T = np.ascontiguousarray(f["attn_W2"].T)           # (40, 1)
    b2 = f["attn_b2"].reshape(1, 1)
    augW = np.concatenate(
        [np.ascontiguousarray(f[k][:, p * D:(p + 1) * D].T)
         for k in ("aug_Wr", "aug_Wu", "aug_Wh") for p in (0, 1)],
        axis=1)                                          # (128, 768)
    augb = np.stack([f["aug_br"], f["aug_bu"], f["aug_bh"]], axis=1)  # (128,3)
    outWT = np.ascontiguousarray(f["out_W"].reshape(5, D).T)          # (128,5)
    outb = f["out_b"].reshape(1, 1)

    shared_bf = dict(WihT=WihT, WhhT=WhhT, W0T=W0T, W1T=W1T, W2T=W2T,
                     augW=augW, outWT=outWT)
    shared = dict(bihT=bihT, bhhT=bhhT, b0=b0, b1=b1, b2=b2, augb=augb,
                  outb=outb)
    shared = {k: np.ascontiguousarray(v.astype(np.float32)) for k, v in
              shared.items()}
    shared.update({k: np.ascontiguousarray(v.astype(BF)) for k, v in
                   shared_bf.items()})

    in_maps = []
    for c in range(NCORES):
        s = slice(c * BL, (c + 1) * BL)
        m = dict(shared)
        hs = f["item_historical_embedding"][s].astype(BF)
        m["hist"] = np.ascontiguousarray(hs)
        m["histTd"] = np.ascontiguousarray(
            hs.transpose(2, 1, 0).reshape(D, T * BL))
        m["item"] = np.ascontiguousarray(f["item_embedding"][s].astype(np.float32))
        m["user"] = np.ascontiguousarray(f["user_embedding"][s].astype(np.float32))
        m["maskd"] = np.ascontiguousarray(f["mask"][s].astype(np.float32))
        m["seqd"] = np.ascontiguousarray(
            f["sequential_length"][s].reshape(BL, 1).astype(np.int32))
        in_maps.append(m)
    return in_maps


def kernel(**inputs) -> np.ndarray:
    nc = _get_nc()
    in_maps = make_in_maps(inputs)
    res = run_bass_kernel_spmd(nc, in_maps, list(range(NCORES)))
    return np.concatenate(
        [np.asarray(res.results[c]["out"]).reshape(BL) for c in range(NCORES)])
